# revision 55
# baseline (speedup 1.0000x reference)
"""CantorMultiheadFusion kernel for 8 Trainium2 NeuronCores.

Math: out = x + A @ x @ (W_in @ W_out) + b_out, where A is the (S,S) sparse
fusion matrix with A[s, routes[s,k]] += fusion_weights[s,k].

Fast path (v2): the Cantor routing tables make A massively degenerate — the
whole sequence has only ~353 DISTINCT rows (output positions sharing an
identical (routes, weights) pattern), and those rows touch only ~1.4K distinct
source positions. Each core therefore computes only the unique fused rows:

    Zc = (U^T X)^T @ Wc        U: [nr, nu] unique-row table (nu <= 128)
                               X: [nr, D]  the used source rows of x
                               Wc = W_in @ W_out

sharded (batch b x unique-group g) over 8 cores, with the uniques split into
4 groups ordered by source-row locality so per-core (nu, nr) stay small. The
host expands Zc back to the full (B, S, D) output with a pure gather and adds
the residual x + b_out in fp32 (the same class of host epilogue work the v1
path did when assembling its xrb residual tensor and transposed output).

Per-core HBM traffic is ~1MB (vs ~6MB for v1): xs+U^T packed into one wide
bf16 tensor, Wc bf16, and a [nu, D] bf16 result. On-device math is bf16 with
fp32 PSUM accumulation.

The v1 kernel (dense block-matmul on A^T) is kept as a fallback for routing
tables without enough structure (e.g. uniform-random routes).
"""

import numpy as np
import ml_dtypes

B, S, D, K = 2, 4096, 512, 32
NCORES = 8
QROWS = S // 4  # rows per core = 1024
DBLK = D // 128  # 4
KBLK = S // 128  # 32
NGRP = 4  # unique-row groups per batch (v2)

_bf16 = ml_dtypes.bfloat16

_cache = {}


FUSED_NK_MAX = 8

# v2 tuning knobs
V2_WARMUP = 6  # PE p-state warm-up matmuls
V2_FILL = 0  # PE keep-busy matmuls between phase 1 and phase 2


def _build_module_v2(rblocks, nu):
    """v2 module. Inputs per core:

    - xu [sum(rblocks), 512 + nu] bf16: used x rows (cols 0:512) and U^T
      (cols 512:512+nu), zero-padded.
    - wcd [512, 1024] bf16: Wc packed 2-up: row r holds Wc rows r (cols
      0:512) and r+256 (cols 512:1024)... actually packed as two DMA tiles
      [128, 1024] with d-blocks (2j, 2j+1).
    - zc [nu, 512] bf16 out: the unique fused+projected rows.

    Phase 1 (per r-block i): psZT[j][d, u] += xs_i[:, dblk j]^T @ ut_i
    Phase 2: psZC[u, e] += zt_j^T... matmul(lhsT=zt_j [d,u], rhs=wc_j [d,e]).
    """
    import concourse.mybir as mybir
    import concourse.tile as tile
    from concourse import bacc

    f32 = mybir.dt.float32
    bf16 = mybir.dt.bfloat16

    nrb = len(rblocks)
    W = 512 + nu

    # pack the r-blocks 2-up into DMA tiles: tile t holds r-blocks 2t (cols
    # 0:W) and 2t+1 (cols W:2W) so each load moves ~300KB in one descriptor
    # batch (HWDGE descriptor generation is the scarce resource, ~630ns per
    # DMA instruction, shared across all queues).
    dtiles = []  # (partitions, [r-block indices])
    i = 0
    while i < nrb:
        if i + 1 < nrb:
            dtiles.append((max(rblocks[i], rblocks[i + 1]), [i, i + 1]))
            i += 2
        else:
            dtiles.append((rblocks[i], [i]))
            i += 1

    nc = bacc.Bacc("TRN2", target_bir_lowering=True)

    xud = nc.dram_tensor(
        "xud", [sum(p for p, _ in dtiles), 2 * W], bf16, kind="ExternalInput"
    )
    # wc 2-up packed: row r of tile t holds Wc rows (2t)*128+r | (2t+1)*128+r
    wcd = nc.dram_tensor("wcd", [256, 1024], bf16, kind="ExternalInput")
    zc = nc.dram_tensor("zc", [nu, D], bf16, kind="ExternalOutput")

    with tile.TileContext(nc) as tc:
        with (
            tc.tile_pool(name="const", bufs=1) as cpool,
            tc.tile_pool(name="work", bufs=1) as wpool,
            tc.tile_pool(name="psum", bufs=1, space="PSUM") as ppool,
        ):
            # PE p-state warm-up on a memset tile (no DMA dependency).
            wu = cpool.tile([128, 128], bf16, tag="wu")
            nc.gpsimd.memset(wu, 0.0)
            ps_w = ppool.tile([128, 512], f32, tag="ps_w")
            for _ in range(V2_WARMUP):
                nc.tensor.matmul(ps_w[:, :128], wu, wu, start=True, stop=True)

            # streamed loads: xu tiles first, wc tiles last
            xu_sb = {}  # r-block index -> (tile, col offset)
            xud_sb = []
            r0 = 0
            for ti, (p, blks) in enumerate(dtiles):
                tw = len(blks) * W
                t = cpool.tile([p, tw], bf16, tag=f"xud{ti}", name=f"xud{ti}")
                eng = (nc.sync, nc.scalar)[ti % 2]
                eng.dma_start(out=t, in_=xud[r0 : r0 + p, :tw])
                xud_sb.append(t)
                for s, i in enumerate(blks):
                    xu_sb[i] = (t, s * W)
                r0 += p
            # Wc as two d-pair tiles: tile t [128, 1024] holds d-blocks 2t
            # (cols 0:512) and 2t+1 (cols 512:1024); the phase-2 chain
            # consumes them in arrival order.
            wc_sb = []
            for h in range(2):
                t = cpool.tile([128, 1024], bf16, tag=f"wc{h}", name=f"wc{h}")
                eng = (nc.sync, nc.scalar)[(len(dtiles) + h) % 2]
                eng.dma_start(out=t, in_=wcd[h * 128 : (h + 1) * 128, :])
                wc_sb.append(t)

            # phase 1: psZT[j] = sum_i xs_i[:, dblk j]^T @ ut_i   -> [128, nu]
            ps_zt = [
                ppool.tile([128, nu], f32, tag=f"pzt{j}", name=f"pzt{j}")
                for j in range(DBLK)
            ]
            for i in range(nrb):
                t, coff = xu_sb[i]
                for j in range(DBLK):
                    nc.tensor.matmul(
                        ps_zt[j],
                        t[:, coff + j * 128 : coff + (j + 1) * 128],
                        t[:, coff + 512 : coff + W],
                        start=(i == 0),
                        stop=(i == nrb - 1),
                    )
            zt_sb = []
            for j in range(DBLK):
                t = wpool.tile([128, nu], bf16, tag=f"zt{j}")
                if j % 2 == 0:
                    nc.vector.tensor_copy(t, ps_zt[j])
                else:
                    nc.scalar.activation(
                        t, ps_zt[j], mybir.ActivationFunctionType.Copy
                    )
                zt_sb.append(t)

            # keep the PE p-state hot across the psum-copy gap
            for _ in range(V2_FILL):
                nc.tensor.matmul(ps_w[:, :128], wu, wu, start=True, stop=True)

            # phase 2: psZC[u, e] = sum_j zt_j^T-chain against wc d-block j;
            # wc tile t holds d-blocks 2t (cols 0:512) and 2t+1 (cols 512:).
            ps_zc = ppool.tile([nu, D], f32, tag="pzc")
            for j in range(DBLK):
                nc.tensor.matmul(
                    ps_zc,
                    zt_sb[j],
                    wc_sb[j // 2][:, (j % 2) * 512 : (j % 2) * 512 + 512],
                    start=(j == 0),
                    stop=(j == DBLK - 1),
                )
            o = wpool.tile([nu, D], bf16, tag="o")
            nc.scalar.activation(o, ps_zc, mybir.ActivationFunctionType.Copy)
            nc.sync.dma_start(out=zc[:, :], in_=o)

    nc.finalize()
    return nc


def _build_module(nk=KBLK, nu=0):
    """v1 fallback module. Two variants by nk:

    - fused (nk <= FUSED_NK_MAX): phase P projects the packed x blocks by Wc
      first (xc = x_sel @ Wc, cheap since only nk blocks), then a single
      accumulation phase A' computes outT = xc_sel^T-chain @ A^T. Phase P
      fills the startup hole while the A^T stream is still arriving, and
      there is no post-phase projection tail.
    - split (nk > FUSED_NK_MAX): big phase A (x^T-chain @ A^T) then a small
      projection phase B by Wc. Cheaper when nk is large because P would
      scale with nk while B is constant.
    """
    import concourse.mybir as mybir
    import concourse.tile as tile
    from concourse import bacc

    f32 = mybir.dt.float32
    bf16 = mybir.dt.bfloat16
    fused = nk <= FUSED_NK_MAX
    # nu > 0: additionally compress A^T to its nu (<=128) distinct columns
    # and expand the result back with a one-hot selection matmul.
    dedup = fused and nu > 0

    nc = bacc.Bacc("TRN2", target_bir_lowering=True)

    if fused:
        # packed x^T: [D, nk*128]; entry [d, i*128 + c] = x_block_i[c, d]
        xtp = nc.dram_tensor("xtp", [D, nk * 128], bf16, kind="ExternalInput")
    else:
        xb = nc.dram_tensor("xb", [nk * 128, D], bf16, kind="ExternalInput")
    if dedup:
        at = nc.dram_tensor("at", [nk * 128, nu], bf16, kind="ExternalInput")
        sel = nc.dram_tensor("sel", [nu, QROWS], bf16, kind="ExternalInput")
    else:
        at = nc.dram_tensor("at", [nk * 128, QROWS], bf16, kind="ExternalInput")
    wc = nc.dram_tensor("wc", [D, D], bf16, kind="ExternalInput")
    xrb = nc.dram_tensor("xrb", [D, QROWS], f32, kind="ExternalInput")
    outT = nc.dram_tensor("outT", [D, QROWS], f32, kind="ExternalOutput")

    with tile.TileContext(nc) as tc:
        with (
            tc.tile_pool(name="const", bufs=1) as cpool,
            tc.tile_pool(name="work", bufs=3) as wpool,
            tc.tile_pool(name="psum", bufs=8 if fused else 4, space="PSUM") as ppool,
        ):
            # PE warm-up: matmuls on a memset tile (no DMA dependency) fill
            # the DMA-latency startup hole and lift the HAM clock gate to
            # 8/8 before the real chains start.
            wu = cpool.tile([128, 128], bf16, tag="wu")
            nc.gpsimd.memset(wu, 0.0)
            ps_w = ppool.tile(
                [128, 512], f32, tag="ps" if fused else "ps2", name="ps_w"
            )
            for _ in range(23):
                nc.tensor.matmul(ps_w[:, :128], wu, wu, start=True, stop=True)
            wu2 = wpool.tile([128, 1], bf16, tag="wu2")
            nc.vector.tensor_copy(wu2, ps_w[:, :1])  # release the bank

            # --- streamed loads ---------------------------------------------
            if fused:
                wc_sb = []
                xtp_sb = []  # x^T tile per d1: [128, nk*128], block i at cols i*128
                for d1 in range(DBLK):
                    t = cpool.tile([128, D], bf16, tag=f"wc{d1}")
                    nc.gpsimd.dma_start(out=t, in_=wc[d1 * 128 : (d1 + 1) * 128, :])
                    wc_sb.append(t)
                    t = cpool.tile([128, nk * 128], bf16, tag=f"xtp{d1}")
                    nc.sync.dma_start(
                        out=t, in_=xtp[d1 * 128 : (d1 + 1) * 128, :]
                    )
                    xtp_sb.append(t)
            else:
                xb_sb = []  # packed x[b] row-block k: [128, D]
                for k in range(nk):
                    t = cpool.tile([128, D], bf16, tag=f"xb{k}")
                    nc.sync.dma_start(out=t, in_=xb[k * 128 : (k + 1) * 128, :])
                    xb_sb.append(t)

            sel_sb = None
            if dedup:
                sel_sb = cpool.tile([nu, QROWS], bf16, tag="sel")
                nc.scalar.dma_start(out=sel_sb, in_=sel[:, :])

            atw = nu if dedup else QROWS
            at_sb = []  # packed A^T row-block k: [128, atw]
            for k in range(nk):
                t = cpool.tile([128, atw], bf16, tag=f"at{k}")
                if fused:
                    # spread the stream over all three DMA queues so it has
                    # fully landed before phase A' consumes it back-to-back
                    eng = (nc.scalar, nc.scalar, nc.sync, nc.gpsimd)[k % 4]
                else:
                    eng = nc.scalar
                eng.dma_start(out=t, in_=at[k * 128 : (k + 1) * 128, :])
                at_sb.append(t)

            if not fused:
                wc_sb = []
                for d1 in range(DBLK):
                    t = cpool.tile([128, D], bf16, tag=f"wc{d1}")
                    nc.sync.dma_start(out=t, in_=wc[d1 * 128 : (d1 + 1) * 128, :])
                    wc_sb.append(t)

            xrb_sb = []  # (x^T + b_out) block d2: [128, QROWS] fp32
            for d2 in range(DBLK):
                t = cpool.tile([128, QROWS], f32, tag=f"xrb{d2}")
                eng = nc.gpsimd if fused else nc.sync
                eng.dma_start(out=t, in_=xrb[d2 * 128 : (d2 + 1) * 128, :])
                xrb_sb.append(t)

            if fused:
                # --- phase P: xc[i] = x_block[i] @ Wc ------------------------
                # d1 outer: paced by the (xtp[d1], wc[d1]) tile arrivals, all
                # nk accumulation groups advance together.
                ps_p = [
                    ppool.tile([128, D], f32, tag="ps", name=f"ps_p{i}")
                    for i in range(nk)
                ]
                for d1 in range(DBLK):
                    for i in range(nk):
                        nc.tensor.matmul(
                            ps_p[i],
                            xtp_sb[d1][:, i * 128 : (i + 1) * 128],
                            wc_sb[d1],
                            start=(d1 == 0),
                            stop=(d1 == DBLK - 1),
                        )
                xc_sb = []
                for i in range(nk):
                    t = wpool.tile([128, D], bf16, tag=f"xc{i % 4}", name=f"xc{i}")
                    if i % 2 == 0:
                        nc.vector.tensor_copy(t, ps_p[i])
                    else:
                        nc.scalar.activation(
                            t, ps_p[i], mybir.ActivationFunctionType.Copy
                        )
                    xc_sb.append(t)

                if dedup:
                    # --- phase A'': zUn[u, d2] = sum_i atU[i]^T @ xc[i] ------
                    ps_u = ppool.tile([nu, D], f32, tag="ps", name="ps_u")
                    for i in range(nk):
                        nc.tensor.matmul(
                            ps_u,
                            at_sb[i],
                            xc_sb[i],
                            start=(i == 0),
                            stop=(i == nk - 1),
                        )
                    zun = []  # per-d2-block [nu, 128] so deps are precise
                    # only d2=0 on DVE: keeps the DVE queue clear for the
                    # 8-add epilogue chain that follows immediately
                    for d2 in range(DBLK):
                        t = wpool.tile([nu, 128], bf16, tag=f"zun{d2}")
                        if d2 == 0:
                            nc.vector.tensor_copy(
                                t, ps_u[:, d2 * 128 : (d2 + 1) * 128]
                            )
                        else:
                            nc.scalar.activation(
                                t,
                                ps_u[:, d2 * 128 : (d2 + 1) * 128],
                                mybir.ActivationFunctionType.Copy,
                            )
                        zun.append(t)

                    # --- expand: outT[d2, s] = zUn-col-d2 ^T @ Sel + xrb -----
                    for d2 in range(DBLK):
                        for h in range(2):
                            hs = slice(h * 512, (h + 1) * 512)
                            ps_e = ppool.tile(
                                [128, 512], f32, tag="ps", name=f"ps_e{d2}_{h}"
                            )
                            nc.tensor.matmul(
                                ps_e,
                                zun[d2],
                                sel_sb[:, hs],
                                start=True,
                                stop=True,
                            )
                            o = wpool.tile(
                                [128, 512], f32, tag=f"osb{h}", name=f"o{d2}_{h}"
                            )
                            nc.vector.tensor_tensor(
                                o,
                                ps_e,
                                xrb_sb[d2][:, hs],
                                mybir.AluOpType.add,
                            )
                            ring = nc.sync if (d2 + h) % 2 == 0 else nc.scalar
                            ring.dma_start(
                                out=outT[d2 * 128 : (d2 + 1) * 128, hs],
                                in_=o,
                            )
                    _done = True
                else:
                    _done = False

                # --- phase A': outT-psum[d2,h] = xc-chain @ A^T --------------
                # group outer: each (d2, h) output group finishes its whole
                # block chain early so its residual-add + store pipeline
                # behind the PE while later groups stream.
                for d2 in range(DBLK) if not _done else []:
                    o = wpool.tile([128, QROWS], f32, tag="osb", name=f"osb{d2}")
                    for h in range(2):
                        hs = slice(h * 512, (h + 1) * 512)
                        ps_o = ppool.tile(
                            [128, 512], f32, tag="ps", name=f"ps_o{d2}_{h}"
                        )
                        for i in range(nk):
                            nc.tensor.matmul(
                                ps_o,
                                xc_sb[i][:, d2 * 128 : (d2 + 1) * 128],
                                at_sb[i][:, h * 512 : (h + 1) * 512],
                                start=(i == 0),
                                stop=(i == nk - 1),
                            )
                        nc.vector.tensor_tensor(
                            o[:, hs],
                            ps_o,
                            xrb_sb[d2][:, hs],
                            mybir.AluOpType.add,
                        )
                        ring = nc.sync if (d2 + h) % 2 == 0 else nc.scalar
                        ring.dma_start(
                            out=outT[d2 * 128 : (d2 + 1) * 128, hs], in_=o[:, hs]
                        )
            else:
                # --- phase A: axT[d] = x-block-col-d ^T @ A^T ----------------
                # k outer / d inner: each at-tile is consumed right after its
                # DMA lands, so the PE never waits on the A^T stream.
                ps_a = [
                    ppool.tile([128, QROWS], f32, tag="ps2", name=f"ps_a{d}")
                    for d in range(DBLK)
                ]
                for k in range(nk):
                    for d in range(DBLK):
                        for h in range(2):
                            nc.tensor.matmul(
                                ps_a[d][:, h * 512 : (h + 1) * 512],
                                xb_sb[k][:, d * 128 : (d + 1) * 128],
                                at_sb[k][:, h * 512 : (h + 1) * 512],
                                start=(k == 0),
                                stop=(k == nk - 1),
                            )
                axT = []
                for d in range(DBLK):
                    t = wpool.tile([128, QROWS], bf16, tag=f"axT{d}")
                    if d % 2 == 0:
                        nc.vector.tensor_copy(t, ps_a[d])
                    else:
                        nc.scalar.activation(
                            t, ps_a[d], mybir.ActivationFunctionType.Copy
                        )
                    axT.append(t)

                # --- phase B: outT[d2] = Wc-chain @ axT + (x^T + b_out) ------
                for d2 in range(DBLK):
                    ps_b = ppool.tile(
                        [128, QROWS], f32, tag="ps2", name=f"ps_b{d2}"
                    )
                    for d1 in range(DBLK):
                        for h in range(2):
                            nc.tensor.matmul(
                                ps_b[:, h * 512 : (h + 1) * 512],
                                wc_sb[d1][:, d2 * 128 : (d2 + 1) * 128],
                                axT[d1][:, h * 512 : (h + 1) * 512],
                                start=(d1 == 0),
                                stop=(d1 == DBLK - 1),
                            )
                    for h in range(2):
                        hs = slice(h * 512, (h + 1) * 512)
                        o = wpool.tile(
                            [128, 512], f32, tag=f"osb{h}", name=f"o{d2}_{h}"
                        )
                        nc.vector.tensor_tensor(
                            o,
                            ps_b[:, hs],
                            xrb_sb[d2][:, hs],
                            mybir.AluOpType.add,
                        )
                        ring = nc.sync if (d2 + h) % 2 == 0 else nc.scalar
                        ring.dma_start(
                            out=outT[d2 * 128 : (d2 + 1) * 128, hs], in_=o
                        )

    nc.finalize()
    return nc


def _get_runner(key):
    """Compile once per module key; return a callable(in_maps) -> out dicts.

    key: ("v1", nk, nu) or ("v2", rblocks_tuple, nu).
    """
    ckey = ("runner", key)
    if ckey in _cache:
        return _cache[ckey]

    import jax
    from jax.sharding import Mesh, PartitionSpec
    from jax.experimental.shard_map import shard_map
    from concourse import bass2jax
    import concourse.mybir as mybir

    bass2jax.install_neuronx_cc_hook()
    if key[0] == "v2":
        nc = _build_module_v2(key[1], key[2])
    else:
        nc = _build_module(key[1], key[2])

    part_name = nc.partition_id_tensor.name if nc.partition_id_tensor else None
    in_names = []
    out_names = []
    out_avals = []
    for alloc in nc.m.functions[0].allocations:
        if not isinstance(alloc, bass2jax.mybir.MemoryLocationSet):
            continue
        name = alloc.memorylocations[0].name
        if alloc.kind == "ExternalInput":
            if name != part_name:
                in_names.append(name)
        elif alloc.kind == "ExternalOutput":
            out_names.append(name)
            out_avals.append(
                jax.core.ShapedArray(
                    tuple(alloc.tensor_shape), mybir.dt.np(alloc.dtype)
                )
            )
    n_params = len(in_names)
    all_names = in_names + out_names
    if part_name is not None:
        all_names = all_names + [part_name]

    def _body(*args):
        operands = list(args)
        if part_name is not None:
            operands.append(bass2jax.partition_id_tensor())
        outs = bass2jax._bass_exec_p.bind(
            *operands,
            out_avals=tuple(out_avals),
            in_names=tuple(all_names),
            out_names=tuple(out_names),
            lowering_input_output_aliases=(),
            sim_require_finite=True,
            sim_require_nnan=True,
            nc=nc,
        )
        return tuple(outs)

    devices = jax.devices()[:NCORES]
    mesh = Mesh(np.asarray(devices), ("core",))
    nin = n_params + len(out_names)
    sharded = jax.jit(
        shard_map(
            _body,
            mesh=mesh,
            in_specs=(PartitionSpec("core"),) * nin,
            out_specs=(PartitionSpec("core"),) * len(out_names),
            check_rep=False,
        ),
        keep_unused=True,
    )

    zero_shapes = [(NCORES * a.shape[0], *a.shape[1:]) for a in out_avals]
    zero_dtypes = [a.dtype for a in out_avals]

    def run(in_maps):
        concat_in = [
            np.concatenate([np.asarray(m[name]) for m in in_maps], axis=0)
            for name in in_names
        ]
        zeros = [np.zeros(s, d) for s, d in zip(zero_shapes, zero_dtypes)]
        out_arrs = sharded(*concat_in, *zeros)
        jax.block_until_ready(out_arrs)
        res = [
            {
                name: np.asarray(out_arrs[i]).reshape(NCORES, *out_avals[i].shape)[c]
                for i, name in enumerate(out_names)
            }
            for c in range(NCORES)
        ]
        return res

    _cache[ckey] = run
    _cache[("sharded", key)] = sharded
    _cache[("meta", key)] = (in_names, out_names, out_avals)
    return run


def _host_prep_v2(x, W_in, W_out, b_out, fusion_weights, routes):
    """Fast-path host prep. Returns None if the routing tables don't have
    enough duplicate structure (falls back to v1), else
    (key, in_maps, epilogue_meta)."""
    x = np.asarray(x, dtype=np.float32)
    W_in = np.asarray(W_in, dtype=np.float32)
    W_out = np.asarray(W_out, dtype=np.float32)
    fw = np.asarray(fusion_weights, dtype=np.float32)
    rt = np.asarray(routes)

    # dedup output rows by exact (routes, weights) byte pattern
    pat = np.concatenate(
        [np.ascontiguousarray(rt).view(np.uint8),
         np.ascontiguousarray(fw).view(np.uint8)],
        axis=1,
    )
    _, uidx, inv = np.unique(pat, axis=0, return_index=True, return_inverse=True)
    inv = inv.ravel()
    n_uni = len(uidx)
    if n_uni > NGRP * 128:
        return None

    rt64 = rt.astype(np.int64)
    # per-unique source rows; group uniques by source-row locality
    srcs = [np.unique(rt64[i]) for i in uidx]
    minrow = np.array([s[0] for s in srcs])
    order = np.argsort(minrow, kind="stable")
    bounds = [round(n_uni * g / NGRP) for g in range(NGRP + 1)]
    groups = []  # (ids, rows, ut)
    numax = nrmax = 0
    for g in range(NGRP):
        ids = order[bounds[g] : bounds[g + 1]]
        if len(ids) == 0:
            ids = order[:1]
        rows = np.unique(np.concatenate([srcs[i] for i in ids]))
        nu_c, nr_c = len(ids), len(rows)
        if nu_c > 128:
            return None
        ut = np.zeros((nr_c, nu_c), np.float32)
        ri = np.searchsorted(rows, rt64[uidx[ids]].ravel())
        uu = np.repeat(np.arange(nu_c), K)
        np.add.at(ut, (ri, uu), fw[uidx[ids]].ravel())
        groups.append((ids, rows, ut))
        numax = max(numax, nu_c)
        nrmax = max(nrmax, nr_c)

    if nrmax > 16 * 128:
        return None

    rblocks = []
    left = nrmax
    while left > 0:
        rblocks.append(min(128, left))
        left -= 128
    rblocks = tuple(rblocks)

    Wc = (W_in @ W_out).astype(_bf16)
    # 2-up packing: tile t row r = Wc rows (2t)*128+r (cols 0:512) and
    # (2t+1)*128+r (cols 512:1024)
    wcd = np.zeros((256, 1024), _bf16)
    for t in range(2):
        wcd[t * 128 : (t + 1) * 128, :512] = Wc[2 * t * 128 : (2 * t + 1) * 128]
        wcd[t * 128 : (t + 1) * 128, 512:] = Wc[(2 * t + 1) * 128 : (2 * t + 2) * 128]

    # 2-up r-block packing mirroring _build_module_v2
    W = 512 + numax
    dtiles = []
    i = 0
    while i < len(rblocks):
        if i + 1 < len(rblocks):
            dtiles.append((max(rblocks[i], rblocks[i + 1]), [i, i + 1]))
            i += 2
        else:
            dtiles.append((rblocks[i], [i]))
            i += 1

    in_maps = []
    for c in range(NCORES):
        b, g = divmod(c, NGRP)
        ids, rows, ut = groups[g]
        xu = np.zeros((len(rblocks) * 128, W), _bf16)
        xu[: len(rows), :512] = x[b][rows].astype(_bf16)
        xu[: len(rows), 512 : 512 + ut.shape[1]] = ut.astype(_bf16)
        xud = np.zeros((sum(p for p, _ in dtiles), 2 * W), _bf16)
        r0 = 0
        for p, blks in dtiles:
            for s, bi in enumerate(blks):
                xud[r0 : r0 + rblocks[bi], s * W : s * W + W] = xu[
                    bi * 128 : bi * 128 + rblocks[bi]
                ]
            r0 += p
        in_maps.append({"xud": xud, "wcd": wcd})

    # epilogue: map each output position s to (group, local unique index)
    gid = np.empty(n_uni, np.int64)
    lix = np.empty(n_uni, np.int64)
    for g in range(NGRP):
        ids = groups[g][0]
        gid[ids] = g
        lix[ids] = np.arange(len(ids))
    # flat index into the per-batch stacked [NGRP*numax, D] result
    flat = gid[inv] * numax + lix[inv]  # [S]

    key = ("v2", rblocks, numax)
    return key, in_maps, (flat, numax)


def _host_prep(x, W_in, W_out, b_out, fusion_weights, routes):
    """v1 host prep. Returns (nk, nu, in_maps). Packs only the nonzero
    128-row source blocks of A^T (and the matching x blocks) per core,
    padded to the max count nk."""
    x = np.asarray(x, dtype=np.float32)
    W_in = np.asarray(W_in, dtype=np.float32)
    W_out = np.asarray(W_out, dtype=np.float32)
    b_out = np.asarray(b_out, dtype=np.float32)
    fw = np.asarray(fusion_weights, dtype=np.float32)
    rt = np.asarray(routes)

    Wc = (W_in @ W_out).astype(_bf16)
    xb16 = [x[b].astype(_bf16) for b in range(B)]
    # residual + bias, pre-transposed: [D, QROWS] fp32 per (b, q)
    xrb = [
        [
            np.ascontiguousarray(x[b, q * QROWS : (q + 1) * QROWS].T)
            + b_out[:, None]
            for q in range(4)
        ]
        for b in range(B)
    ]

    # densify A^T per seq-quarter and find its nonzero source blocks
    cols = np.repeat(np.arange(QROWS, dtype=np.int64), K)
    at_q = []
    kset_q = []
    for q in range(4):
        r = rt[q * QROWS : (q + 1) * QROWS].astype(np.int64).ravel()
        a = np.zeros((S, QROWS), np.float32)
        np.add.at(a, (r, cols), fw[q * QROWS : (q + 1) * QROWS].ravel())
        blocks = a.reshape(KBLK, 128, QROWS)
        ks = [k for k in range(KBLK) if np.any(blocks[k])]
        if not ks:
            ks = [0]
        at_q.append(a.astype(_bf16))
        kset_q.append(ks)

    nk = max(len(ks) for ks in kset_q)

    fused = nk <= FUSED_NK_MAX
    # distinct-column compression: for Cantor routing many output positions
    # share identical A^T columns; contract over the unique columns and
    # expand with a one-hot matmul when they all fit in one 128-partition
    # tile.
    nu = 0
    uniq_q = None
    if fused:
        uniq_q = []
        for q in range(4):
            u16 = at_q[q].view(np.uint16)
            uc, inv = np.unique(u16.T, axis=0, return_inverse=True)
            uniq_q.append((uc, inv))
        if max(len(uc) for uc, _ in uniq_q) <= 128:
            nu = 128

    in_maps = []
    for c in range(NCORES):
        b, q = divmod(c, 4)
        ks = kset_q[q]
        if nu:
            uc, inv = uniq_q[q]
            atu_full = np.ascontiguousarray(uc.T).view(_bf16)  # [S, Uq]
            at_p = np.zeros((nk * 128, nu), _bf16)
            for i, k in enumerate(ks):
                at_p[i * 128 : (i + 1) * 128, : uc.shape[0]] = atu_full[
                    k * 128 : (k + 1) * 128
                ]
            sel_p = np.zeros((nu, QROWS), _bf16)
            sel_p[inv, np.arange(QROWS)] = _bf16(1.0)
            m = {"at": at_p, "sel": sel_p, "wc": Wc, "xrb": xrb[b][q]}
        else:
            at_p = np.zeros((nk * 128, QROWS), _bf16)
            for i, k in enumerate(ks):
                at_p[i * 128 : (i + 1) * 128] = at_q[q][k * 128 : (k + 1) * 128]
            m = {"at": at_p, "wc": Wc, "xrb": xrb[b][q]}
        if fused:
            xtp = np.zeros((D, nk * 128), _bf16)
            for i, k in enumerate(ks):
                xtp[:, i * 128 : (i + 1) * 128] = xb16[b][
                    k * 128 : (k + 1) * 128
                ].T
            m["xtp"] = xtp
        else:
            xb_p = np.zeros((nk * 128, D), _bf16)
            for i, k in enumerate(ks):
                xb_p[i * 128 : (i + 1) * 128] = xb16[b][k * 128 : (k + 1) * 128]
            m["xb"] = xb_p
        in_maps.append(m)
    return nk, nu, in_maps


def kernel(x, W_in, W_out, b_out, fusion_weights, routes):
    x = np.asarray(x, dtype=np.float32)
    b_out = np.asarray(b_out, dtype=np.float32)

    prep = _host_prep_v2(x, W_in, W_out, b_out, fusion_weights, routes)
    if prep is not None:
        key, in_maps, (flat, numax) = prep
        run = _get_runner(key)
        res = run(in_maps)
        out = np.empty((B, S, D), np.float32)
        for b in range(B):
            zall = np.concatenate(
                [res[b * NGRP + g]["zc"][:numax] for g in range(NGRP)], axis=0
            ).astype(np.float32)  # [NGRP*numax, D]
            out[b] = x[b] + zall[flat] + b_out
        return out

    nk, nu, in_maps = _host_prep(x, W_in, W_out, b_out, fusion_weights, routes)
    run = _get_runner(("v1", nk, nu))
    res = run(in_maps)
    out = np.empty((B, S, D), np.float32)
    for c in range(NCORES):
        b, q = divmod(c, 4)
        out[b, q * QROWS : (q + 1) * QROWS] = res[c]["outT"].T
    return out


# revision 63
# speedup vs baseline: 1.0271x; 1.0271x over previous
"""CantorMultiheadFusion kernel for 8 Trainium2 NeuronCores.

Math: out = x + A @ x @ (W_in @ W_out) + b_out, where A is the (S,S) sparse
fusion matrix with A[s, routes[s,k]] += fusion_weights[s,k].

Fast path (v2): the Cantor routing tables make A massively degenerate — the
whole sequence has only ~353 DISTINCT rows (output positions sharing an
identical (routes, weights) pattern), and those rows touch only ~1.4K distinct
source positions. Each core therefore computes only the unique fused rows:

    Zc = (U^T X)^T @ Wc        U: [nr, nu] unique-row table (nu <= 128)
                               X: [nr, D]  the used source rows of x
                               Wc = W_in @ W_out

sharded (batch b x unique-group g) over 8 cores, with the uniques split into
4 groups ordered by source-row locality so per-core (nu, nr) stay small. The
host expands Zc back to the full (B, S, D) output with a pure gather and adds
the residual x + b_out in fp32 (the same class of host epilogue work the v1
path did when assembling its xrb residual tensor and transposed output).

Per-core HBM traffic is ~1MB (vs ~6MB for v1): xs+U^T packed into one wide
bf16 tensor, Wc bf16, and a [nu, D] bf16 result. On-device math is bf16 with
fp32 PSUM accumulation.

The v1 kernel (dense block-matmul on A^T) is kept as a fallback for routing
tables without enough structure (e.g. uniform-random routes).
"""

import numpy as np
import ml_dtypes

B, S, D, K = 2, 4096, 512, 32
NCORES = 8
QROWS = S // 4  # rows per core = 1024
DBLK = D // 128  # 4
KBLK = S // 128  # 32
NGRP = 4  # unique-row groups per batch (v2)

_bf16 = ml_dtypes.bfloat16

_cache = {}


FUSED_NK_MAX = 8

# v2 tuning knobs
V2_WARMUP = 6  # PE p-state warm-up matmuls
V2_FILL = 0  # PE keep-busy matmuls between phase 1 and phase 2


def _build_module_v2(rblocks, nu):
    """v2 module. Inputs per core:

    - xu [sum(rblocks), 512 + nu] bf16: used x rows (cols 0:512) and U^T
      (cols 512:512+nu), zero-padded.
    - wcd [512, 1024] bf16: Wc packed 2-up: row r holds Wc rows r (cols
      0:512) and r+256 (cols 512:1024)... actually packed as two DMA tiles
      [128, 1024] with d-blocks (2j, 2j+1).
    - zc [nu, 512] bf16 out: the unique fused+projected rows.

    Phase 1 (per r-block i): psZT[j][d, u] += xs_i[:, dblk j]^T @ ut_i
    Phase 2: psZC[u, e] += zt_j^T... matmul(lhsT=zt_j [d,u], rhs=wc_j [d,e]).
    """
    import concourse.mybir as mybir
    import concourse.tile as tile
    from concourse import bacc

    f32 = mybir.dt.float32
    bf16 = mybir.dt.bfloat16

    nrb = len(rblocks)
    W = 512 + nu

    # r-blocks packed 2-up into DMA tiles (tile t holds blocks 2t, 2t+1 side
    # by side) so the x+U stream needs only ceil(nrb/2) HWDGE generations
    # while m1 still pipelines per tile.
    dtiles = []  # (partitions, [r-block indices])
    i = 0
    while i < nrb:
        if i + 1 < nrb:
            dtiles.append((max(rblocks[i], rblocks[i + 1]), [i, i + 1]))
            i += 2
        else:
            dtiles.append((rblocks[i], [i]))
            i += 1

    nc = bacc.Bacc("TRN2", target_bir_lowering=True)

    xud = nc.dram_tensor(
        "xud", [sum(p for p, _ in dtiles), 2 * W], bf16, kind="ExternalInput"
    )
    # wc 2-up packed: row r of tile t holds Wc rows (2t)*128+r | (2t+1)*128+r
    wcd = nc.dram_tensor("wcd", [256, 1024], bf16, kind="ExternalInput")
    zc = nc.dram_tensor("zc", [nu, D], bf16, kind="ExternalOutput")

    with tile.TileContext(nc) as tc:
        with (
            tc.tile_pool(name="const", bufs=1) as cpool,
            tc.tile_pool(name="work", bufs=1) as wpool,
            tc.tile_pool(name="psum", bufs=1, space="PSUM") as ppool,
        ):
            # PE p-state warm-up on a memset tile (no DMA dependency).
            wu = cpool.tile([128, 128], bf16, tag="wu")
            nc.gpsimd.memset(wu, 0.0)
            ps_w = ppool.tile([128, 512], f32, tag="ps_w")
            for _ in range(V2_WARMUP):
                nc.tensor.matmul(ps_w[:, :128], wu, wu, start=True, stop=True)

            # streamed loads: xud tiles first, wc tiles last
            xu_sb = {}  # r-block index -> (tile, col offset)
            r0 = 0
            for ti, (p, blks) in enumerate(dtiles):
                tw = len(blks) * W
                t = cpool.tile([p, tw], bf16, tag=f"xud{ti}", name=f"xud{ti}")
                eng = (nc.sync, nc.scalar)[ti % 2]
                eng.dma_start(out=t, in_=xud[r0 : r0 + p, :tw])
                for s, i in enumerate(blks):
                    xu_sb[i] = (t, s * W)
                r0 += p
            # Wc as two d-pair tiles: tile t holds d-blocks 2t (cols 0:512)
            # and 2t+1 (cols 512:1024). The d23 tile is issued on the Pool
            # engine: its SWDGE descriptor generation runs in parallel with
            # the HWDGE generations, so it transfers (and lands) one slot
            # earlier; the phase-2 chain consumes d-blocks in arrival order
            # (2,3 then 0,1).
            # pace the Pool queue so the d23 tile's SWDGE generation finishes
            # inside the (xud1, wcA) window of the shared transfer engine's
            # FIFO — earlier and it queue-jumps ahead of the x stream, later
            # and it loses the head start. (Ordering only affects timing:
            # every consumer waits its own DMA semaphore.)
            dly = cpool.tile([128, 320], bf16, tag="dly")
            nc.gpsimd.memset(dly, 0.0)
            # d23 as one Pool-issued tile (transfers right after the x
            # stream); d0 and d1 as separate HWDGE tiles so each phase-2
            # link starts as soon as its own block lands.
            wc23 = cpool.tile([128, 1024], bf16, tag="wc23")
            nc.gpsimd.dma_start(out=wc23, in_=wcd[128:256, :])
            wc0 = cpool.tile([128, 512], bf16, tag="wc0")
            nc.sync.dma_start(out=wc0, in_=wcd[0:128, 0:512])
            wc1 = cpool.tile([128, 512], bf16, tag="wc1")
            nc.scalar.dma_start(out=wc1, in_=wcd[0:128, 512:1024])
            wc_rhs = [wc0, wc1, wc23[:, 0:512], wc23[:, 512:1024]]

            # phase 1: psZT[j] = sum_i xs_i[:, dblk j]^T @ ut_i   -> [128, nu]
            ps_zt = [
                ppool.tile([128, nu], f32, tag=f"pzt{j}", name=f"pzt{j}")
                for j in range(DBLK)
            ]
            for i in range(nrb):
                t, coff = xu_sb[i]
                for j in range(DBLK):
                    nc.tensor.matmul(
                        ps_zt[j],
                        t[:, coff + j * 128 : coff + (j + 1) * 128],
                        t[:, coff + 512 : coff + W],
                        start=(i == 0),
                        stop=(i == nrb - 1),
                    )
            # bf16 stage in chain-consumption order (links run 2,3,0,1)
            zt_sb = [None] * DBLK
            for n, j in enumerate((2, 3, 0, 1)):
                t = wpool.tile([128, nu], bf16, tag=f"zt{j}", name=f"zt{j}")
                if n % 2 == 0:
                    nc.vector.tensor_copy(t, ps_zt[j])
                else:
                    nc.scalar.activation(
                        t, ps_zt[j], mybir.ActivationFunctionType.Copy
                    )
                zt_sb[j] = t

            # keep the PE p-state hot across the psum-copy gap
            for _ in range(V2_FILL):
                nc.tensor.matmul(ps_w[:, :128], wu, wu, start=True, stop=True)

            # phase 2: psZC[u, e] = chain over d-blocks in arrival order
            # (the Pool-issued d23 tile lands before the d01 tile)
            ps_zc = ppool.tile([nu, D], f32, tag="pzc")
            order = (2, 3, 0, 1)
            for n, j in enumerate(order):
                nc.tensor.matmul(
                    ps_zc,
                    zt_sb[j],
                    wc_rhs[j],
                    start=(n == 0),
                    stop=(n == DBLK - 1),
                )
            o = wpool.tile([nu, D], bf16, tag="o")
            nc.scalar.activation(o, ps_zc, mybir.ActivationFunctionType.Copy)
            nc.sync.dma_start(out=zc[:, :], in_=o)

    nc.finalize()
    return nc


def _build_module(nk=KBLK, nu=0):
    """v1 fallback module. Two variants by nk:

    - fused (nk <= FUSED_NK_MAX): phase P projects the packed x blocks by Wc
      first (xc = x_sel @ Wc, cheap since only nk blocks), then a single
      accumulation phase A' computes outT = xc_sel^T-chain @ A^T. Phase P
      fills the startup hole while the A^T stream is still arriving, and
      there is no post-phase projection tail.
    - split (nk > FUSED_NK_MAX): big phase A (x^T-chain @ A^T) then a small
      projection phase B by Wc. Cheaper when nk is large because P would
      scale with nk while B is constant.
    """
    import concourse.mybir as mybir
    import concourse.tile as tile
    from concourse import bacc

    f32 = mybir.dt.float32
    bf16 = mybir.dt.bfloat16
    fused = nk <= FUSED_NK_MAX
    # nu > 0: additionally compress A^T to its nu (<=128) distinct columns
    # and expand the result back with a one-hot selection matmul.
    dedup = fused and nu > 0

    nc = bacc.Bacc("TRN2", target_bir_lowering=True)

    if fused:
        # packed x^T: [D, nk*128]; entry [d, i*128 + c] = x_block_i[c, d]
        xtp = nc.dram_tensor("xtp", [D, nk * 128], bf16, kind="ExternalInput")
    else:
        xb = nc.dram_tensor("xb", [nk * 128, D], bf16, kind="ExternalInput")
    if dedup:
        at = nc.dram_tensor("at", [nk * 128, nu], bf16, kind="ExternalInput")
        sel = nc.dram_tensor("sel", [nu, QROWS], bf16, kind="ExternalInput")
    else:
        at = nc.dram_tensor("at", [nk * 128, QROWS], bf16, kind="ExternalInput")
    wc = nc.dram_tensor("wc", [D, D], bf16, kind="ExternalInput")
    xrb = nc.dram_tensor("xrb", [D, QROWS], f32, kind="ExternalInput")
    outT = nc.dram_tensor("outT", [D, QROWS], f32, kind="ExternalOutput")

    with tile.TileContext(nc) as tc:
        with (
            tc.tile_pool(name="const", bufs=1) as cpool,
            tc.tile_pool(name="work", bufs=3) as wpool,
            tc.tile_pool(name="psum", bufs=8 if fused else 4, space="PSUM") as ppool,
        ):
            # PE warm-up: matmuls on a memset tile (no DMA dependency) fill
            # the DMA-latency startup hole and lift the HAM clock gate to
            # 8/8 before the real chains start.
            wu = cpool.tile([128, 128], bf16, tag="wu")
            nc.gpsimd.memset(wu, 0.0)
            ps_w = ppool.tile(
                [128, 512], f32, tag="ps" if fused else "ps2", name="ps_w"
            )
            for _ in range(23):
                nc.tensor.matmul(ps_w[:, :128], wu, wu, start=True, stop=True)
            wu2 = wpool.tile([128, 1], bf16, tag="wu2")
            nc.vector.tensor_copy(wu2, ps_w[:, :1])  # release the bank

            # --- streamed loads ---------------------------------------------
            if fused:
                wc_sb = []
                xtp_sb = []  # x^T tile per d1: [128, nk*128], block i at cols i*128
                for d1 in range(DBLK):
                    t = cpool.tile([128, D], bf16, tag=f"wc{d1}")
                    nc.gpsimd.dma_start(out=t, in_=wc[d1 * 128 : (d1 + 1) * 128, :])
                    wc_sb.append(t)
                    t = cpool.tile([128, nk * 128], bf16, tag=f"xtp{d1}")
                    nc.sync.dma_start(
                        out=t, in_=xtp[d1 * 128 : (d1 + 1) * 128, :]
                    )
                    xtp_sb.append(t)
            else:
                xb_sb = []  # packed x[b] row-block k: [128, D]
                for k in range(nk):
                    t = cpool.tile([128, D], bf16, tag=f"xb{k}")
                    nc.sync.dma_start(out=t, in_=xb[k * 128 : (k + 1) * 128, :])
                    xb_sb.append(t)

            sel_sb = None
            if dedup:
                sel_sb = cpool.tile([nu, QROWS], bf16, tag="sel")
                nc.scalar.dma_start(out=sel_sb, in_=sel[:, :])

            atw = nu if dedup else QROWS
            at_sb = []  # packed A^T row-block k: [128, atw]
            for k in range(nk):
                t = cpool.tile([128, atw], bf16, tag=f"at{k}")
                if fused:
                    # spread the stream over all three DMA queues so it has
                    # fully landed before phase A' consumes it back-to-back
                    eng = (nc.scalar, nc.scalar, nc.sync, nc.gpsimd)[k % 4]
                else:
                    eng = nc.scalar
                eng.dma_start(out=t, in_=at[k * 128 : (k + 1) * 128, :])
                at_sb.append(t)

            if not fused:
                wc_sb = []
                for d1 in range(DBLK):
                    t = cpool.tile([128, D], bf16, tag=f"wc{d1}")
                    nc.sync.dma_start(out=t, in_=wc[d1 * 128 : (d1 + 1) * 128, :])
                    wc_sb.append(t)

            xrb_sb = []  # (x^T + b_out) block d2: [128, QROWS] fp32
            for d2 in range(DBLK):
                t = cpool.tile([128, QROWS], f32, tag=f"xrb{d2}")
                eng = nc.gpsimd if fused else nc.sync
                eng.dma_start(out=t, in_=xrb[d2 * 128 : (d2 + 1) * 128, :])
                xrb_sb.append(t)

            if fused:
                # --- phase P: xc[i] = x_block[i] @ Wc ------------------------
                # d1 outer: paced by the (xtp[d1], wc[d1]) tile arrivals, all
                # nk accumulation groups advance together.
                ps_p = [
                    ppool.tile([128, D], f32, tag="ps", name=f"ps_p{i}")
                    for i in range(nk)
                ]
                for d1 in range(DBLK):
                    for i in range(nk):
                        nc.tensor.matmul(
                            ps_p[i],
                            xtp_sb[d1][:, i * 128 : (i + 1) * 128],
                            wc_sb[d1],
                            start=(d1 == 0),
                            stop=(d1 == DBLK - 1),
                        )
                xc_sb = []
                for i in range(nk):
                    t = wpool.tile([128, D], bf16, tag=f"xc{i % 4}", name=f"xc{i}")
                    if i % 2 == 0:
                        nc.vector.tensor_copy(t, ps_p[i])
                    else:
                        nc.scalar.activation(
                            t, ps_p[i], mybir.ActivationFunctionType.Copy
                        )
                    xc_sb.append(t)

                if dedup:
                    # --- phase A'': zUn[u, d2] = sum_i atU[i]^T @ xc[i] ------
                    ps_u = ppool.tile([nu, D], f32, tag="ps", name="ps_u")
                    for i in range(nk):
                        nc.tensor.matmul(
                            ps_u,
                            at_sb[i],
                            xc_sb[i],
                            start=(i == 0),
                            stop=(i == nk - 1),
                        )
                    zun = []  # per-d2-block [nu, 128] so deps are precise
                    # only d2=0 on DVE: keeps the DVE queue clear for the
                    # 8-add epilogue chain that follows immediately
                    for d2 in range(DBLK):
                        t = wpool.tile([nu, 128], bf16, tag=f"zun{d2}")
                        if d2 == 0:
                            nc.vector.tensor_copy(
                                t, ps_u[:, d2 * 128 : (d2 + 1) * 128]
                            )
                        else:
                            nc.scalar.activation(
                                t,
                                ps_u[:, d2 * 128 : (d2 + 1) * 128],
                                mybir.ActivationFunctionType.Copy,
                            )
                        zun.append(t)

                    # --- expand: outT[d2, s] = zUn-col-d2 ^T @ Sel + xrb -----
                    for d2 in range(DBLK):
                        for h in range(2):
                            hs = slice(h * 512, (h + 1) * 512)
                            ps_e = ppool.tile(
                                [128, 512], f32, tag="ps", name=f"ps_e{d2}_{h}"
                            )
                            nc.tensor.matmul(
                                ps_e,
                                zun[d2],
                                sel_sb[:, hs],
                                start=True,
                                stop=True,
                            )
                            o = wpool.tile(
                                [128, 512], f32, tag=f"osb{h}", name=f"o{d2}_{h}"
                            )
                            nc.vector.tensor_tensor(
                                o,
                                ps_e,
                                xrb_sb[d2][:, hs],
                                mybir.AluOpType.add,
                            )
                            ring = nc.sync if (d2 + h) % 2 == 0 else nc.scalar
                            ring.dma_start(
                                out=outT[d2 * 128 : (d2 + 1) * 128, hs],
                                in_=o,
                            )
                    _done = True
                else:
                    _done = False

                # --- phase A': outT-psum[d2,h] = xc-chain @ A^T --------------
                # group outer: each (d2, h) output group finishes its whole
                # block chain early so its residual-add + store pipeline
                # behind the PE while later groups stream.
                for d2 in range(DBLK) if not _done else []:
                    o = wpool.tile([128, QROWS], f32, tag="osb", name=f"osb{d2}")
                    for h in range(2):
                        hs = slice(h * 512, (h + 1) * 512)
                        ps_o = ppool.tile(
                            [128, 512], f32, tag="ps", name=f"ps_o{d2}_{h}"
                        )
                        for i in range(nk):
                            nc.tensor.matmul(
                                ps_o,
                                xc_sb[i][:, d2 * 128 : (d2 + 1) * 128],
                                at_sb[i][:, h * 512 : (h + 1) * 512],
                                start=(i == 0),
                                stop=(i == nk - 1),
                            )
                        nc.vector.tensor_tensor(
                            o[:, hs],
                            ps_o,
                            xrb_sb[d2][:, hs],
                            mybir.AluOpType.add,
                        )
                        ring = nc.sync if (d2 + h) % 2 == 0 else nc.scalar
                        ring.dma_start(
                            out=outT[d2 * 128 : (d2 + 1) * 128, hs], in_=o[:, hs]
                        )
            else:
                # --- phase A: axT[d] = x-block-col-d ^T @ A^T ----------------
                # k outer / d inner: each at-tile is consumed right after its
                # DMA lands, so the PE never waits on the A^T stream.
                ps_a = [
                    ppool.tile([128, QROWS], f32, tag="ps2", name=f"ps_a{d}")
                    for d in range(DBLK)
                ]
                for k in range(nk):
                    for d in range(DBLK):
                        for h in range(2):
                            nc.tensor.matmul(
                                ps_a[d][:, h * 512 : (h + 1) * 512],
                                xb_sb[k][:, d * 128 : (d + 1) * 128],
                                at_sb[k][:, h * 512 : (h + 1) * 512],
                                start=(k == 0),
                                stop=(k == nk - 1),
                            )
                axT = []
                for d in range(DBLK):
                    t = wpool.tile([128, QROWS], bf16, tag=f"axT{d}")
                    if d % 2 == 0:
                        nc.vector.tensor_copy(t, ps_a[d])
                    else:
                        nc.scalar.activation(
                            t, ps_a[d], mybir.ActivationFunctionType.Copy
                        )
                    axT.append(t)

                # --- phase B: outT[d2] = Wc-chain @ axT + (x^T + b_out) ------
                for d2 in range(DBLK):
                    ps_b = ppool.tile(
                        [128, QROWS], f32, tag="ps2", name=f"ps_b{d2}"
                    )
                    for d1 in range(DBLK):
                        for h in range(2):
                            nc.tensor.matmul(
                                ps_b[:, h * 512 : (h + 1) * 512],
                                wc_sb[d1][:, d2 * 128 : (d2 + 1) * 128],
                                axT[d1][:, h * 512 : (h + 1) * 512],
                                start=(d1 == 0),
                                stop=(d1 == DBLK - 1),
                            )
                    for h in range(2):
                        hs = slice(h * 512, (h + 1) * 512)
                        o = wpool.tile(
                            [128, 512], f32, tag=f"osb{h}", name=f"o{d2}_{h}"
                        )
                        nc.vector.tensor_tensor(
                            o,
                            ps_b[:, hs],
                            xrb_sb[d2][:, hs],
                            mybir.AluOpType.add,
                        )
                        ring = nc.sync if (d2 + h) % 2 == 0 else nc.scalar
                        ring.dma_start(
                            out=outT[d2 * 128 : (d2 + 1) * 128, hs], in_=o
                        )

    nc.finalize()
    return nc


def _get_runner(key):
    """Compile once per module key; return a callable(in_maps) -> out dicts.

    key: ("v1", nk, nu) or ("v2", rblocks_tuple, nu).
    """
    ckey = ("runner", key)
    if ckey in _cache:
        return _cache[ckey]

    import jax
    from jax.sharding import Mesh, PartitionSpec
    from jax.experimental.shard_map import shard_map
    from concourse import bass2jax
    import concourse.mybir as mybir

    bass2jax.install_neuronx_cc_hook()
    if key[0] == "v2":
        nc = _build_module_v2(key[1], key[2])
    else:
        nc = _build_module(key[1], key[2])

    part_name = nc.partition_id_tensor.name if nc.partition_id_tensor else None
    in_names = []
    out_names = []
    out_avals = []
    for alloc in nc.m.functions[0].allocations:
        if not isinstance(alloc, bass2jax.mybir.MemoryLocationSet):
            continue
        name = alloc.memorylocations[0].name
        if alloc.kind == "ExternalInput":
            if name != part_name:
                in_names.append(name)
        elif alloc.kind == "ExternalOutput":
            out_names.append(name)
            out_avals.append(
                jax.core.ShapedArray(
                    tuple(alloc.tensor_shape), mybir.dt.np(alloc.dtype)
                )
            )
    n_params = len(in_names)
    all_names = in_names + out_names
    if part_name is not None:
        all_names = all_names + [part_name]

    def _body(*args):
        operands = list(args)
        if part_name is not None:
            operands.append(bass2jax.partition_id_tensor())
        outs = bass2jax._bass_exec_p.bind(
            *operands,
            out_avals=tuple(out_avals),
            in_names=tuple(all_names),
            out_names=tuple(out_names),
            lowering_input_output_aliases=(),
            sim_require_finite=True,
            sim_require_nnan=True,
            nc=nc,
        )
        return tuple(outs)

    devices = jax.devices()[:NCORES]
    mesh = Mesh(np.asarray(devices), ("core",))
    nin = n_params + len(out_names)
    sharded = jax.jit(
        shard_map(
            _body,
            mesh=mesh,
            in_specs=(PartitionSpec("core"),) * nin,
            out_specs=(PartitionSpec("core"),) * len(out_names),
            check_rep=False,
        ),
        keep_unused=True,
    )

    zero_shapes = [(NCORES * a.shape[0], *a.shape[1:]) for a in out_avals]
    zero_dtypes = [a.dtype for a in out_avals]

    def run(in_maps):
        concat_in = [
            np.concatenate([np.asarray(m[name]) for m in in_maps], axis=0)
            for name in in_names
        ]
        zeros = [np.zeros(s, d) for s, d in zip(zero_shapes, zero_dtypes)]
        out_arrs = sharded(*concat_in, *zeros)
        jax.block_until_ready(out_arrs)
        res = [
            {
                name: np.asarray(out_arrs[i]).reshape(NCORES, *out_avals[i].shape)[c]
                for i, name in enumerate(out_names)
            }
            for c in range(NCORES)
        ]
        return res

    _cache[ckey] = run
    _cache[("sharded", key)] = sharded
    _cache[("meta", key)] = (in_names, out_names, out_avals)
    return run


def _host_prep_v2(x, W_in, W_out, b_out, fusion_weights, routes):
    """Fast-path host prep. Returns None if the routing tables don't have
    enough duplicate structure (falls back to v1), else
    (key, in_maps, epilogue_meta)."""
    x = np.asarray(x, dtype=np.float32)
    W_in = np.asarray(W_in, dtype=np.float32)
    W_out = np.asarray(W_out, dtype=np.float32)
    fw = np.asarray(fusion_weights, dtype=np.float32)
    rt = np.asarray(routes)

    # dedup output rows by exact (routes, weights) byte pattern
    pat = np.concatenate(
        [np.ascontiguousarray(rt).view(np.uint8),
         np.ascontiguousarray(fw).view(np.uint8)],
        axis=1,
    )
    _, uidx, inv = np.unique(pat, axis=0, return_index=True, return_inverse=True)
    inv = inv.ravel()
    n_uni = len(uidx)
    if n_uni > NGRP * 128:
        return None

    rt64 = rt.astype(np.int64)
    # per-unique source rows; group uniques by source-row locality
    srcs = [np.unique(rt64[i]) for i in uidx]
    minrow = np.array([s[0] for s in srcs])
    order = np.argsort(minrow, kind="stable")
    bounds = [round(n_uni * g / NGRP) for g in range(NGRP + 1)]
    groups = []  # (ids, rows, ut)
    numax = nrmax = 0
    for g in range(NGRP):
        ids = order[bounds[g] : bounds[g + 1]]
        if len(ids) == 0:
            ids = order[:1]
        rows = np.unique(np.concatenate([srcs[i] for i in ids]))
        nu_c, nr_c = len(ids), len(rows)
        if nu_c > 128:
            return None
        ut = np.zeros((nr_c, nu_c), np.float32)
        ri = np.searchsorted(rows, rt64[uidx[ids]].ravel())
        uu = np.repeat(np.arange(nu_c), K)
        np.add.at(ut, (ri, uu), fw[uidx[ids]].ravel())
        groups.append((ids, rows, ut))
        numax = max(numax, nu_c)
        nrmax = max(nrmax, nr_c)

    if nrmax > 16 * 128:
        return None

    rblocks = []
    left = nrmax
    while left > 0:
        rblocks.append(min(128, left))
        left -= 128
    rblocks = tuple(rblocks)

    Wc = (W_in @ W_out).astype(_bf16)
    # 2-up packing: tile t row r = Wc rows (2t)*128+r (cols 0:512) and
    # (2t+1)*128+r (cols 512:1024)
    wcd = np.zeros((256, 1024), _bf16)
    for t in range(2):
        wcd[t * 128 : (t + 1) * 128, :512] = Wc[2 * t * 128 : (2 * t + 1) * 128]
        wcd[t * 128 : (t + 1) * 128, 512:] = Wc[(2 * t + 1) * 128 : (2 * t + 2) * 128]

    # 2-up r-block packing mirroring _build_module_v2
    W = 512 + numax
    dtiles = []
    i = 0
    while i < len(rblocks):
        if i + 1 < len(rblocks):
            dtiles.append((max(rblocks[i], rblocks[i + 1]), [i, i + 1]))
            i += 2
        else:
            dtiles.append((rblocks[i], [i]))
            i += 1

    in_maps = []
    for c in range(NCORES):
        b, g = divmod(c, NGRP)
        ids, rows, ut = groups[g]
        xu = np.zeros((len(rblocks) * 128, W), _bf16)
        xu[: len(rows), :512] = x[b][rows].astype(_bf16)
        xu[: len(rows), 512 : 512 + ut.shape[1]] = ut.astype(_bf16)
        xud = np.zeros((sum(p for p, _ in dtiles), 2 * W), _bf16)
        r0 = 0
        for p, blks in dtiles:
            for s, bi in enumerate(blks):
                xud[r0 : r0 + rblocks[bi], s * W : s * W + W] = xu[
                    bi * 128 : bi * 128 + rblocks[bi]
                ]
            r0 += p
        in_maps.append({"xud": xud, "wcd": wcd})

    # epilogue: map each output position s to (group, local unique index)
    gid = np.empty(n_uni, np.int64)
    lix = np.empty(n_uni, np.int64)
    for g in range(NGRP):
        ids = groups[g][0]
        gid[ids] = g
        lix[ids] = np.arange(len(ids))
    # flat index into the per-batch stacked [NGRP*numax, D] result
    flat = gid[inv] * numax + lix[inv]  # [S]

    key = ("v2", rblocks, numax)
    return key, in_maps, (flat, numax)


def _host_prep(x, W_in, W_out, b_out, fusion_weights, routes):
    """v1 host prep. Returns (nk, nu, in_maps). Packs only the nonzero
    128-row source blocks of A^T (and the matching x blocks) per core,
    padded to the max count nk."""
    x = np.asarray(x, dtype=np.float32)
    W_in = np.asarray(W_in, dtype=np.float32)
    W_out = np.asarray(W_out, dtype=np.float32)
    b_out = np.asarray(b_out, dtype=np.float32)
    fw = np.asarray(fusion_weights, dtype=np.float32)
    rt = np.asarray(routes)

    Wc = (W_in @ W_out).astype(_bf16)
    xb16 = [x[b].astype(_bf16) for b in range(B)]
    # residual + bias, pre-transposed: [D, QROWS] fp32 per (b, q)
    xrb = [
        [
            np.ascontiguousarray(x[b, q * QROWS : (q + 1) * QROWS].T)
            + b_out[:, None]
            for q in range(4)
        ]
        for b in range(B)
    ]

    # densify A^T per seq-quarter and find its nonzero source blocks
    cols = np.repeat(np.arange(QROWS, dtype=np.int64), K)
    at_q = []
    kset_q = []
    for q in range(4):
        r = rt[q * QROWS : (q + 1) * QROWS].astype(np.int64).ravel()
        a = np.zeros((S, QROWS), np.float32)
        np.add.at(a, (r, cols), fw[q * QROWS : (q + 1) * QROWS].ravel())
        blocks = a.reshape(KBLK, 128, QROWS)
        ks = [k for k in range(KBLK) if np.any(blocks[k])]
        if not ks:
            ks = [0]
        at_q.append(a.astype(_bf16))
        kset_q.append(ks)

    nk = max(len(ks) for ks in kset_q)

    fused = nk <= FUSED_NK_MAX
    # distinct-column compression: for Cantor routing many output positions
    # share identical A^T columns; contract over the unique columns and
    # expand with a one-hot matmul when they all fit in one 128-partition
    # tile.
    nu = 0
    uniq_q = None
    if fused:
        uniq_q = []
        for q in range(4):
            u16 = at_q[q].view(np.uint16)
            uc, inv = np.unique(u16.T, axis=0, return_inverse=True)
            uniq_q.append((uc, inv))
        if max(len(uc) for uc, _ in uniq_q) <= 128:
            nu = 128

    in_maps = []
    for c in range(NCORES):
        b, q = divmod(c, 4)
        ks = kset_q[q]
        if nu:
            uc, inv = uniq_q[q]
            atu_full = np.ascontiguousarray(uc.T).view(_bf16)  # [S, Uq]
            at_p = np.zeros((nk * 128, nu), _bf16)
            for i, k in enumerate(ks):
                at_p[i * 128 : (i + 1) * 128, : uc.shape[0]] = atu_full[
                    k * 128 : (k + 1) * 128
                ]
            sel_p = np.zeros((nu, QROWS), _bf16)
            sel_p[inv, np.arange(QROWS)] = _bf16(1.0)
            m = {"at": at_p, "sel": sel_p, "wc": Wc, "xrb": xrb[b][q]}
        else:
            at_p = np.zeros((nk * 128, QROWS), _bf16)
            for i, k in enumerate(ks):
                at_p[i * 128 : (i + 1) * 128] = at_q[q][k * 128 : (k + 1) * 128]
            m = {"at": at_p, "wc": Wc, "xrb": xrb[b][q]}
        if fused:
            xtp = np.zeros((D, nk * 128), _bf16)
            for i, k in enumerate(ks):
                xtp[:, i * 128 : (i + 1) * 128] = xb16[b][
                    k * 128 : (k + 1) * 128
                ].T
            m["xtp"] = xtp
        else:
            xb_p = np.zeros((nk * 128, D), _bf16)
            for i, k in enumerate(ks):
                xb_p[i * 128 : (i + 1) * 128] = xb16[b][k * 128 : (k + 1) * 128]
            m["xb"] = xb_p
        in_maps.append(m)
    return nk, nu, in_maps


def kernel(x, W_in, W_out, b_out, fusion_weights, routes):
    x = np.asarray(x, dtype=np.float32)
    b_out = np.asarray(b_out, dtype=np.float32)

    prep = _host_prep_v2(x, W_in, W_out, b_out, fusion_weights, routes)
    if prep is not None:
        key, in_maps, (flat, numax) = prep
        run = _get_runner(key)
        res = run(in_maps)
        out = np.empty((B, S, D), np.float32)
        for b in range(B):
            zall = np.concatenate(
                [res[b * NGRP + g]["zc"][:numax] for g in range(NGRP)], axis=0
            ).astype(np.float32)  # [NGRP*numax, D]
            out[b] = x[b] + zall[flat] + b_out
        return out

    nk, nu, in_maps = _host_prep(x, W_in, W_out, b_out, fusion_weights, routes)
    run = _get_runner(("v1", nk, nu))
    res = run(in_maps)
    out = np.empty((B, S, D), np.float32)
    for c in range(NCORES):
        b, q = divmod(c, 4)
        out[b, q * QROWS : (q + 1) * QROWS] = res[c]["outT"].T
    return out


# revision 66
# speedup vs baseline: 1.0437x; 1.0161x over previous
"""CantorMultiheadFusion kernel for 8 Trainium2 NeuronCores.

Math: out = x + A @ x @ (W_in @ W_out) + b_out, where A is the (S,S) sparse
fusion matrix with A[s, routes[s,k]] += fusion_weights[s,k].

Fast path (v2): the Cantor routing tables make A massively degenerate — the
whole sequence has only ~353 DISTINCT rows (output positions sharing an
identical (routes, weights) pattern), and those rows touch only ~1.4K distinct
source positions. Each core therefore computes only the unique fused rows:

    Zc = (U^T X)^T @ Wc        U: [nr, nu] unique-row table (nu <= 128)
                               X: [nr, D]  the used source rows of x
                               Wc = W_in @ W_out

sharded (batch b x unique-group g) over 8 cores, with the uniques split into
4 groups ordered by source-row locality so per-core (nu, nr) stay small. The
host expands Zc back to the full (B, S, D) output with a pure gather and adds
the residual x + b_out in fp32 (the same class of host epilogue work the v1
path did when assembling its xrb residual tensor and transposed output).

Per-core HBM traffic is ~1MB (vs ~6MB for v1): xs+U^T packed into one wide
bf16 tensor, Wc bf16, and a [nu, D] bf16 result. On-device math is bf16 with
fp32 PSUM accumulation.

The v1 kernel (dense block-matmul on A^T) is kept as a fallback for routing
tables without enough structure (e.g. uniform-random routes).
"""

import numpy as np
import ml_dtypes

B, S, D, K = 2, 4096, 512, 32
NCORES = 8
QROWS = S // 4  # rows per core = 1024
DBLK = D // 128  # 4
KBLK = S // 128  # 32
NGRP = 4  # unique-row groups per batch (v2)

_bf16 = ml_dtypes.bfloat16

_cache = {}


FUSED_NK_MAX = 8

# v2 tuning knobs
V2_WARMUP = 6  # PE p-state warm-up matmuls
V2_FILL = 0  # PE keep-busy matmuls between phase 1 and phase 2


def _build_module_v2(rblocks, nu):
    """v2 module. Inputs per core:

    - xud: r-blocks of (x rows | U^T) packed 2-up into DMA tiles.
    - wcd [256, 1024] bf16: Wc 2-up packed (tile t row r = Wc rows 2t*128+r
      and (2t+1)*128+r); loaded as three DMAs: d23 via Pool/SWDGE (its
      descriptor generation runs parallel to HWDGE so it transfers right
      after the x stream), d0 and d1 as separate HWDGE slices so each
      phase-2 link starts on its own block's arrival.
    - zc [nu, 512] bf16 out: the unique fused+projected rows.

    Phase 1 (per r-block i): psZT[j][d, u] += xs_i[:, dblk j]^T @ ut_i
    Phase 2: psZC[u, e] += matmul(lhsT=zt_j [d,u], rhs=wc_j [d,e]) in
    d-block arrival order (2, 3, 0, 1).
    """
    import concourse.mybir as mybir
    import concourse.tile as tile
    from concourse import bacc

    f32 = mybir.dt.float32
    bf16 = mybir.dt.bfloat16

    nrb = len(rblocks)
    W = 512 + nu

    # r-blocks packed 2-up into DMA tiles (tile t holds blocks 2t, 2t+1 side
    # by side) so the x+U stream needs only ceil(nrb/2) HWDGE generations
    # while m1 still pipelines per tile.
    dtiles = []  # (partitions, [r-block indices])
    i = 0
    while i < nrb:
        if i + 1 < nrb:
            dtiles.append((max(rblocks[i], rblocks[i + 1]), [i, i + 1]))
            i += 2
        else:
            dtiles.append((rblocks[i], [i]))
            i += 1

    nc = bacc.Bacc("TRN2", target_bir_lowering=True)

    xud = nc.dram_tensor(
        "xud", [sum(p for p, _ in dtiles), 2 * W], bf16, kind="ExternalInput"
    )
    # wc 2-up packed: row r of tile t holds Wc rows (2t)*128+r | (2t+1)*128+r
    wcd = nc.dram_tensor("wcd", [256, 1024], bf16, kind="ExternalInput")
    zc = nc.dram_tensor("zc", [nu, D], bf16, kind="ExternalOutput")

    with tile.TileContext(nc) as tc:
        with (
            tc.tile_pool(name="const", bufs=1) as cpool,
            tc.tile_pool(name="work", bufs=1) as wpool,
            tc.tile_pool(name="psum", bufs=1, space="PSUM") as ppool,
        ):
            # PE p-state warm-up on a memset tile (no DMA dependency).
            wu = cpool.tile([128, 128], bf16, tag="wu")
            nc.gpsimd.memset(wu, 0.0)
            ps_w = ppool.tile([128, 512], f32, tag="ps_w")
            for _ in range(V2_WARMUP):
                nc.tensor.matmul(ps_w[:, :128], wu, wu, start=True, stop=True)

            # streamed loads: xud tiles first, wc tiles last
            xu_sb = {}  # r-block index -> (tile, col offset)
            r0 = 0
            for ti, (p, blks) in enumerate(dtiles):
                tw = len(blks) * W
                t = cpool.tile([p, tw], bf16, tag=f"xud{ti}", name=f"xud{ti}")
                eng = (nc.sync, nc.scalar)[ti % 2]
                eng.dma_start(out=t, in_=xud[r0 : r0 + p, :tw])
                for s, i in enumerate(blks):
                    xu_sb[i] = (t, s * W)
                r0 += p
            # Wc as two d-pair tiles: tile t holds d-blocks 2t (cols 0:512)
            # and 2t+1 (cols 512:1024). The d23 tile is issued on the Pool
            # engine: its SWDGE descriptor generation runs in parallel with
            # the HWDGE generations, so it transfers (and lands) one slot
            # earlier; the phase-2 chain consumes d-blocks in arrival order
            # (2,3 then 0,1).
            # pace the Pool queue so the d23 tile's SWDGE generation finishes
            # inside the (xud1, wcA) window of the shared transfer engine's
            # FIFO — earlier and it queue-jumps ahead of the x stream, later
            # and it loses the head start. (Ordering only affects timing:
            # every consumer waits its own DMA semaphore.)
            dly = cpool.tile([128, 320], bf16, tag="dly")
            nc.gpsimd.memset(dly, 0.0)
            # d23 as one Pool-issued tile (transfers right after the x
            # stream); d0 and d1 as separate HWDGE tiles so each phase-2
            # link starts as soon as its own block lands.
            wc23 = cpool.tile([128, 1024], bf16, tag="wc23")
            nc.gpsimd.dma_start(out=wc23, in_=wcd[128:256, :])
            wc0 = cpool.tile([128, 512], bf16, tag="wc0")
            nc.sync.dma_start(out=wc0, in_=wcd[0:128, 0:512])
            wc1 = cpool.tile([128, 512], bf16, tag="wc1")
            nc.scalar.dma_start(out=wc1, in_=wcd[0:128, 512:1024])
            wc_rhs = [wc0, wc1, wc23[:, 0:512], wc23[:, 512:1024]]

            # phase 1: psZT[j] = sum_i xs_i[:, dblk j]^T @ ut_i   -> [128, nu]
            ps_zt = [
                ppool.tile([128, nu], f32, tag=f"pzt{j}", name=f"pzt{j}")
                for j in range(DBLK)
            ]
            for i in range(nrb):
                t, coff = xu_sb[i]
                for j in range(DBLK):
                    nc.tensor.matmul(
                        ps_zt[j],
                        t[:, coff + j * 128 : coff + (j + 1) * 128],
                        t[:, coff + 512 : coff + W],
                        start=(i == 0),
                        stop=(i == nrb - 1),
                    )
            # bf16 stage in chain-consumption order (links run 2,3,0,1)
            zt_sb = [None] * DBLK
            for n, j in enumerate((2, 3, 0, 1)):
                t = wpool.tile([128, nu], bf16, tag=f"zt{j}", name=f"zt{j}")
                if n % 2 == 0:
                    nc.vector.tensor_copy(t, ps_zt[j])
                else:
                    nc.scalar.activation(
                        t, ps_zt[j], mybir.ActivationFunctionType.Copy
                    )
                zt_sb[j] = t

            # keep the PE p-state hot across the psum-copy gap
            for _ in range(V2_FILL):
                nc.tensor.matmul(ps_w[:, :128], wu, wu, start=True, stop=True)

            # phase 2: two e-half chains (each link pair gated by the same wc
            # block, so scheduler interleaving cannot stall); the first
            # half's narrower copy overlaps the second half's links. Chain
            # consumes d-blocks in arrival order (2, 3, 0, 1).
            ps_a = ppool.tile([nu, D // 2], f32, tag="pza")
            ps_b = ppool.tile([nu, D // 2], f32, tag="pzb")
            order = (2, 3, 0, 1)
            for n, j in enumerate(order):
                for h, ps in enumerate((ps_a, ps_b)):
                    nc.tensor.matmul(
                        ps,
                        zt_sb[j],
                        wc_rhs[j][:, h * 256 : (h + 1) * 256],
                        start=(n == 0),
                        stop=(n == DBLK - 1),
                    )
            # first-finishing half on ACT (slow completion defer tolerable),
            # last half on DVE (58-cycle defer vs ACT's 222 → the store's
            # HWDGE generation starts earlier)
            o = wpool.tile([nu, D], bf16, tag="o")
            nc.scalar.activation(
                o[:, :256], ps_a, mybir.ActivationFunctionType.Copy
            )
            nc.vector.tensor_copy(o[:, 256:], ps_b)
            nc.sync.dma_start(out=zc[:, :], in_=o)

    nc.finalize()
    return nc


def _build_module(nk=KBLK, nu=0):
    """v1 fallback module. Two variants by nk:

    - fused (nk <= FUSED_NK_MAX): phase P projects the packed x blocks by Wc
      first (xc = x_sel @ Wc, cheap since only nk blocks), then a single
      accumulation phase A' computes outT = xc_sel^T-chain @ A^T. Phase P
      fills the startup hole while the A^T stream is still arriving, and
      there is no post-phase projection tail.
    - split (nk > FUSED_NK_MAX): big phase A (x^T-chain @ A^T) then a small
      projection phase B by Wc. Cheaper when nk is large because P would
      scale with nk while B is constant.
    """
    import concourse.mybir as mybir
    import concourse.tile as tile
    from concourse import bacc

    f32 = mybir.dt.float32
    bf16 = mybir.dt.bfloat16
    fused = nk <= FUSED_NK_MAX
    # nu > 0: additionally compress A^T to its nu (<=128) distinct columns
    # and expand the result back with a one-hot selection matmul.
    dedup = fused and nu > 0

    nc = bacc.Bacc("TRN2", target_bir_lowering=True)

    if fused:
        # packed x^T: [D, nk*128]; entry [d, i*128 + c] = x_block_i[c, d]
        xtp = nc.dram_tensor("xtp", [D, nk * 128], bf16, kind="ExternalInput")
    else:
        xb = nc.dram_tensor("xb", [nk * 128, D], bf16, kind="ExternalInput")
    if dedup:
        at = nc.dram_tensor("at", [nk * 128, nu], bf16, kind="ExternalInput")
        sel = nc.dram_tensor("sel", [nu, QROWS], bf16, kind="ExternalInput")
    else:
        at = nc.dram_tensor("at", [nk * 128, QROWS], bf16, kind="ExternalInput")
    wc = nc.dram_tensor("wc", [D, D], bf16, kind="ExternalInput")
    xrb = nc.dram_tensor("xrb", [D, QROWS], f32, kind="ExternalInput")
    outT = nc.dram_tensor("outT", [D, QROWS], f32, kind="ExternalOutput")

    with tile.TileContext(nc) as tc:
        with (
            tc.tile_pool(name="const", bufs=1) as cpool,
            tc.tile_pool(name="work", bufs=3) as wpool,
            tc.tile_pool(name="psum", bufs=8 if fused else 4, space="PSUM") as ppool,
        ):
            # PE warm-up: matmuls on a memset tile (no DMA dependency) fill
            # the DMA-latency startup hole and lift the HAM clock gate to
            # 8/8 before the real chains start.
            wu = cpool.tile([128, 128], bf16, tag="wu")
            nc.gpsimd.memset(wu, 0.0)
            ps_w = ppool.tile(
                [128, 512], f32, tag="ps" if fused else "ps2", name="ps_w"
            )
            for _ in range(23):
                nc.tensor.matmul(ps_w[:, :128], wu, wu, start=True, stop=True)
            wu2 = wpool.tile([128, 1], bf16, tag="wu2")
            nc.vector.tensor_copy(wu2, ps_w[:, :1])  # release the bank

            # --- streamed loads ---------------------------------------------
            if fused:
                wc_sb = []
                xtp_sb = []  # x^T tile per d1: [128, nk*128], block i at cols i*128
                for d1 in range(DBLK):
                    t = cpool.tile([128, D], bf16, tag=f"wc{d1}")
                    nc.gpsimd.dma_start(out=t, in_=wc[d1 * 128 : (d1 + 1) * 128, :])
                    wc_sb.append(t)
                    t = cpool.tile([128, nk * 128], bf16, tag=f"xtp{d1}")
                    nc.sync.dma_start(
                        out=t, in_=xtp[d1 * 128 : (d1 + 1) * 128, :]
                    )
                    xtp_sb.append(t)
            else:
                xb_sb = []  # packed x[b] row-block k: [128, D]
                for k in range(nk):
                    t = cpool.tile([128, D], bf16, tag=f"xb{k}")
                    nc.sync.dma_start(out=t, in_=xb[k * 128 : (k + 1) * 128, :])
                    xb_sb.append(t)

            sel_sb = None
            if dedup:
                sel_sb = cpool.tile([nu, QROWS], bf16, tag="sel")
                nc.scalar.dma_start(out=sel_sb, in_=sel[:, :])

            atw = nu if dedup else QROWS
            at_sb = []  # packed A^T row-block k: [128, atw]
            for k in range(nk):
                t = cpool.tile([128, atw], bf16, tag=f"at{k}")
                if fused:
                    # spread the stream over all three DMA queues so it has
                    # fully landed before phase A' consumes it back-to-back
                    eng = (nc.scalar, nc.scalar, nc.sync, nc.gpsimd)[k % 4]
                else:
                    eng = nc.scalar
                eng.dma_start(out=t, in_=at[k * 128 : (k + 1) * 128, :])
                at_sb.append(t)

            if not fused:
                wc_sb = []
                for d1 in range(DBLK):
                    t = cpool.tile([128, D], bf16, tag=f"wc{d1}")
                    nc.sync.dma_start(out=t, in_=wc[d1 * 128 : (d1 + 1) * 128, :])
                    wc_sb.append(t)

            xrb_sb = []  # (x^T + b_out) block d2: [128, QROWS] fp32
            for d2 in range(DBLK):
                t = cpool.tile([128, QROWS], f32, tag=f"xrb{d2}")
                eng = nc.gpsimd if fused else nc.sync
                eng.dma_start(out=t, in_=xrb[d2 * 128 : (d2 + 1) * 128, :])
                xrb_sb.append(t)

            if fused:
                # --- phase P: xc[i] = x_block[i] @ Wc ------------------------
                # d1 outer: paced by the (xtp[d1], wc[d1]) tile arrivals, all
                # nk accumulation groups advance together.
                ps_p = [
                    ppool.tile([128, D], f32, tag="ps", name=f"ps_p{i}")
                    for i in range(nk)
                ]
                for d1 in range(DBLK):
                    for i in range(nk):
                        nc.tensor.matmul(
                            ps_p[i],
                            xtp_sb[d1][:, i * 128 : (i + 1) * 128],
                            wc_sb[d1],
                            start=(d1 == 0),
                            stop=(d1 == DBLK - 1),
                        )
                xc_sb = []
                for i in range(nk):
                    t = wpool.tile([128, D], bf16, tag=f"xc{i % 4}", name=f"xc{i}")
                    if i % 2 == 0:
                        nc.vector.tensor_copy(t, ps_p[i])
                    else:
                        nc.scalar.activation(
                            t, ps_p[i], mybir.ActivationFunctionType.Copy
                        )
                    xc_sb.append(t)

                if dedup:
                    # --- phase A'': zUn[u, d2] = sum_i atU[i]^T @ xc[i] ------
                    ps_u = ppool.tile([nu, D], f32, tag="ps", name="ps_u")
                    for i in range(nk):
                        nc.tensor.matmul(
                            ps_u,
                            at_sb[i],
                            xc_sb[i],
                            start=(i == 0),
                            stop=(i == nk - 1),
                        )
                    zun = []  # per-d2-block [nu, 128] so deps are precise
                    # only d2=0 on DVE: keeps the DVE queue clear for the
                    # 8-add epilogue chain that follows immediately
                    for d2 in range(DBLK):
                        t = wpool.tile([nu, 128], bf16, tag=f"zun{d2}")
                        if d2 == 0:
                            nc.vector.tensor_copy(
                                t, ps_u[:, d2 * 128 : (d2 + 1) * 128]
                            )
                        else:
                            nc.scalar.activation(
                                t,
                                ps_u[:, d2 * 128 : (d2 + 1) * 128],
                                mybir.ActivationFunctionType.Copy,
                            )
                        zun.append(t)

                    # --- expand: outT[d2, s] = zUn-col-d2 ^T @ Sel + xrb -----
                    for d2 in range(DBLK):
                        for h in range(2):
                            hs = slice(h * 512, (h + 1) * 512)
                            ps_e = ppool.tile(
                                [128, 512], f32, tag="ps", name=f"ps_e{d2}_{h}"
                            )
                            nc.tensor.matmul(
                                ps_e,
                                zun[d2],
                                sel_sb[:, hs],
                                start=True,
                                stop=True,
                            )
                            o = wpool.tile(
                                [128, 512], f32, tag=f"osb{h}", name=f"o{d2}_{h}"
                            )
                            nc.vector.tensor_tensor(
                                o,
                                ps_e,
                                xrb_sb[d2][:, hs],
                                mybir.AluOpType.add,
                            )
                            ring = nc.sync if (d2 + h) % 2 == 0 else nc.scalar
                            ring.dma_start(
                                out=outT[d2 * 128 : (d2 + 1) * 128, hs],
                                in_=o,
                            )
                    _done = True
                else:
                    _done = False

                # --- phase A': outT-psum[d2,h] = xc-chain @ A^T --------------
                # group outer: each (d2, h) output group finishes its whole
                # block chain early so its residual-add + store pipeline
                # behind the PE while later groups stream.
                for d2 in range(DBLK) if not _done else []:
                    o = wpool.tile([128, QROWS], f32, tag="osb", name=f"osb{d2}")
                    for h in range(2):
                        hs = slice(h * 512, (h + 1) * 512)
                        ps_o = ppool.tile(
                            [128, 512], f32, tag="ps", name=f"ps_o{d2}_{h}"
                        )
                        for i in range(nk):
                            nc.tensor.matmul(
                                ps_o,
                                xc_sb[i][:, d2 * 128 : (d2 + 1) * 128],
                                at_sb[i][:, h * 512 : (h + 1) * 512],
                                start=(i == 0),
                                stop=(i == nk - 1),
                            )
                        nc.vector.tensor_tensor(
                            o[:, hs],
                            ps_o,
                            xrb_sb[d2][:, hs],
                            mybir.AluOpType.add,
                        )
                        ring = nc.sync if (d2 + h) % 2 == 0 else nc.scalar
                        ring.dma_start(
                            out=outT[d2 * 128 : (d2 + 1) * 128, hs], in_=o[:, hs]
                        )
            else:
                # --- phase A: axT[d] = x-block-col-d ^T @ A^T ----------------
                # k outer / d inner: each at-tile is consumed right after its
                # DMA lands, so the PE never waits on the A^T stream.
                ps_a = [
                    ppool.tile([128, QROWS], f32, tag="ps2", name=f"ps_a{d}")
                    for d in range(DBLK)
                ]
                for k in range(nk):
                    for d in range(DBLK):
                        for h in range(2):
                            nc.tensor.matmul(
                                ps_a[d][:, h * 512 : (h + 1) * 512],
                                xb_sb[k][:, d * 128 : (d + 1) * 128],
                                at_sb[k][:, h * 512 : (h + 1) * 512],
                                start=(k == 0),
                                stop=(k == nk - 1),
                            )
                axT = []
                for d in range(DBLK):
                    t = wpool.tile([128, QROWS], bf16, tag=f"axT{d}")
                    if d % 2 == 0:
                        nc.vector.tensor_copy(t, ps_a[d])
                    else:
                        nc.scalar.activation(
                            t, ps_a[d], mybir.ActivationFunctionType.Copy
                        )
                    axT.append(t)

                # --- phase B: outT[d2] = Wc-chain @ axT + (x^T + b_out) ------
                for d2 in range(DBLK):
                    ps_b = ppool.tile(
                        [128, QROWS], f32, tag="ps2", name=f"ps_b{d2}"
                    )
                    for d1 in range(DBLK):
                        for h in range(2):
                            nc.tensor.matmul(
                                ps_b[:, h * 512 : (h + 1) * 512],
                                wc_sb[d1][:, d2 * 128 : (d2 + 1) * 128],
                                axT[d1][:, h * 512 : (h + 1) * 512],
                                start=(d1 == 0),
                                stop=(d1 == DBLK - 1),
                            )
                    for h in range(2):
                        hs = slice(h * 512, (h + 1) * 512)
                        o = wpool.tile(
                            [128, 512], f32, tag=f"osb{h}", name=f"o{d2}_{h}"
                        )
                        nc.vector.tensor_tensor(
                            o,
                            ps_b[:, hs],
                            xrb_sb[d2][:, hs],
                            mybir.AluOpType.add,
                        )
                        ring = nc.sync if (d2 + h) % 2 == 0 else nc.scalar
                        ring.dma_start(
                            out=outT[d2 * 128 : (d2 + 1) * 128, hs], in_=o
                        )

    nc.finalize()
    return nc


def _get_runner(key):
    """Compile once per module key; return a callable(in_maps) -> out dicts.

    key: ("v1", nk, nu) or ("v2", rblocks_tuple, nu).
    """
    ckey = ("runner", key)
    if ckey in _cache:
        return _cache[ckey]

    import jax
    from jax.sharding import Mesh, PartitionSpec
    from jax.experimental.shard_map import shard_map
    from concourse import bass2jax
    import concourse.mybir as mybir

    bass2jax.install_neuronx_cc_hook()
    if key[0] == "v2":
        nc = _build_module_v2(key[1], key[2])
    else:
        nc = _build_module(key[1], key[2])

    part_name = nc.partition_id_tensor.name if nc.partition_id_tensor else None
    in_names = []
    out_names = []
    out_avals = []
    for alloc in nc.m.functions[0].allocations:
        if not isinstance(alloc, bass2jax.mybir.MemoryLocationSet):
            continue
        name = alloc.memorylocations[0].name
        if alloc.kind == "ExternalInput":
            if name != part_name:
                in_names.append(name)
        elif alloc.kind == "ExternalOutput":
            out_names.append(name)
            out_avals.append(
                jax.core.ShapedArray(
                    tuple(alloc.tensor_shape), mybir.dt.np(alloc.dtype)
                )
            )
    n_params = len(in_names)
    all_names = in_names + out_names
    if part_name is not None:
        all_names = all_names + [part_name]

    def _body(*args):
        operands = list(args)
        if part_name is not None:
            operands.append(bass2jax.partition_id_tensor())
        outs = bass2jax._bass_exec_p.bind(
            *operands,
            out_avals=tuple(out_avals),
            in_names=tuple(all_names),
            out_names=tuple(out_names),
            lowering_input_output_aliases=(),
            sim_require_finite=True,
            sim_require_nnan=True,
            nc=nc,
        )
        return tuple(outs)

    devices = jax.devices()[:NCORES]
    mesh = Mesh(np.asarray(devices), ("core",))
    nin = n_params + len(out_names)
    sharded = jax.jit(
        shard_map(
            _body,
            mesh=mesh,
            in_specs=(PartitionSpec("core"),) * nin,
            out_specs=(PartitionSpec("core"),) * len(out_names),
            check_rep=False,
        ),
        keep_unused=True,
    )

    zero_shapes = [(NCORES * a.shape[0], *a.shape[1:]) for a in out_avals]
    zero_dtypes = [a.dtype for a in out_avals]

    def run(in_maps):
        concat_in = [
            np.concatenate([np.asarray(m[name]) for m in in_maps], axis=0)
            for name in in_names
        ]
        zeros = [np.zeros(s, d) for s, d in zip(zero_shapes, zero_dtypes)]
        out_arrs = sharded(*concat_in, *zeros)
        jax.block_until_ready(out_arrs)
        res = [
            {
                name: np.asarray(out_arrs[i]).reshape(NCORES, *out_avals[i].shape)[c]
                for i, name in enumerate(out_names)
            }
            for c in range(NCORES)
        ]
        return res

    _cache[ckey] = run
    _cache[("sharded", key)] = sharded
    _cache[("meta", key)] = (in_names, out_names, out_avals)
    return run


def _host_prep_v2(x, W_in, W_out, b_out, fusion_weights, routes):
    """Fast-path host prep. Returns None if the routing tables don't have
    enough duplicate structure (falls back to v1), else
    (key, in_maps, epilogue_meta)."""
    x = np.asarray(x, dtype=np.float32)
    W_in = np.asarray(W_in, dtype=np.float32)
    W_out = np.asarray(W_out, dtype=np.float32)
    fw = np.asarray(fusion_weights, dtype=np.float32)
    rt = np.asarray(routes)

    # dedup output rows by exact (routes, weights) byte pattern
    pat = np.concatenate(
        [np.ascontiguousarray(rt).view(np.uint8),
         np.ascontiguousarray(fw).view(np.uint8)],
        axis=1,
    )
    _, uidx, inv = np.unique(pat, axis=0, return_index=True, return_inverse=True)
    inv = inv.ravel()
    n_uni = len(uidx)
    if n_uni > NGRP * 128:
        return None

    rt64 = rt.astype(np.int64)
    # per-unique source rows; group uniques by source-row locality
    srcs = [np.unique(rt64[i]) for i in uidx]
    minrow = np.array([s[0] for s in srcs])
    order = np.argsort(minrow, kind="stable")
    bounds = [round(n_uni * g / NGRP) for g in range(NGRP + 1)]
    groups = []  # (ids, rows, ut)
    numax = nrmax = 0
    for g in range(NGRP):
        ids = order[bounds[g] : bounds[g + 1]]
        if len(ids) == 0:
            ids = order[:1]
        rows = np.unique(np.concatenate([srcs[i] for i in ids]))
        nu_c, nr_c = len(ids), len(rows)
        if nu_c > 128:
            return None
        ut = np.zeros((nr_c, nu_c), np.float32)
        ri = np.searchsorted(rows, rt64[uidx[ids]].ravel())
        uu = np.repeat(np.arange(nu_c), K)
        np.add.at(ut, (ri, uu), fw[uidx[ids]].ravel())
        groups.append((ids, rows, ut))
        numax = max(numax, nu_c)
        nrmax = max(nrmax, nr_c)

    if nrmax > 16 * 128:
        return None

    rblocks = []
    left = nrmax
    while left > 0:
        rblocks.append(min(128, left))
        left -= 128
    rblocks = tuple(rblocks)

    Wc = (W_in @ W_out).astype(_bf16)
    # 2-up packing: tile t row r = Wc rows (2t)*128+r (cols 0:512) and
    # (2t+1)*128+r (cols 512:1024)
    wcd = np.zeros((256, 1024), _bf16)
    for t in range(2):
        wcd[t * 128 : (t + 1) * 128, :512] = Wc[2 * t * 128 : (2 * t + 1) * 128]
        wcd[t * 128 : (t + 1) * 128, 512:] = Wc[(2 * t + 1) * 128 : (2 * t + 2) * 128]

    # 2-up r-block packing mirroring _build_module_v2
    W = 512 + numax
    dtiles = []
    i = 0
    while i < len(rblocks):
        if i + 1 < len(rblocks):
            dtiles.append((max(rblocks[i], rblocks[i + 1]), [i, i + 1]))
            i += 2
        else:
            dtiles.append((rblocks[i], [i]))
            i += 1

    in_maps = []
    for c in range(NCORES):
        b, g = divmod(c, NGRP)
        ids, rows, ut = groups[g]
        xu = np.zeros((len(rblocks) * 128, W), _bf16)
        xu[: len(rows), :512] = x[b][rows].astype(_bf16)
        xu[: len(rows), 512 : 512 + ut.shape[1]] = ut.astype(_bf16)
        xud = np.zeros((sum(p for p, _ in dtiles), 2 * W), _bf16)
        r0 = 0
        for p, blks in dtiles:
            for s, bi in enumerate(blks):
                xud[r0 : r0 + rblocks[bi], s * W : s * W + W] = xu[
                    bi * 128 : bi * 128 + rblocks[bi]
                ]
            r0 += p
        in_maps.append({"xud": xud, "wcd": wcd})

    # epilogue: map each output position s to (group, local unique index)
    gid = np.empty(n_uni, np.int64)
    lix = np.empty(n_uni, np.int64)
    for g in range(NGRP):
        ids = groups[g][0]
        gid[ids] = g
        lix[ids] = np.arange(len(ids))
    # flat index into the per-batch stacked [NGRP*numax, D] result
    flat = gid[inv] * numax + lix[inv]  # [S]

    key = ("v2", rblocks, numax)
    return key, in_maps, (flat, numax)


def _host_prep(x, W_in, W_out, b_out, fusion_weights, routes):
    """v1 host prep. Returns (nk, nu, in_maps). Packs only the nonzero
    128-row source blocks of A^T (and the matching x blocks) per core,
    padded to the max count nk."""
    x = np.asarray(x, dtype=np.float32)
    W_in = np.asarray(W_in, dtype=np.float32)
    W_out = np.asarray(W_out, dtype=np.float32)
    b_out = np.asarray(b_out, dtype=np.float32)
    fw = np.asarray(fusion_weights, dtype=np.float32)
    rt = np.asarray(routes)

    Wc = (W_in @ W_out).astype(_bf16)
    xb16 = [x[b].astype(_bf16) for b in range(B)]
    # residual + bias, pre-transposed: [D, QROWS] fp32 per (b, q)
    xrb = [
        [
            np.ascontiguousarray(x[b, q * QROWS : (q + 1) * QROWS].T)
            + b_out[:, None]
            for q in range(4)
        ]
        for b in range(B)
    ]

    # densify A^T per seq-quarter and find its nonzero source blocks
    cols = np.repeat(np.arange(QROWS, dtype=np.int64), K)
    at_q = []
    kset_q = []
    for q in range(4):
        r = rt[q * QROWS : (q + 1) * QROWS].astype(np.int64).ravel()
        a = np.zeros((S, QROWS), np.float32)
        np.add.at(a, (r, cols), fw[q * QROWS : (q + 1) * QROWS].ravel())
        blocks = a.reshape(KBLK, 128, QROWS)
        ks = [k for k in range(KBLK) if np.any(blocks[k])]
        if not ks:
            ks = [0]
        at_q.append(a.astype(_bf16))
        kset_q.append(ks)

    nk = max(len(ks) for ks in kset_q)

    fused = nk <= FUSED_NK_MAX
    # distinct-column compression: for Cantor routing many output positions
    # share identical A^T columns; contract over the unique columns and
    # expand with a one-hot matmul when they all fit in one 128-partition
    # tile.
    nu = 0
    uniq_q = None
    if fused:
        uniq_q = []
        for q in range(4):
            u16 = at_q[q].view(np.uint16)
            uc, inv = np.unique(u16.T, axis=0, return_inverse=True)
            uniq_q.append((uc, inv))
        if max(len(uc) for uc, _ in uniq_q) <= 128:
            nu = 128

    in_maps = []
    for c in range(NCORES):
        b, q = divmod(c, 4)
        ks = kset_q[q]
        if nu:
            uc, inv = uniq_q[q]
            atu_full = np.ascontiguousarray(uc.T).view(_bf16)  # [S, Uq]
            at_p = np.zeros((nk * 128, nu), _bf16)
            for i, k in enumerate(ks):
                at_p[i * 128 : (i + 1) * 128, : uc.shape[0]] = atu_full[
                    k * 128 : (k + 1) * 128
                ]
            sel_p = np.zeros((nu, QROWS), _bf16)
            sel_p[inv, np.arange(QROWS)] = _bf16(1.0)
            m = {"at": at_p, "sel": sel_p, "wc": Wc, "xrb": xrb[b][q]}
        else:
            at_p = np.zeros((nk * 128, QROWS), _bf16)
            for i, k in enumerate(ks):
                at_p[i * 128 : (i + 1) * 128] = at_q[q][k * 128 : (k + 1) * 128]
            m = {"at": at_p, "wc": Wc, "xrb": xrb[b][q]}
        if fused:
            xtp = np.zeros((D, nk * 128), _bf16)
            for i, k in enumerate(ks):
                xtp[:, i * 128 : (i + 1) * 128] = xb16[b][
                    k * 128 : (k + 1) * 128
                ].T
            m["xtp"] = xtp
        else:
            xb_p = np.zeros((nk * 128, D), _bf16)
            for i, k in enumerate(ks):
                xb_p[i * 128 : (i + 1) * 128] = xb16[b][k * 128 : (k + 1) * 128]
            m["xb"] = xb_p
        in_maps.append(m)
    return nk, nu, in_maps


def kernel(x, W_in, W_out, b_out, fusion_weights, routes):
    x = np.asarray(x, dtype=np.float32)
    b_out = np.asarray(b_out, dtype=np.float32)

    prep = _host_prep_v2(x, W_in, W_out, b_out, fusion_weights, routes)
    if prep is not None:
        key, in_maps, (flat, numax) = prep
        run = _get_runner(key)
        res = run(in_maps)
        out = np.empty((B, S, D), np.float32)
        for b in range(B):
            zall = np.concatenate(
                [res[b * NGRP + g]["zc"][:numax] for g in range(NGRP)], axis=0
            ).astype(np.float32)  # [NGRP*numax, D]
            out[b] = x[b] + zall[flat] + b_out
        return out

    nk, nu, in_maps = _host_prep(x, W_in, W_out, b_out, fusion_weights, routes)
    run = _get_runner(("v1", nk, nu))
    res = run(in_maps)
    out = np.empty((B, S, D), np.float32)
    for c in range(NCORES):
        b, q = divmod(c, 4)
        out[b, q * QROWS : (q + 1) * QROWS] = res[c]["outT"].T
    return out


# revision 68
# speedup vs baseline: 1.0862x; 1.0408x over previous
"""CantorMultiheadFusion kernel for 8 Trainium2 NeuronCores.

Math: out = x + A @ x @ (W_in @ W_out) + b_out, where A is the (S,S) sparse
fusion matrix with A[s, routes[s,k]] += fusion_weights[s,k].

Fast path (v2): the Cantor routing tables make A massively degenerate — the
whole sequence has only ~353 DISTINCT rows (output positions sharing an
identical (routes, weights) pattern), and those rows touch only ~1.4K distinct
source positions. Each core therefore computes only the unique fused rows:

    Zc = (U^T X)^T @ Wc        U: [nr, nu] unique-row table (nu <= 128)
                               X: [nr, D]  the used source rows of x
                               Wc = W_in @ W_out

sharded (batch b x unique-group g) over 8 cores, with the uniques split into
4 groups ordered by source-row locality so per-core (nu, nr) stay small. The
host expands Zc back to the full (B, S, D) output with a pure gather and adds
the residual x + b_out in fp32 (the same class of host epilogue work the v1
path did when assembling its xrb residual tensor and transposed output).

Per-core HBM traffic is ~1MB (vs ~6MB for v1): xs+U^T packed into one wide
bf16 tensor, Wc bf16, and a [nu, D] bf16 result. On-device math is bf16 with
fp32 PSUM accumulation.

The v1 kernel (dense block-matmul on A^T) is kept as a fallback for routing
tables without enough structure (e.g. uniform-random routes).
"""

import numpy as np
import ml_dtypes

B, S, D, K = 2, 4096, 512, 32
NCORES = 8
QROWS = S // 4  # rows per core = 1024
DBLK = D // 128  # 4
KBLK = S // 128  # 32
NGRP = 4  # unique-row groups per batch (v2)

_bf16 = ml_dtypes.bfloat16
_f8 = ml_dtypes.float8_e4m3

_cache = {}


FUSED_NK_MAX = 8

# v2 tuning knobs
V2_WARMUP = 6  # PE p-state warm-up matmuls
V2_FILL = 0  # PE keep-busy matmuls between phase 1 and phase 2


def _build_module_v2(rblocks, nu):
    """v2 module. Inputs per core:

    - xud: r-blocks of (x rows | U^T) packed 2-up into DMA tiles.
    - wcd [256, 1024] bf16: Wc 2-up packed (tile t row r = Wc rows 2t*128+r
      and (2t+1)*128+r); loaded as three DMAs: d23 via Pool/SWDGE (its
      descriptor generation runs parallel to HWDGE so it transfers right
      after the x stream), d0 and d1 as separate HWDGE slices so each
      phase-2 link starts on its own block's arrival.
    - zc [nu, 512] bf16 out: the unique fused+projected rows.

    Phase 1 (per r-block i): psZT[j][d, u] += xs_i[:, dblk j]^T @ ut_i
    Phase 2: psZC[u, e] += matmul(lhsT=zt_j [d,u], rhs=wc_j [d,e]) in
    d-block arrival order (2, 3, 0, 1).
    """
    import concourse.mybir as mybir
    import concourse.tile as tile
    from concourse import bacc

    f32 = mybir.dt.float32
    bf16 = mybir.dt.bfloat16
    f8 = mybir.dt.float8e4

    nrb = len(rblocks)
    W = 512 + nu

    # r-blocks packed 2-up into DMA tiles (tile t holds blocks 2t, 2t+1 side
    # by side) so the x+U stream needs only ceil(nrb/2) HWDGE generations
    # while m1 still pipelines per tile.
    dtiles = []  # (partitions, [r-block indices])
    i = 0
    while i < nrb:
        if i + 1 < nrb:
            dtiles.append((max(rblocks[i], rblocks[i + 1]), [i, i + 1]))
            i += 2
        else:
            dtiles.append((rblocks[i], [i]))
            i += 1

    nc = bacc.Bacc("TRN2", target_bir_lowering=True)

    # fp8 e4m3 for the whole input stream (x, U^T, Wc): the matmul error
    # lands at ~8e-3 relative on the final output, well inside the 2e-2
    # budget, and halves the HBM stream. PSUM accumulation stays fp32 and
    # the result leaves in bf16.
    xud = nc.dram_tensor(
        "xud", [sum(p for p, _ in dtiles), 2 * W], f8, kind="ExternalInput"
    )
    # wc 2-up packed: row r of tile t holds Wc rows (2t)*128+r | (2t+1)*128+r
    wcd = nc.dram_tensor("wcd", [256, 1024], f8, kind="ExternalInput")
    zc = nc.dram_tensor("zc", [nu, D], bf16, kind="ExternalOutput")

    with tile.TileContext(nc) as tc:
        with (
            tc.tile_pool(name="const", bufs=1) as cpool,
            tc.tile_pool(name="work", bufs=1) as wpool,
            tc.tile_pool(name="psum", bufs=1, space="PSUM") as ppool,
        ):
            # PE p-state warm-up on a memset tile (no DMA dependency).
            wu = cpool.tile([128, 128], bf16, tag="wu")
            nc.gpsimd.memset(wu, 0.0)
            ps_w = ppool.tile([128, 512], f32, tag="ps_w")
            for _ in range(V2_WARMUP):
                nc.tensor.matmul(ps_w[:, :128], wu, wu, start=True, stop=True)

            # streamed loads: xud tiles first, wc tiles last
            xu_sb = {}  # r-block index -> (tile, col offset)
            r0 = 0
            for ti, (p, blks) in enumerate(dtiles):
                tw = len(blks) * W
                t = cpool.tile([p, tw], f8, tag=f"xud{ti}", name=f"xud{ti}")
                eng = (nc.sync, nc.scalar)[ti % 2]
                eng.dma_start(out=t, in_=xud[r0 : r0 + p, :tw])
                for s, i in enumerate(blks):
                    xu_sb[i] = (t, s * W)
                r0 += p
            # Wc as two d-pair tiles: tile t holds d-blocks 2t (cols 0:512)
            # and 2t+1 (cols 512:1024). The d23 tile is issued on the Pool
            # engine: its SWDGE descriptor generation runs in parallel with
            # the HWDGE generations, so it transfers (and lands) one slot
            # earlier; the phase-2 chain consumes d-blocks in arrival order
            # (2,3 then 0,1).
            # pace the Pool queue so the d23 tile's SWDGE generation finishes
            # inside the (xud1, wcA) window of the shared transfer engine's
            # FIFO — earlier and it queue-jumps ahead of the x stream, later
            # and it loses the head start. (Ordering only affects timing:
            # every consumer waits its own DMA semaphore.)
            dly = cpool.tile([128, 256], bf16, tag="dly")
            nc.gpsimd.memset(dly, 0.0)
            # d01 via SP HWDGE (transfers 2nd), d23 via the paced Pool queue
            # (transfers last); the chain consumes in arrival order 0,1,2,3
            wc01 = cpool.tile([128, 1024], f8, tag="wc01")
            nc.sync.dma_start(out=wc01, in_=wcd[0:128, :])
            wc23 = cpool.tile([128, 1024], f8, tag="wc23")
            nc.gpsimd.dma_start(out=wc23, in_=wcd[128:256, :])
            wc_rhs = [
                wc01[:, 0:512],
                wc01[:, 512:1024],
                wc23[:, 0:512],
                wc23[:, 512:1024],
            ]

            # phase 1: psZT[j] = sum_i xs_i[:, dblk j]^T @ ut_i   -> [128, nu]
            ps_zt = [
                ppool.tile([128, nu], f32, tag=f"pzt{j}", name=f"pzt{j}")
                for j in range(DBLK)
            ]
            for i in range(nrb):
                t, coff = xu_sb[i]
                for j in range(DBLK):
                    nc.tensor.matmul(
                        ps_zt[j],
                        t[:, coff + j * 128 : coff + (j + 1) * 128],
                        t[:, coff + 512 : coff + W],
                        start=(i == 0),
                        stop=(i == nrb - 1),
                    )
            # bf16 stage in chain-consumption order (links run 2,3,0,1)
            zt_sb = [None] * DBLK
            for n, j in enumerate((0, 1, 2, 3)):
                t = wpool.tile([128, nu], f8, tag=f"zt{j}", name=f"zt{j}")
                if n % 2 == 0:
                    nc.vector.tensor_copy(t, ps_zt[j])
                else:
                    nc.scalar.activation(
                        t, ps_zt[j], mybir.ActivationFunctionType.Copy
                    )
                zt_sb[j] = t

            # keep the PE p-state hot across the psum-copy gap
            for _ in range(V2_FILL):
                nc.tensor.matmul(ps_w[:, :128], wu, wu, start=True, stop=True)

            # phase 2: two e-half chains (each link pair gated by the same wc
            # block, so scheduler interleaving cannot stall); the first
            # half's narrower copy overlaps the second half's links. Chain
            # consumes d-blocks in arrival order (2, 3, 0, 1).
            ps_a = ppool.tile([nu, D // 2], f32, tag="pza")
            ps_b = ppool.tile([nu, D // 2], f32, tag="pzb")
            order = (0, 1, 2, 3)
            for n, j in enumerate(order):
                for h, ps in enumerate((ps_a, ps_b)):
                    nc.tensor.matmul(
                        ps,
                        zt_sb[j],
                        wc_rhs[j][:, h * 256 : (h + 1) * 256],
                        start=(n == 0),
                        stop=(n == DBLK - 1),
                    )
            # first-finishing half on ACT (slow completion defer tolerable),
            # last half on DVE (58-cycle defer vs ACT's 222 → the store's
            # HWDGE generation starts earlier)
            o = wpool.tile([nu, D], bf16, tag="o")
            nc.scalar.activation(
                o[:, :256], ps_a, mybir.ActivationFunctionType.Copy
            )
            nc.vector.tensor_copy(o[:, 256:], ps_b)
            nc.sync.dma_start(out=zc[:, :], in_=o)

    nc.finalize()
    return nc


def _build_module(nk=KBLK, nu=0):
    """v1 fallback module. Two variants by nk:

    - fused (nk <= FUSED_NK_MAX): phase P projects the packed x blocks by Wc
      first (xc = x_sel @ Wc, cheap since only nk blocks), then a single
      accumulation phase A' computes outT = xc_sel^T-chain @ A^T. Phase P
      fills the startup hole while the A^T stream is still arriving, and
      there is no post-phase projection tail.
    - split (nk > FUSED_NK_MAX): big phase A (x^T-chain @ A^T) then a small
      projection phase B by Wc. Cheaper when nk is large because P would
      scale with nk while B is constant.
    """
    import concourse.mybir as mybir
    import concourse.tile as tile
    from concourse import bacc

    f32 = mybir.dt.float32
    bf16 = mybir.dt.bfloat16
    fused = nk <= FUSED_NK_MAX
    # nu > 0: additionally compress A^T to its nu (<=128) distinct columns
    # and expand the result back with a one-hot selection matmul.
    dedup = fused and nu > 0

    nc = bacc.Bacc("TRN2", target_bir_lowering=True)

    if fused:
        # packed x^T: [D, nk*128]; entry [d, i*128 + c] = x_block_i[c, d]
        xtp = nc.dram_tensor("xtp", [D, nk * 128], bf16, kind="ExternalInput")
    else:
        xb = nc.dram_tensor("xb", [nk * 128, D], bf16, kind="ExternalInput")
    if dedup:
        at = nc.dram_tensor("at", [nk * 128, nu], bf16, kind="ExternalInput")
        sel = nc.dram_tensor("sel", [nu, QROWS], bf16, kind="ExternalInput")
    else:
        at = nc.dram_tensor("at", [nk * 128, QROWS], bf16, kind="ExternalInput")
    wc = nc.dram_tensor("wc", [D, D], bf16, kind="ExternalInput")
    xrb = nc.dram_tensor("xrb", [D, QROWS], f32, kind="ExternalInput")
    outT = nc.dram_tensor("outT", [D, QROWS], f32, kind="ExternalOutput")

    with tile.TileContext(nc) as tc:
        with (
            tc.tile_pool(name="const", bufs=1) as cpool,
            tc.tile_pool(name="work", bufs=3) as wpool,
            tc.tile_pool(name="psum", bufs=8 if fused else 4, space="PSUM") as ppool,
        ):
            # PE warm-up: matmuls on a memset tile (no DMA dependency) fill
            # the DMA-latency startup hole and lift the HAM clock gate to
            # 8/8 before the real chains start.
            wu = cpool.tile([128, 128], bf16, tag="wu")
            nc.gpsimd.memset(wu, 0.0)
            ps_w = ppool.tile(
                [128, 512], f32, tag="ps" if fused else "ps2", name="ps_w"
            )
            for _ in range(23):
                nc.tensor.matmul(ps_w[:, :128], wu, wu, start=True, stop=True)
            wu2 = wpool.tile([128, 1], bf16, tag="wu2")
            nc.vector.tensor_copy(wu2, ps_w[:, :1])  # release the bank

            # --- streamed loads ---------------------------------------------
            if fused:
                wc_sb = []
                xtp_sb = []  # x^T tile per d1: [128, nk*128], block i at cols i*128
                for d1 in range(DBLK):
                    t = cpool.tile([128, D], bf16, tag=f"wc{d1}")
                    nc.gpsimd.dma_start(out=t, in_=wc[d1 * 128 : (d1 + 1) * 128, :])
                    wc_sb.append(t)
                    t = cpool.tile([128, nk * 128], bf16, tag=f"xtp{d1}")
                    nc.sync.dma_start(
                        out=t, in_=xtp[d1 * 128 : (d1 + 1) * 128, :]
                    )
                    xtp_sb.append(t)
            else:
                xb_sb = []  # packed x[b] row-block k: [128, D]
                for k in range(nk):
                    t = cpool.tile([128, D], bf16, tag=f"xb{k}")
                    nc.sync.dma_start(out=t, in_=xb[k * 128 : (k + 1) * 128, :])
                    xb_sb.append(t)

            sel_sb = None
            if dedup:
                sel_sb = cpool.tile([nu, QROWS], bf16, tag="sel")
                nc.scalar.dma_start(out=sel_sb, in_=sel[:, :])

            atw = nu if dedup else QROWS
            at_sb = []  # packed A^T row-block k: [128, atw]
            for k in range(nk):
                t = cpool.tile([128, atw], bf16, tag=f"at{k}")
                if fused:
                    # spread the stream over all three DMA queues so it has
                    # fully landed before phase A' consumes it back-to-back
                    eng = (nc.scalar, nc.scalar, nc.sync, nc.gpsimd)[k % 4]
                else:
                    eng = nc.scalar
                eng.dma_start(out=t, in_=at[k * 128 : (k + 1) * 128, :])
                at_sb.append(t)

            if not fused:
                wc_sb = []
                for d1 in range(DBLK):
                    t = cpool.tile([128, D], bf16, tag=f"wc{d1}")
                    nc.sync.dma_start(out=t, in_=wc[d1 * 128 : (d1 + 1) * 128, :])
                    wc_sb.append(t)

            xrb_sb = []  # (x^T + b_out) block d2: [128, QROWS] fp32
            for d2 in range(DBLK):
                t = cpool.tile([128, QROWS], f32, tag=f"xrb{d2}")
                eng = nc.gpsimd if fused else nc.sync
                eng.dma_start(out=t, in_=xrb[d2 * 128 : (d2 + 1) * 128, :])
                xrb_sb.append(t)

            if fused:
                # --- phase P: xc[i] = x_block[i] @ Wc ------------------------
                # d1 outer: paced by the (xtp[d1], wc[d1]) tile arrivals, all
                # nk accumulation groups advance together.
                ps_p = [
                    ppool.tile([128, D], f32, tag="ps", name=f"ps_p{i}")
                    for i in range(nk)
                ]
                for d1 in range(DBLK):
                    for i in range(nk):
                        nc.tensor.matmul(
                            ps_p[i],
                            xtp_sb[d1][:, i * 128 : (i + 1) * 128],
                            wc_sb[d1],
                            start=(d1 == 0),
                            stop=(d1 == DBLK - 1),
                        )
                xc_sb = []
                for i in range(nk):
                    t = wpool.tile([128, D], bf16, tag=f"xc{i % 4}", name=f"xc{i}")
                    if i % 2 == 0:
                        nc.vector.tensor_copy(t, ps_p[i])
                    else:
                        nc.scalar.activation(
                            t, ps_p[i], mybir.ActivationFunctionType.Copy
                        )
                    xc_sb.append(t)

                if dedup:
                    # --- phase A'': zUn[u, d2] = sum_i atU[i]^T @ xc[i] ------
                    ps_u = ppool.tile([nu, D], f32, tag="ps", name="ps_u")
                    for i in range(nk):
                        nc.tensor.matmul(
                            ps_u,
                            at_sb[i],
                            xc_sb[i],
                            start=(i == 0),
                            stop=(i == nk - 1),
                        )
                    zun = []  # per-d2-block [nu, 128] so deps are precise
                    # only d2=0 on DVE: keeps the DVE queue clear for the
                    # 8-add epilogue chain that follows immediately
                    for d2 in range(DBLK):
                        t = wpool.tile([nu, 128], bf16, tag=f"zun{d2}")
                        if d2 == 0:
                            nc.vector.tensor_copy(
                                t, ps_u[:, d2 * 128 : (d2 + 1) * 128]
                            )
                        else:
                            nc.scalar.activation(
                                t,
                                ps_u[:, d2 * 128 : (d2 + 1) * 128],
                                mybir.ActivationFunctionType.Copy,
                            )
                        zun.append(t)

                    # --- expand: outT[d2, s] = zUn-col-d2 ^T @ Sel + xrb -----
                    for d2 in range(DBLK):
                        for h in range(2):
                            hs = slice(h * 512, (h + 1) * 512)
                            ps_e = ppool.tile(
                                [128, 512], f32, tag="ps", name=f"ps_e{d2}_{h}"
                            )
                            nc.tensor.matmul(
                                ps_e,
                                zun[d2],
                                sel_sb[:, hs],
                                start=True,
                                stop=True,
                            )
                            o = wpool.tile(
                                [128, 512], f32, tag=f"osb{h}", name=f"o{d2}_{h}"
                            )
                            nc.vector.tensor_tensor(
                                o,
                                ps_e,
                                xrb_sb[d2][:, hs],
                                mybir.AluOpType.add,
                            )
                            ring = nc.sync if (d2 + h) % 2 == 0 else nc.scalar
                            ring.dma_start(
                                out=outT[d2 * 128 : (d2 + 1) * 128, hs],
                                in_=o,
                            )
                    _done = True
                else:
                    _done = False

                # --- phase A': outT-psum[d2,h] = xc-chain @ A^T --------------
                # group outer: each (d2, h) output group finishes its whole
                # block chain early so its residual-add + store pipeline
                # behind the PE while later groups stream.
                for d2 in range(DBLK) if not _done else []:
                    o = wpool.tile([128, QROWS], f32, tag="osb", name=f"osb{d2}")
                    for h in range(2):
                        hs = slice(h * 512, (h + 1) * 512)
                        ps_o = ppool.tile(
                            [128, 512], f32, tag="ps", name=f"ps_o{d2}_{h}"
                        )
                        for i in range(nk):
                            nc.tensor.matmul(
                                ps_o,
                                xc_sb[i][:, d2 * 128 : (d2 + 1) * 128],
                                at_sb[i][:, h * 512 : (h + 1) * 512],
                                start=(i == 0),
                                stop=(i == nk - 1),
                            )
                        nc.vector.tensor_tensor(
                            o[:, hs],
                            ps_o,
                            xrb_sb[d2][:, hs],
                            mybir.AluOpType.add,
                        )
                        ring = nc.sync if (d2 + h) % 2 == 0 else nc.scalar
                        ring.dma_start(
                            out=outT[d2 * 128 : (d2 + 1) * 128, hs], in_=o[:, hs]
                        )
            else:
                # --- phase A: axT[d] = x-block-col-d ^T @ A^T ----------------
                # k outer / d inner: each at-tile is consumed right after its
                # DMA lands, so the PE never waits on the A^T stream.
                ps_a = [
                    ppool.tile([128, QROWS], f32, tag="ps2", name=f"ps_a{d}")
                    for d in range(DBLK)
                ]
                for k in range(nk):
                    for d in range(DBLK):
                        for h in range(2):
                            nc.tensor.matmul(
                                ps_a[d][:, h * 512 : (h + 1) * 512],
                                xb_sb[k][:, d * 128 : (d + 1) * 128],
                                at_sb[k][:, h * 512 : (h + 1) * 512],
                                start=(k == 0),
                                stop=(k == nk - 1),
                            )
                axT = []
                for d in range(DBLK):
                    t = wpool.tile([128, QROWS], bf16, tag=f"axT{d}")
                    if d % 2 == 0:
                        nc.vector.tensor_copy(t, ps_a[d])
                    else:
                        nc.scalar.activation(
                            t, ps_a[d], mybir.ActivationFunctionType.Copy
                        )
                    axT.append(t)

                # --- phase B: outT[d2] = Wc-chain @ axT + (x^T + b_out) ------
                for d2 in range(DBLK):
                    ps_b = ppool.tile(
                        [128, QROWS], f32, tag="ps2", name=f"ps_b{d2}"
                    )
                    for d1 in range(DBLK):
                        for h in range(2):
                            nc.tensor.matmul(
                                ps_b[:, h * 512 : (h + 1) * 512],
                                wc_sb[d1][:, d2 * 128 : (d2 + 1) * 128],
                                axT[d1][:, h * 512 : (h + 1) * 512],
                                start=(d1 == 0),
                                stop=(d1 == DBLK - 1),
                            )
                    for h in range(2):
                        hs = slice(h * 512, (h + 1) * 512)
                        o = wpool.tile(
                            [128, 512], f32, tag=f"osb{h}", name=f"o{d2}_{h}"
                        )
                        nc.vector.tensor_tensor(
                            o,
                            ps_b[:, hs],
                            xrb_sb[d2][:, hs],
                            mybir.AluOpType.add,
                        )
                        ring = nc.sync if (d2 + h) % 2 == 0 else nc.scalar
                        ring.dma_start(
                            out=outT[d2 * 128 : (d2 + 1) * 128, hs], in_=o
                        )

    nc.finalize()
    return nc


def _get_runner(key):
    """Compile once per module key; return a callable(in_maps) -> out dicts.

    key: ("v1", nk, nu) or ("v2", rblocks_tuple, nu).
    """
    ckey = ("runner", key)
    if ckey in _cache:
        return _cache[ckey]

    import jax
    from jax.sharding import Mesh, PartitionSpec
    from jax.experimental.shard_map import shard_map
    from concourse import bass2jax
    import concourse.mybir as mybir

    bass2jax.install_neuronx_cc_hook()
    if key[0] == "v2":
        nc = _build_module_v2(key[1], key[2])
    else:
        nc = _build_module(key[1], key[2])

    part_name = nc.partition_id_tensor.name if nc.partition_id_tensor else None
    in_names = []
    out_names = []
    out_avals = []
    for alloc in nc.m.functions[0].allocations:
        if not isinstance(alloc, bass2jax.mybir.MemoryLocationSet):
            continue
        name = alloc.memorylocations[0].name
        if alloc.kind == "ExternalInput":
            if name != part_name:
                in_names.append(name)
        elif alloc.kind == "ExternalOutput":
            out_names.append(name)
            out_avals.append(
                jax.core.ShapedArray(
                    tuple(alloc.tensor_shape), mybir.dt.np(alloc.dtype)
                )
            )
    n_params = len(in_names)
    all_names = in_names + out_names
    if part_name is not None:
        all_names = all_names + [part_name]

    def _body(*args):
        operands = list(args)
        if part_name is not None:
            operands.append(bass2jax.partition_id_tensor())
        outs = bass2jax._bass_exec_p.bind(
            *operands,
            out_avals=tuple(out_avals),
            in_names=tuple(all_names),
            out_names=tuple(out_names),
            lowering_input_output_aliases=(),
            sim_require_finite=True,
            sim_require_nnan=True,
            nc=nc,
        )
        return tuple(outs)

    devices = jax.devices()[:NCORES]
    mesh = Mesh(np.asarray(devices), ("core",))
    nin = n_params + len(out_names)
    sharded = jax.jit(
        shard_map(
            _body,
            mesh=mesh,
            in_specs=(PartitionSpec("core"),) * nin,
            out_specs=(PartitionSpec("core"),) * len(out_names),
            check_rep=False,
        ),
        keep_unused=True,
    )

    zero_shapes = [(NCORES * a.shape[0], *a.shape[1:]) for a in out_avals]
    zero_dtypes = [a.dtype for a in out_avals]

    def run(in_maps):
        concat_in = [
            np.concatenate([np.asarray(m[name]) for m in in_maps], axis=0)
            for name in in_names
        ]
        zeros = [np.zeros(s, d) for s, d in zip(zero_shapes, zero_dtypes)]
        out_arrs = sharded(*concat_in, *zeros)
        jax.block_until_ready(out_arrs)
        res = [
            {
                name: np.asarray(out_arrs[i]).reshape(NCORES, *out_avals[i].shape)[c]
                for i, name in enumerate(out_names)
            }
            for c in range(NCORES)
        ]
        return res

    _cache[ckey] = run
    _cache[("sharded", key)] = sharded
    _cache[("meta", key)] = (in_names, out_names, out_avals)
    return run


def _host_prep_v2(x, W_in, W_out, b_out, fusion_weights, routes):
    """Fast-path host prep. Returns None if the routing tables don't have
    enough duplicate structure (falls back to v1), else
    (key, in_maps, epilogue_meta)."""
    x = np.asarray(x, dtype=np.float32)
    W_in = np.asarray(W_in, dtype=np.float32)
    W_out = np.asarray(W_out, dtype=np.float32)
    fw = np.asarray(fusion_weights, dtype=np.float32)
    rt = np.asarray(routes)

    # dedup output rows by exact (routes, weights) byte pattern
    pat = np.concatenate(
        [np.ascontiguousarray(rt).view(np.uint8),
         np.ascontiguousarray(fw).view(np.uint8)],
        axis=1,
    )
    _, uidx, inv = np.unique(pat, axis=0, return_index=True, return_inverse=True)
    inv = inv.ravel()
    n_uni = len(uidx)
    if n_uni > NGRP * 128:
        return None

    rt64 = rt.astype(np.int64)
    # per-unique source rows; group uniques by source-row locality
    srcs = [np.unique(rt64[i]) for i in uidx]
    minrow = np.array([s[0] for s in srcs])
    order = np.argsort(minrow, kind="stable")
    bounds = [round(n_uni * g / NGRP) for g in range(NGRP + 1)]
    groups = []  # (ids, rows, ut)
    numax = nrmax = 0
    for g in range(NGRP):
        ids = order[bounds[g] : bounds[g + 1]]
        if len(ids) == 0:
            ids = order[:1]
        rows = np.unique(np.concatenate([srcs[i] for i in ids]))
        nu_c, nr_c = len(ids), len(rows)
        if nu_c > 128:
            return None
        ut = np.zeros((nr_c, nu_c), np.float32)
        ri = np.searchsorted(rows, rt64[uidx[ids]].ravel())
        uu = np.repeat(np.arange(nu_c), K)
        np.add.at(ut, (ri, uu), fw[uidx[ids]].ravel())
        groups.append((ids, rows, ut))
        numax = max(numax, nu_c)
        nrmax = max(nrmax, nr_c)

    if nrmax > 16 * 128:
        return None

    rblocks = []
    left = nrmax
    while left > 0:
        rblocks.append(min(128, left))
        left -= 128
    rblocks = tuple(rblocks)

    Wc = (W_in @ W_out).astype(_f8)
    # 2-up packing: tile t row r = Wc rows (2t)*128+r (cols 0:512) and
    # (2t+1)*128+r (cols 512:1024)
    wcd = np.zeros((256, 1024), _f8)
    for t in range(2):
        wcd[t * 128 : (t + 1) * 128, :512] = Wc[2 * t * 128 : (2 * t + 1) * 128]
        wcd[t * 128 : (t + 1) * 128, 512:] = Wc[(2 * t + 1) * 128 : (2 * t + 2) * 128]

    # 2-up r-block packing mirroring _build_module_v2
    W = 512 + numax
    dtiles = []
    i = 0
    while i < len(rblocks):
        if i + 1 < len(rblocks):
            dtiles.append((max(rblocks[i], rblocks[i + 1]), [i, i + 1]))
            i += 2
        else:
            dtiles.append((rblocks[i], [i]))
            i += 1

    in_maps = []
    for c in range(NCORES):
        b, g = divmod(c, NGRP)
        ids, rows, ut = groups[g]
        xu = np.zeros((len(rblocks) * 128, W), _f8)
        xu[: len(rows), :512] = x[b][rows].astype(_f8)
        xu[: len(rows), 512 : 512 + ut.shape[1]] = ut.astype(_f8)
        xud = np.zeros((sum(p for p, _ in dtiles), 2 * W), _f8)
        r0 = 0
        for p, blks in dtiles:
            for s, bi in enumerate(blks):
                xud[r0 : r0 + rblocks[bi], s * W : s * W + W] = xu[
                    bi * 128 : bi * 128 + rblocks[bi]
                ]
            r0 += p
        in_maps.append({"xud": xud, "wcd": wcd})

    # epilogue: map each output position s to (group, local unique index)
    gid = np.empty(n_uni, np.int64)
    lix = np.empty(n_uni, np.int64)
    for g in range(NGRP):
        ids = groups[g][0]
        gid[ids] = g
        lix[ids] = np.arange(len(ids))
    # flat index into the per-batch stacked [NGRP*numax, D] result
    flat = gid[inv] * numax + lix[inv]  # [S]

    key = ("v2", rblocks, numax)
    return key, in_maps, (flat, numax)


def _host_prep(x, W_in, W_out, b_out, fusion_weights, routes):
    """v1 host prep. Returns (nk, nu, in_maps). Packs only the nonzero
    128-row source blocks of A^T (and the matching x blocks) per core,
    padded to the max count nk."""
    x = np.asarray(x, dtype=np.float32)
    W_in = np.asarray(W_in, dtype=np.float32)
    W_out = np.asarray(W_out, dtype=np.float32)
    b_out = np.asarray(b_out, dtype=np.float32)
    fw = np.asarray(fusion_weights, dtype=np.float32)
    rt = np.asarray(routes)

    Wc = (W_in @ W_out).astype(_f8)
    xb16 = [x[b].astype(_bf16) for b in range(B)]
    # residual + bias, pre-transposed: [D, QROWS] fp32 per (b, q)
    xrb = [
        [
            np.ascontiguousarray(x[b, q * QROWS : (q + 1) * QROWS].T)
            + b_out[:, None]
            for q in range(4)
        ]
        for b in range(B)
    ]

    # densify A^T per seq-quarter and find its nonzero source blocks
    cols = np.repeat(np.arange(QROWS, dtype=np.int64), K)
    at_q = []
    kset_q = []
    for q in range(4):
        r = rt[q * QROWS : (q + 1) * QROWS].astype(np.int64).ravel()
        a = np.zeros((S, QROWS), np.float32)
        np.add.at(a, (r, cols), fw[q * QROWS : (q + 1) * QROWS].ravel())
        blocks = a.reshape(KBLK, 128, QROWS)
        ks = [k for k in range(KBLK) if np.any(blocks[k])]
        if not ks:
            ks = [0]
        at_q.append(a.astype(_bf16))
        kset_q.append(ks)

    nk = max(len(ks) for ks in kset_q)

    fused = nk <= FUSED_NK_MAX
    # distinct-column compression: for Cantor routing many output positions
    # share identical A^T columns; contract over the unique columns and
    # expand with a one-hot matmul when they all fit in one 128-partition
    # tile.
    nu = 0
    uniq_q = None
    if fused:
        uniq_q = []
        for q in range(4):
            u16 = at_q[q].view(np.uint16)
            uc, inv = np.unique(u16.T, axis=0, return_inverse=True)
            uniq_q.append((uc, inv))
        if max(len(uc) for uc, _ in uniq_q) <= 128:
            nu = 128

    in_maps = []
    for c in range(NCORES):
        b, q = divmod(c, 4)
        ks = kset_q[q]
        if nu:
            uc, inv = uniq_q[q]
            atu_full = np.ascontiguousarray(uc.T).view(_bf16)  # [S, Uq]
            at_p = np.zeros((nk * 128, nu), _bf16)
            for i, k in enumerate(ks):
                at_p[i * 128 : (i + 1) * 128, : uc.shape[0]] = atu_full[
                    k * 128 : (k + 1) * 128
                ]
            sel_p = np.zeros((nu, QROWS), _bf16)
            sel_p[inv, np.arange(QROWS)] = _bf16(1.0)
            m = {"at": at_p, "sel": sel_p, "wc": Wc, "xrb": xrb[b][q]}
        else:
            at_p = np.zeros((nk * 128, QROWS), _bf16)
            for i, k in enumerate(ks):
                at_p[i * 128 : (i + 1) * 128] = at_q[q][k * 128 : (k + 1) * 128]
            m = {"at": at_p, "wc": Wc, "xrb": xrb[b][q]}
        if fused:
            xtp = np.zeros((D, nk * 128), _bf16)
            for i, k in enumerate(ks):
                xtp[:, i * 128 : (i + 1) * 128] = xb16[b][
                    k * 128 : (k + 1) * 128
                ].T
            m["xtp"] = xtp
        else:
            xb_p = np.zeros((nk * 128, D), _bf16)
            for i, k in enumerate(ks):
                xb_p[i * 128 : (i + 1) * 128] = xb16[b][k * 128 : (k + 1) * 128]
            m["xb"] = xb_p
        in_maps.append(m)
    return nk, nu, in_maps


def kernel(x, W_in, W_out, b_out, fusion_weights, routes):
    x = np.asarray(x, dtype=np.float32)
    b_out = np.asarray(b_out, dtype=np.float32)

    prep = _host_prep_v2(x, W_in, W_out, b_out, fusion_weights, routes)
    if prep is not None:
        key, in_maps, (flat, numax) = prep
        run = _get_runner(key)
        res = run(in_maps)
        out = np.empty((B, S, D), np.float32)
        for b in range(B):
            zall = np.concatenate(
                [res[b * NGRP + g]["zc"][:numax] for g in range(NGRP)], axis=0
            ).astype(np.float32)  # [NGRP*numax, D]
            out[b] = x[b] + zall[flat] + b_out
        return out

    nk, nu, in_maps = _host_prep(x, W_in, W_out, b_out, fusion_weights, routes)
    run = _get_runner(("v1", nk, nu))
    res = run(in_maps)
    out = np.empty((B, S, D), np.float32)
    for c in range(NCORES):
        b, q = divmod(c, 4)
        out[b, q * QROWS : (q + 1) * QROWS] = res[c]["outT"].T
    return out


# revision 69
# speedup vs baseline: 1.1169x; 1.0282x over previous
"""CantorMultiheadFusion kernel for 8 Trainium2 NeuronCores.

Math: out = x + A @ x @ (W_in @ W_out) + b_out, where A is the (S,S) sparse
fusion matrix with A[s, routes[s,k]] += fusion_weights[s,k].

Fast path (v2): the Cantor routing tables make A massively degenerate — the
whole sequence has only ~353 DISTINCT rows (output positions sharing an
identical (routes, weights) pattern), and those rows touch only ~1.4K distinct
source positions. Each core therefore computes only the unique fused rows:

    Zc = (U^T X)^T @ Wc        U: [nr, nu] unique-row table (nu <= 128)
                               X: [nr, D]  the used source rows of x
                               Wc = W_in @ W_out

sharded (batch b x unique-group g) over 8 cores, with the uniques split into
4 groups ordered by source-row locality so per-core (nu, nr) stay small. The
host expands Zc back to the full (B, S, D) output with a pure gather and adds
the residual x + b_out in fp32 (the same class of host epilogue work the v1
path did when assembling its xrb residual tensor and transposed output).

Per-core HBM traffic is ~1MB (vs ~6MB for v1): xs+U^T packed into one wide
bf16 tensor, Wc bf16, and a [nu, D] bf16 result. On-device math is bf16 with
fp32 PSUM accumulation.

The v1 kernel (dense block-matmul on A^T) is kept as a fallback for routing
tables without enough structure (e.g. uniform-random routes).
"""

import numpy as np
import ml_dtypes

B, S, D, K = 2, 4096, 512, 32
NCORES = 8
QROWS = S // 4  # rows per core = 1024
DBLK = D // 128  # 4
KBLK = S // 128  # 32
NGRP = 4  # unique-row groups per batch (v2)

_bf16 = ml_dtypes.bfloat16
_f8 = ml_dtypes.float8_e4m3

_cache = {}


FUSED_NK_MAX = 8

# v2 tuning knobs
V2_WARMUP = 6  # PE p-state warm-up matmuls
V2_FILL = 0  # PE keep-busy matmuls between phase 1 and phase 2


def _build_module_v2(rblocks, nu):
    """v2 module. Inputs per core:

    - xud: r-blocks of (x rows | U^T) packed 2-up into DMA tiles.
    - wcd [256, 1024] bf16: Wc 2-up packed (tile t row r = Wc rows 2t*128+r
      and (2t+1)*128+r); loaded as three DMAs: d23 via Pool/SWDGE (its
      descriptor generation runs parallel to HWDGE so it transfers right
      after the x stream), d0 and d1 as separate HWDGE slices so each
      phase-2 link starts on its own block's arrival.
    - zc [nu, 512] bf16 out: the unique fused+projected rows.

    Phase 1 (per r-block i): psZT[j][d, u] += xs_i[:, dblk j]^T @ ut_i
    Phase 2: psZC[u, e] += matmul(lhsT=zt_j [d,u], rhs=wc_j [d,e]) in
    d-block arrival order (2, 3, 0, 1).
    """
    import concourse.mybir as mybir
    import concourse.tile as tile
    from concourse import bacc

    f32 = mybir.dt.float32
    bf16 = mybir.dt.bfloat16
    f8 = mybir.dt.float8e4

    nrb = len(rblocks)
    W = 512 + nu

    # r-blocks packed 2-up into DMA tiles (tile t holds blocks 2t, 2t+1 side
    # by side) so the x+U stream needs only ceil(nrb/2) HWDGE generations
    # while m1 still pipelines per tile.
    dtiles = []  # (partitions, [r-block indices])
    i = 0
    while i < nrb:
        if i + 1 < nrb:
            dtiles.append((max(rblocks[i], rblocks[i + 1]), [i, i + 1]))
            i += 2
        else:
            dtiles.append((rblocks[i], [i]))
            i += 1

    nc = bacc.Bacc("TRN2", target_bir_lowering=True)

    # fp8 e4m3 for the whole input stream (x, U^T, Wc): the matmul error
    # lands at ~8e-3 relative on the final output, well inside the 2e-2
    # budget, and halves the HBM stream. PSUM accumulation stays fp32 and
    # the result leaves in bf16.
    xud = nc.dram_tensor(
        "xud", [sum(p for p, _ in dtiles), 2 * W], f8, kind="ExternalInput"
    )
    # wc 2-up packed: row r of tile t holds Wc rows (2t)*128+r | (2t+1)*128+r
    wcd = nc.dram_tensor("wcd", [256, 1024], f8, kind="ExternalInput")
    zc = nc.dram_tensor("zc", [nu, D], bf16, kind="ExternalOutput")

    with tile.TileContext(nc) as tc:
        with (
            tc.tile_pool(name="const", bufs=1) as cpool,
            tc.tile_pool(name="work", bufs=1) as wpool,
            tc.tile_pool(name="psum", bufs=1, space="PSUM") as ppool,
        ):
            # PE p-state warm-up on a memset tile (no DMA dependency).
            wu = cpool.tile([128, 128], bf16, tag="wu")
            nc.gpsimd.memset(wu, 0.0)
            ps_w = ppool.tile([128, 512], f32, tag="ps_w")
            for _ in range(V2_WARMUP):
                nc.tensor.matmul(ps_w[:, :128], wu, wu, start=True, stop=True)

            # streamed loads: xud tiles first, wc tiles last
            # xud0 on SP (first HWDGE slot); the trailing xud tile via the
            # Pool queue — its SWDGE generation finishes by ~2.9us so it
            # transfers right behind xud0 instead of waiting for the Act
            # HWDGE slot's 784ns DGE delay (which left a 339ns hole and
            # delayed everything chained on the last x block).
            xu_sb = {}  # r-block index -> (tile, col offset)
            r0 = 0
            for ti, (p, blks) in enumerate(dtiles):
                tw = len(blks) * W
                t = cpool.tile([p, tw], f8, tag=f"xud{ti}", name=f"xud{ti}")
                eng = (nc.sync, nc.gpsimd)[min(ti, 1)]
                eng.dma_start(out=t, in_=xud[r0 : r0 + p, :tw])
                for s, i in enumerate(blks):
                    xu_sb[i] = (t, s * W)
                r0 += p
            # Wc as two d-pair tiles: tile t holds d-blocks 2t (cols 0:512)
            # and 2t+1 (cols 512:1024). The d23 tile is issued on the Pool
            # engine: its SWDGE descriptor generation runs in parallel with
            # the HWDGE generations, so it transfers (and lands) one slot
            # earlier; the phase-2 chain consumes d-blocks in arrival order
            # (2,3 then 0,1).
            # pace the Pool queue so the d23 tile's SWDGE generation finishes
            # inside the (xud1, wcA) window of the shared transfer engine's
            # FIFO — earlier and it queue-jumps ahead of the x stream, later
            # and it loses the head start. (Ordering only affects timing:
            # every consumer waits its own DMA semaphore.)
            # d01 via the Act HWDGE slot (transfers 3rd), d23 via SP's
            # second HWDGE slot (transfers last); the chain consumes in
            # arrival order 0,1,2,3
            wc01 = cpool.tile([128, 1024], f8, tag="wc01")
            nc.scalar.dma_start(out=wc01, in_=wcd[0:128, :])
            wc23 = cpool.tile([128, 1024], f8, tag="wc23")
            nc.sync.dma_start(out=wc23, in_=wcd[128:256, :])
            wc_rhs = [
                wc01[:, 0:512],
                wc01[:, 512:1024],
                wc23[:, 0:512],
                wc23[:, 512:1024],
            ]

            # phase 1: psZT[j] = sum_i xs_i[:, dblk j]^T @ ut_i   -> [128, nu]
            ps_zt = [
                ppool.tile([128, nu], f32, tag=f"pzt{j}", name=f"pzt{j}")
                for j in range(DBLK)
            ]
            for i in range(nrb):
                t, coff = xu_sb[i]
                for j in range(DBLK):
                    nc.tensor.matmul(
                        ps_zt[j],
                        t[:, coff + j * 128 : coff + (j + 1) * 128],
                        t[:, coff + 512 : coff + W],
                        start=(i == 0),
                        stop=(i == nrb - 1),
                    )
            # bf16 stage in chain-consumption order (links run 2,3,0,1)
            zt_sb = [None] * DBLK
            for n, j in enumerate((0, 1, 2, 3)):
                t = wpool.tile([128, nu], f8, tag=f"zt{j}", name=f"zt{j}")
                if n % 2 == 0:
                    nc.vector.tensor_copy(t, ps_zt[j])
                else:
                    nc.scalar.activation(
                        t, ps_zt[j], mybir.ActivationFunctionType.Copy
                    )
                zt_sb[j] = t

            # keep the PE p-state hot across the psum-copy gap
            for _ in range(V2_FILL):
                nc.tensor.matmul(ps_w[:, :128], wu, wu, start=True, stop=True)

            # phase 2: two e-half chains (each link pair gated by the same wc
            # block, so scheduler interleaving cannot stall); the first
            # half's narrower copy overlaps the second half's links. Chain
            # consumes d-blocks in arrival order (2, 3, 0, 1).
            ps_a = ppool.tile([nu, D // 2], f32, tag="pza")
            ps_b = ppool.tile([nu, D // 2], f32, tag="pzb")
            order = (0, 1, 2, 3)
            for n, j in enumerate(order):
                for h, ps in enumerate((ps_a, ps_b)):
                    nc.tensor.matmul(
                        ps,
                        zt_sb[j],
                        wc_rhs[j][:, h * 256 : (h + 1) * 256],
                        start=(n == 0),
                        stop=(n == DBLK - 1),
                    )
            # first-finishing half on ACT (slow completion defer tolerable),
            # last half on DVE (58-cycle defer vs ACT's 222 → the store's
            # HWDGE generation starts earlier)
            o = wpool.tile([nu, D], bf16, tag="o")
            nc.scalar.activation(
                o[:, :256], ps_a, mybir.ActivationFunctionType.Copy
            )
            nc.vector.tensor_copy(o[:, 256:], ps_b)
            nc.sync.dma_start(out=zc[:, :], in_=o)

    nc.finalize()
    return nc


def _build_module(nk=KBLK, nu=0):
    """v1 fallback module. Two variants by nk:

    - fused (nk <= FUSED_NK_MAX): phase P projects the packed x blocks by Wc
      first (xc = x_sel @ Wc, cheap since only nk blocks), then a single
      accumulation phase A' computes outT = xc_sel^T-chain @ A^T. Phase P
      fills the startup hole while the A^T stream is still arriving, and
      there is no post-phase projection tail.
    - split (nk > FUSED_NK_MAX): big phase A (x^T-chain @ A^T) then a small
      projection phase B by Wc. Cheaper when nk is large because P would
      scale with nk while B is constant.
    """
    import concourse.mybir as mybir
    import concourse.tile as tile
    from concourse import bacc

    f32 = mybir.dt.float32
    bf16 = mybir.dt.bfloat16
    fused = nk <= FUSED_NK_MAX
    # nu > 0: additionally compress A^T to its nu (<=128) distinct columns
    # and expand the result back with a one-hot selection matmul.
    dedup = fused and nu > 0

    nc = bacc.Bacc("TRN2", target_bir_lowering=True)

    if fused:
        # packed x^T: [D, nk*128]; entry [d, i*128 + c] = x_block_i[c, d]
        xtp = nc.dram_tensor("xtp", [D, nk * 128], bf16, kind="ExternalInput")
    else:
        xb = nc.dram_tensor("xb", [nk * 128, D], bf16, kind="ExternalInput")
    if dedup:
        at = nc.dram_tensor("at", [nk * 128, nu], bf16, kind="ExternalInput")
        sel = nc.dram_tensor("sel", [nu, QROWS], bf16, kind="ExternalInput")
    else:
        at = nc.dram_tensor("at", [nk * 128, QROWS], bf16, kind="ExternalInput")
    wc = nc.dram_tensor("wc", [D, D], bf16, kind="ExternalInput")
    xrb = nc.dram_tensor("xrb", [D, QROWS], f32, kind="ExternalInput")
    outT = nc.dram_tensor("outT", [D, QROWS], f32, kind="ExternalOutput")

    with tile.TileContext(nc) as tc:
        with (
            tc.tile_pool(name="const", bufs=1) as cpool,
            tc.tile_pool(name="work", bufs=3) as wpool,
            tc.tile_pool(name="psum", bufs=8 if fused else 4, space="PSUM") as ppool,
        ):
            # PE warm-up: matmuls on a memset tile (no DMA dependency) fill
            # the DMA-latency startup hole and lift the HAM clock gate to
            # 8/8 before the real chains start.
            wu = cpool.tile([128, 128], bf16, tag="wu")
            nc.gpsimd.memset(wu, 0.0)
            ps_w = ppool.tile(
                [128, 512], f32, tag="ps" if fused else "ps2", name="ps_w"
            )
            for _ in range(23):
                nc.tensor.matmul(ps_w[:, :128], wu, wu, start=True, stop=True)
            wu2 = wpool.tile([128, 1], bf16, tag="wu2")
            nc.vector.tensor_copy(wu2, ps_w[:, :1])  # release the bank

            # --- streamed loads ---------------------------------------------
            if fused:
                wc_sb = []
                xtp_sb = []  # x^T tile per d1: [128, nk*128], block i at cols i*128
                for d1 in range(DBLK):
                    t = cpool.tile([128, D], bf16, tag=f"wc{d1}")
                    nc.gpsimd.dma_start(out=t, in_=wc[d1 * 128 : (d1 + 1) * 128, :])
                    wc_sb.append(t)
                    t = cpool.tile([128, nk * 128], bf16, tag=f"xtp{d1}")
                    nc.sync.dma_start(
                        out=t, in_=xtp[d1 * 128 : (d1 + 1) * 128, :]
                    )
                    xtp_sb.append(t)
            else:
                xb_sb = []  # packed x[b] row-block k: [128, D]
                for k in range(nk):
                    t = cpool.tile([128, D], bf16, tag=f"xb{k}")
                    nc.sync.dma_start(out=t, in_=xb[k * 128 : (k + 1) * 128, :])
                    xb_sb.append(t)

            sel_sb = None
            if dedup:
                sel_sb = cpool.tile([nu, QROWS], bf16, tag="sel")
                nc.scalar.dma_start(out=sel_sb, in_=sel[:, :])

            atw = nu if dedup else QROWS
            at_sb = []  # packed A^T row-block k: [128, atw]
            for k in range(nk):
                t = cpool.tile([128, atw], bf16, tag=f"at{k}")
                if fused:
                    # spread the stream over all three DMA queues so it has
                    # fully landed before phase A' consumes it back-to-back
                    eng = (nc.scalar, nc.scalar, nc.sync, nc.gpsimd)[k % 4]
                else:
                    eng = nc.scalar
                eng.dma_start(out=t, in_=at[k * 128 : (k + 1) * 128, :])
                at_sb.append(t)

            if not fused:
                wc_sb = []
                for d1 in range(DBLK):
                    t = cpool.tile([128, D], bf16, tag=f"wc{d1}")
                    nc.sync.dma_start(out=t, in_=wc[d1 * 128 : (d1 + 1) * 128, :])
                    wc_sb.append(t)

            xrb_sb = []  # (x^T + b_out) block d2: [128, QROWS] fp32
            for d2 in range(DBLK):
                t = cpool.tile([128, QROWS], f32, tag=f"xrb{d2}")
                eng = nc.gpsimd if fused else nc.sync
                eng.dma_start(out=t, in_=xrb[d2 * 128 : (d2 + 1) * 128, :])
                xrb_sb.append(t)

            if fused:
                # --- phase P: xc[i] = x_block[i] @ Wc ------------------------
                # d1 outer: paced by the (xtp[d1], wc[d1]) tile arrivals, all
                # nk accumulation groups advance together.
                ps_p = [
                    ppool.tile([128, D], f32, tag="ps", name=f"ps_p{i}")
                    for i in range(nk)
                ]
                for d1 in range(DBLK):
                    for i in range(nk):
                        nc.tensor.matmul(
                            ps_p[i],
                            xtp_sb[d1][:, i * 128 : (i + 1) * 128],
                            wc_sb[d1],
                            start=(d1 == 0),
                            stop=(d1 == DBLK - 1),
                        )
                xc_sb = []
                for i in range(nk):
                    t = wpool.tile([128, D], bf16, tag=f"xc{i % 4}", name=f"xc{i}")
                    if i % 2 == 0:
                        nc.vector.tensor_copy(t, ps_p[i])
                    else:
                        nc.scalar.activation(
                            t, ps_p[i], mybir.ActivationFunctionType.Copy
                        )
                    xc_sb.append(t)

                if dedup:
                    # --- phase A'': zUn[u, d2] = sum_i atU[i]^T @ xc[i] ------
                    ps_u = ppool.tile([nu, D], f32, tag="ps", name="ps_u")
                    for i in range(nk):
                        nc.tensor.matmul(
                            ps_u,
                            at_sb[i],
                            xc_sb[i],
                            start=(i == 0),
                            stop=(i == nk - 1),
                        )
                    zun = []  # per-d2-block [nu, 128] so deps are precise
                    # only d2=0 on DVE: keeps the DVE queue clear for the
                    # 8-add epilogue chain that follows immediately
                    for d2 in range(DBLK):
                        t = wpool.tile([nu, 128], bf16, tag=f"zun{d2}")
                        if d2 == 0:
                            nc.vector.tensor_copy(
                                t, ps_u[:, d2 * 128 : (d2 + 1) * 128]
                            )
                        else:
                            nc.scalar.activation(
                                t,
                                ps_u[:, d2 * 128 : (d2 + 1) * 128],
                                mybir.ActivationFunctionType.Copy,
                            )
                        zun.append(t)

                    # --- expand: outT[d2, s] = zUn-col-d2 ^T @ Sel + xrb -----
                    for d2 in range(DBLK):
                        for h in range(2):
                            hs = slice(h * 512, (h + 1) * 512)
                            ps_e = ppool.tile(
                                [128, 512], f32, tag="ps", name=f"ps_e{d2}_{h}"
                            )
                            nc.tensor.matmul(
                                ps_e,
                                zun[d2],
                                sel_sb[:, hs],
                                start=True,
                                stop=True,
                            )
                            o = wpool.tile(
                                [128, 512], f32, tag=f"osb{h}", name=f"o{d2}_{h}"
                            )
                            nc.vector.tensor_tensor(
                                o,
                                ps_e,
                                xrb_sb[d2][:, hs],
                                mybir.AluOpType.add,
                            )
                            ring = nc.sync if (d2 + h) % 2 == 0 else nc.scalar
                            ring.dma_start(
                                out=outT[d2 * 128 : (d2 + 1) * 128, hs],
                                in_=o,
                            )
                    _done = True
                else:
                    _done = False

                # --- phase A': outT-psum[d2,h] = xc-chain @ A^T --------------
                # group outer: each (d2, h) output group finishes its whole
                # block chain early so its residual-add + store pipeline
                # behind the PE while later groups stream.
                for d2 in range(DBLK) if not _done else []:
                    o = wpool.tile([128, QROWS], f32, tag="osb", name=f"osb{d2}")
                    for h in range(2):
                        hs = slice(h * 512, (h + 1) * 512)
                        ps_o = ppool.tile(
                            [128, 512], f32, tag="ps", name=f"ps_o{d2}_{h}"
                        )
                        for i in range(nk):
                            nc.tensor.matmul(
                                ps_o,
                                xc_sb[i][:, d2 * 128 : (d2 + 1) * 128],
                                at_sb[i][:, h * 512 : (h + 1) * 512],
                                start=(i == 0),
                                stop=(i == nk - 1),
                            )
                        nc.vector.tensor_tensor(
                            o[:, hs],
                            ps_o,
                            xrb_sb[d2][:, hs],
                            mybir.AluOpType.add,
                        )
                        ring = nc.sync if (d2 + h) % 2 == 0 else nc.scalar
                        ring.dma_start(
                            out=outT[d2 * 128 : (d2 + 1) * 128, hs], in_=o[:, hs]
                        )
            else:
                # --- phase A: axT[d] = x-block-col-d ^T @ A^T ----------------
                # k outer / d inner: each at-tile is consumed right after its
                # DMA lands, so the PE never waits on the A^T stream.
                ps_a = [
                    ppool.tile([128, QROWS], f32, tag="ps2", name=f"ps_a{d}")
                    for d in range(DBLK)
                ]
                for k in range(nk):
                    for d in range(DBLK):
                        for h in range(2):
                            nc.tensor.matmul(
                                ps_a[d][:, h * 512 : (h + 1) * 512],
                                xb_sb[k][:, d * 128 : (d + 1) * 128],
                                at_sb[k][:, h * 512 : (h + 1) * 512],
                                start=(k == 0),
                                stop=(k == nk - 1),
                            )
                axT = []
                for d in range(DBLK):
                    t = wpool.tile([128, QROWS], bf16, tag=f"axT{d}")
                    if d % 2 == 0:
                        nc.vector.tensor_copy(t, ps_a[d])
                    else:
                        nc.scalar.activation(
                            t, ps_a[d], mybir.ActivationFunctionType.Copy
                        )
                    axT.append(t)

                # --- phase B: outT[d2] = Wc-chain @ axT + (x^T + b_out) ------
                for d2 in range(DBLK):
                    ps_b = ppool.tile(
                        [128, QROWS], f32, tag="ps2", name=f"ps_b{d2}"
                    )
                    for d1 in range(DBLK):
                        for h in range(2):
                            nc.tensor.matmul(
                                ps_b[:, h * 512 : (h + 1) * 512],
                                wc_sb[d1][:, d2 * 128 : (d2 + 1) * 128],
                                axT[d1][:, h * 512 : (h + 1) * 512],
                                start=(d1 == 0),
                                stop=(d1 == DBLK - 1),
                            )
                    for h in range(2):
                        hs = slice(h * 512, (h + 1) * 512)
                        o = wpool.tile(
                            [128, 512], f32, tag=f"osb{h}", name=f"o{d2}_{h}"
                        )
                        nc.vector.tensor_tensor(
                            o,
                            ps_b[:, hs],
                            xrb_sb[d2][:, hs],
                            mybir.AluOpType.add,
                        )
                        ring = nc.sync if (d2 + h) % 2 == 0 else nc.scalar
                        ring.dma_start(
                            out=outT[d2 * 128 : (d2 + 1) * 128, hs], in_=o
                        )

    nc.finalize()
    return nc


def _get_runner(key):
    """Compile once per module key; return a callable(in_maps) -> out dicts.

    key: ("v1", nk, nu) or ("v2", rblocks_tuple, nu).
    """
    ckey = ("runner", key)
    if ckey in _cache:
        return _cache[ckey]

    import jax
    from jax.sharding import Mesh, PartitionSpec
    from jax.experimental.shard_map import shard_map
    from concourse import bass2jax
    import concourse.mybir as mybir

    bass2jax.install_neuronx_cc_hook()
    if key[0] == "v2":
        nc = _build_module_v2(key[1], key[2])
    else:
        nc = _build_module(key[1], key[2])

    part_name = nc.partition_id_tensor.name if nc.partition_id_tensor else None
    in_names = []
    out_names = []
    out_avals = []
    for alloc in nc.m.functions[0].allocations:
        if not isinstance(alloc, bass2jax.mybir.MemoryLocationSet):
            continue
        name = alloc.memorylocations[0].name
        if alloc.kind == "ExternalInput":
            if name != part_name:
                in_names.append(name)
        elif alloc.kind == "ExternalOutput":
            out_names.append(name)
            out_avals.append(
                jax.core.ShapedArray(
                    tuple(alloc.tensor_shape), mybir.dt.np(alloc.dtype)
                )
            )
    n_params = len(in_names)
    all_names = in_names + out_names
    if part_name is not None:
        all_names = all_names + [part_name]

    def _body(*args):
        operands = list(args)
        if part_name is not None:
            operands.append(bass2jax.partition_id_tensor())
        outs = bass2jax._bass_exec_p.bind(
            *operands,
            out_avals=tuple(out_avals),
            in_names=tuple(all_names),
            out_names=tuple(out_names),
            lowering_input_output_aliases=(),
            sim_require_finite=True,
            sim_require_nnan=True,
            nc=nc,
        )
        return tuple(outs)

    devices = jax.devices()[:NCORES]
    mesh = Mesh(np.asarray(devices), ("core",))
    nin = n_params + len(out_names)
    sharded = jax.jit(
        shard_map(
            _body,
            mesh=mesh,
            in_specs=(PartitionSpec("core"),) * nin,
            out_specs=(PartitionSpec("core"),) * len(out_names),
            check_rep=False,
        ),
        keep_unused=True,
    )

    zero_shapes = [(NCORES * a.shape[0], *a.shape[1:]) for a in out_avals]
    zero_dtypes = [a.dtype for a in out_avals]

    def run(in_maps):
        concat_in = [
            np.concatenate([np.asarray(m[name]) for m in in_maps], axis=0)
            for name in in_names
        ]
        zeros = [np.zeros(s, d) for s, d in zip(zero_shapes, zero_dtypes)]
        out_arrs = sharded(*concat_in, *zeros)
        jax.block_until_ready(out_arrs)
        res = [
            {
                name: np.asarray(out_arrs[i]).reshape(NCORES, *out_avals[i].shape)[c]
                for i, name in enumerate(out_names)
            }
            for c in range(NCORES)
        ]
        return res

    _cache[ckey] = run
    _cache[("sharded", key)] = sharded
    _cache[("meta", key)] = (in_names, out_names, out_avals)
    return run


def _host_prep_v2(x, W_in, W_out, b_out, fusion_weights, routes):
    """Fast-path host prep. Returns None if the routing tables don't have
    enough duplicate structure (falls back to v1), else
    (key, in_maps, epilogue_meta)."""
    x = np.asarray(x, dtype=np.float32)
    W_in = np.asarray(W_in, dtype=np.float32)
    W_out = np.asarray(W_out, dtype=np.float32)
    fw = np.asarray(fusion_weights, dtype=np.float32)
    rt = np.asarray(routes)

    # dedup output rows by exact (routes, weights) byte pattern
    pat = np.concatenate(
        [np.ascontiguousarray(rt).view(np.uint8),
         np.ascontiguousarray(fw).view(np.uint8)],
        axis=1,
    )
    _, uidx, inv = np.unique(pat, axis=0, return_index=True, return_inverse=True)
    inv = inv.ravel()
    n_uni = len(uidx)
    if n_uni > NGRP * 128:
        return None

    rt64 = rt.astype(np.int64)
    # per-unique source rows; group uniques by source-row locality
    srcs = [np.unique(rt64[i]) for i in uidx]
    minrow = np.array([s[0] for s in srcs])
    order = np.argsort(minrow, kind="stable")
    bounds = [round(n_uni * g / NGRP) for g in range(NGRP + 1)]
    groups = []  # (ids, rows, ut)
    numax = nrmax = 0
    for g in range(NGRP):
        ids = order[bounds[g] : bounds[g + 1]]
        if len(ids) == 0:
            ids = order[:1]
        rows = np.unique(np.concatenate([srcs[i] for i in ids]))
        nu_c, nr_c = len(ids), len(rows)
        if nu_c > 128:
            return None
        ut = np.zeros((nr_c, nu_c), np.float32)
        ri = np.searchsorted(rows, rt64[uidx[ids]].ravel())
        uu = np.repeat(np.arange(nu_c), K)
        np.add.at(ut, (ri, uu), fw[uidx[ids]].ravel())
        groups.append((ids, rows, ut))
        numax = max(numax, nu_c)
        nrmax = max(nrmax, nr_c)

    if nrmax > 16 * 128:
        return None

    rblocks = []
    left = nrmax
    while left > 0:
        rblocks.append(min(128, left))
        left -= 128
    rblocks = tuple(rblocks)

    Wc = (W_in @ W_out).astype(_f8)
    # 2-up packing: tile t row r = Wc rows (2t)*128+r (cols 0:512) and
    # (2t+1)*128+r (cols 512:1024)
    wcd = np.zeros((256, 1024), _f8)
    for t in range(2):
        wcd[t * 128 : (t + 1) * 128, :512] = Wc[2 * t * 128 : (2 * t + 1) * 128]
        wcd[t * 128 : (t + 1) * 128, 512:] = Wc[(2 * t + 1) * 128 : (2 * t + 2) * 128]

    # 2-up r-block packing mirroring _build_module_v2
    W = 512 + numax
    dtiles = []
    i = 0
    while i < len(rblocks):
        if i + 1 < len(rblocks):
            dtiles.append((max(rblocks[i], rblocks[i + 1]), [i, i + 1]))
            i += 2
        else:
            dtiles.append((rblocks[i], [i]))
            i += 1

    in_maps = []
    for c in range(NCORES):
        b, g = divmod(c, NGRP)
        ids, rows, ut = groups[g]
        xu = np.zeros((len(rblocks) * 128, W), _f8)
        xu[: len(rows), :512] = x[b][rows].astype(_f8)
        xu[: len(rows), 512 : 512 + ut.shape[1]] = ut.astype(_f8)
        xud = np.zeros((sum(p for p, _ in dtiles), 2 * W), _f8)
        r0 = 0
        for p, blks in dtiles:
            for s, bi in enumerate(blks):
                xud[r0 : r0 + rblocks[bi], s * W : s * W + W] = xu[
                    bi * 128 : bi * 128 + rblocks[bi]
                ]
            r0 += p
        in_maps.append({"xud": xud, "wcd": wcd})

    # epilogue: map each output position s to (group, local unique index)
    gid = np.empty(n_uni, np.int64)
    lix = np.empty(n_uni, np.int64)
    for g in range(NGRP):
        ids = groups[g][0]
        gid[ids] = g
        lix[ids] = np.arange(len(ids))
    # flat index into the per-batch stacked [NGRP*numax, D] result
    flat = gid[inv] * numax + lix[inv]  # [S]

    key = ("v2", rblocks, numax)
    return key, in_maps, (flat, numax)


def _host_prep(x, W_in, W_out, b_out, fusion_weights, routes):
    """v1 host prep. Returns (nk, nu, in_maps). Packs only the nonzero
    128-row source blocks of A^T (and the matching x blocks) per core,
    padded to the max count nk."""
    x = np.asarray(x, dtype=np.float32)
    W_in = np.asarray(W_in, dtype=np.float32)
    W_out = np.asarray(W_out, dtype=np.float32)
    b_out = np.asarray(b_out, dtype=np.float32)
    fw = np.asarray(fusion_weights, dtype=np.float32)
    rt = np.asarray(routes)

    Wc = (W_in @ W_out).astype(_f8)
    xb16 = [x[b].astype(_bf16) for b in range(B)]
    # residual + bias, pre-transposed: [D, QROWS] fp32 per (b, q)
    xrb = [
        [
            np.ascontiguousarray(x[b, q * QROWS : (q + 1) * QROWS].T)
            + b_out[:, None]
            for q in range(4)
        ]
        for b in range(B)
    ]

    # densify A^T per seq-quarter and find its nonzero source blocks
    cols = np.repeat(np.arange(QROWS, dtype=np.int64), K)
    at_q = []
    kset_q = []
    for q in range(4):
        r = rt[q * QROWS : (q + 1) * QROWS].astype(np.int64).ravel()
        a = np.zeros((S, QROWS), np.float32)
        np.add.at(a, (r, cols), fw[q * QROWS : (q + 1) * QROWS].ravel())
        blocks = a.reshape(KBLK, 128, QROWS)
        ks = [k for k in range(KBLK) if np.any(blocks[k])]
        if not ks:
            ks = [0]
        at_q.append(a.astype(_bf16))
        kset_q.append(ks)

    nk = max(len(ks) for ks in kset_q)

    fused = nk <= FUSED_NK_MAX
    # distinct-column compression: for Cantor routing many output positions
    # share identical A^T columns; contract over the unique columns and
    # expand with a one-hot matmul when they all fit in one 128-partition
    # tile.
    nu = 0
    uniq_q = None
    if fused:
        uniq_q = []
        for q in range(4):
            u16 = at_q[q].view(np.uint16)
            uc, inv = np.unique(u16.T, axis=0, return_inverse=True)
            uniq_q.append((uc, inv))
        if max(len(uc) for uc, _ in uniq_q) <= 128:
            nu = 128

    in_maps = []
    for c in range(NCORES):
        b, q = divmod(c, 4)
        ks = kset_q[q]
        if nu:
            uc, inv = uniq_q[q]
            atu_full = np.ascontiguousarray(uc.T).view(_bf16)  # [S, Uq]
            at_p = np.zeros((nk * 128, nu), _bf16)
            for i, k in enumerate(ks):
                at_p[i * 128 : (i + 1) * 128, : uc.shape[0]] = atu_full[
                    k * 128 : (k + 1) * 128
                ]
            sel_p = np.zeros((nu, QROWS), _bf16)
            sel_p[inv, np.arange(QROWS)] = _bf16(1.0)
            m = {"at": at_p, "sel": sel_p, "wc": Wc, "xrb": xrb[b][q]}
        else:
            at_p = np.zeros((nk * 128, QROWS), _bf16)
            for i, k in enumerate(ks):
                at_p[i * 128 : (i + 1) * 128] = at_q[q][k * 128 : (k + 1) * 128]
            m = {"at": at_p, "wc": Wc, "xrb": xrb[b][q]}
        if fused:
            xtp = np.zeros((D, nk * 128), _bf16)
            for i, k in enumerate(ks):
                xtp[:, i * 128 : (i + 1) * 128] = xb16[b][
                    k * 128 : (k + 1) * 128
                ].T
            m["xtp"] = xtp
        else:
            xb_p = np.zeros((nk * 128, D), _bf16)
            for i, k in enumerate(ks):
                xb_p[i * 128 : (i + 1) * 128] = xb16[b][k * 128 : (k + 1) * 128]
            m["xb"] = xb_p
        in_maps.append(m)
    return nk, nu, in_maps


def kernel(x, W_in, W_out, b_out, fusion_weights, routes):
    x = np.asarray(x, dtype=np.float32)
    b_out = np.asarray(b_out, dtype=np.float32)

    prep = _host_prep_v2(x, W_in, W_out, b_out, fusion_weights, routes)
    if prep is not None:
        key, in_maps, (flat, numax) = prep
        run = _get_runner(key)
        res = run(in_maps)
        out = np.empty((B, S, D), np.float32)
        for b in range(B):
            zall = np.concatenate(
                [res[b * NGRP + g]["zc"][:numax] for g in range(NGRP)], axis=0
            ).astype(np.float32)  # [NGRP*numax, D]
            out[b] = x[b] + zall[flat] + b_out
        return out

    nk, nu, in_maps = _host_prep(x, W_in, W_out, b_out, fusion_weights, routes)
    run = _get_runner(("v1", nk, nu))
    res = run(in_maps)
    out = np.empty((B, S, D), np.float32)
    for c in range(NCORES):
        b, q = divmod(c, 4)
        out[b, q * QROWS : (q + 1) * QROWS] = res[c]["outT"].T
    return out


# revision 74
# speedup vs baseline: 1.1398x; 1.0205x over previous
"""CantorMultiheadFusion kernel for 8 Trainium2 NeuronCores.

Math: out = x + A @ x @ (W_in @ W_out) + b_out, where A is the (S,S) sparse
fusion matrix with A[s, routes[s,k]] += fusion_weights[s,k].

Fast path (v2): the Cantor routing tables make A massively degenerate — the
whole sequence has only ~353 DISTINCT rows (output positions sharing an
identical (routes, weights) pattern), and those rows touch only ~1.4K distinct
source positions. Each core therefore computes only the unique fused rows:

    Zc = (U^T X)^T @ Wc        U: [nr, nu] unique-row table (nu <= 128)
                               X: [nr, D]  the used source rows of x
                               Wc = W_in @ W_out

sharded (batch b x unique-group g) over 8 cores, with the uniques split into
4 groups ordered by source-row locality so per-core (nu, nr) stay small. The
host expands Zc back to the full (B, S, D) output with a pure gather and adds
the residual x + b_out in fp32 (the same class of host epilogue work the v1
path did when assembling its xrb residual tensor and transposed output).

Per-core HBM traffic is ~0.57MB (vs ~6MB for v1): xs+U^T packed into wide
fp8 e4m3 tensors, Wc fp8, and a [nu, D] bf16 result. On-device matmuls run
fp8 with fp32 PSUM accumulation (validated 8.1e-3 relative error against the
2e-2 budget).

The v1 kernel (dense block-matmul on A^T) is kept as a fallback for routing
tables without enough structure (e.g. uniform-random routes).
"""

import numpy as np
import ml_dtypes

B, S, D, K = 2, 4096, 512, 32
NCORES = 8
QROWS = S // 4  # rows per core = 1024
DBLK = D // 128  # 4
KBLK = S // 128  # 32
NGRP = 4  # unique-row groups per batch (v2)

_bf16 = ml_dtypes.bfloat16
_f8 = ml_dtypes.float8_e4m3

_cache = {}


FUSED_NK_MAX = 8

# v2 tuning knobs
V2_WARMUP = 6  # PE p-state warm-up matmuls
V2_FILL = 0  # PE keep-busy matmuls between phase 1 and phase 2


def _build_module_v2(rblocks, nu):
    """v2 module. Inputs per core:

    - xud: r-blocks of (x rows | U^T) packed 2-up into DMA tiles.
    - wcd [256, 1024] fp8: Wc 2-up packed (tile t row r = Wc rows 2t*128+r
      and (2t+1)*128+r); d01 via the Act HWDGE slot, d23 via SP's second
      HWDGE slot (last).
    - zc [nu, 512] bf16 out: the unique fused+projected rows.

    Phase 1 (per r-block i): psZT[j][d, u] += xs_i[:, dblk j]^T @ ut_i
    Phase 2: two e-half psum chains over matmul(lhsT=zt_j, rhs=wc_j) in
    d-block arrival order (0, 1, 2, 3); each half copies out on its own
    engine and one HWDGE store writes the result.
    """
    import concourse.mybir as mybir
    import concourse.tile as tile
    from concourse import bacc

    f32 = mybir.dt.float32
    bf16 = mybir.dt.bfloat16
    f8 = mybir.dt.float8e4

    nrb = len(rblocks)
    W = 512 + nu

    # r-blocks packed 2-up into DMA tiles (tile t holds blocks 2t, 2t+1 side
    # by side) so the x+U stream needs only ceil(nrb/2) HWDGE generations
    # while m1 still pipelines per tile.
    dtiles = []  # (partitions, [r-block indices])
    i = 0
    while i < nrb:
        if i + 1 < nrb:
            dtiles.append((max(rblocks[i], rblocks[i + 1]), [i, i + 1]))
            i += 2
        else:
            dtiles.append((rblocks[i], [i]))
            i += 1

    nc = bacc.Bacc("TRN2", target_bir_lowering=True)

    # fp8 e4m3 for the whole input stream (x, U^T, Wc): the matmul error
    # lands at ~8e-3 relative on the final output, well inside the 2e-2
    # budget, and halves the HBM stream. PSUM accumulation stays fp32 and
    # the result leaves in bf16.
    xud = nc.dram_tensor(
        "xud", [sum(p for p, _ in dtiles), 2 * W], f8, kind="ExternalInput"
    )
    # wc 2-up packed: row r of tile t holds Wc rows (2t)*128+r | (2t+1)*128+r
    wcd = nc.dram_tensor("wcd", [256, 1024], f8, kind="ExternalInput")
    zc = nc.dram_tensor("zc", [nu, D], bf16, kind="ExternalOutput")

    with tile.TileContext(nc) as tc:
        with (
            tc.tile_pool(name="const", bufs=1) as cpool,
            tc.tile_pool(name="work", bufs=1) as wpool,
            tc.tile_pool(name="psum", bufs=1, space="PSUM") as ppool,
        ):
            # PE p-state warm-up on a memset tile (no DMA dependency);
            # memset on DVE so the Pool queue's head slot goes to the
            # trailing xud tile's descriptor generation.
            wu = cpool.tile([128, 128], bf16, tag="wu")
            nc.vector.memset(wu, 0.0)
            ps_w = ppool.tile([128, 512], f32, tag="ps_w")
            for _ in range(V2_WARMUP):
                nc.tensor.matmul(ps_w[:, :128], wu, wu, start=True, stop=True)

            # streamed loads: xud tiles first, wc tiles last
            # xud0 on SP (first HWDGE slot); the trailing xud tile via the
            # Pool queue — its SWDGE generation finishes by ~2.9us so it
            # transfers right behind xud0 instead of waiting for the Act
            # HWDGE slot's 784ns DGE delay (which left a 339ns hole and
            # delayed everything chained on the last x block).
            xu_sb = {}  # r-block index -> (tile, col offset)
            r0 = 0
            for ti, (p, blks) in enumerate(dtiles):
                tw = len(blks) * W
                t = cpool.tile([p, tw], f8, tag=f"xud{ti}", name=f"xud{ti}")
                eng = (nc.sync, nc.gpsimd)[min(ti, 1)]
                eng.dma_start(out=t, in_=xud[r0 : r0 + p, :tw])
                for s, i in enumerate(blks):
                    xu_sb[i] = (t, s * W)
                r0 += p
            # d01 via the Act HWDGE slot (transfers 3rd), d23 via SP's
            # second HWDGE slot (transfers last); the chain consumes in
            # arrival order 0,1,2,3. (Arrival order only affects timing —
            # every consumer waits its own DMA semaphore.)
            wc01 = cpool.tile([128, 1024], f8, tag="wc01")
            nc.scalar.dma_start(out=wc01, in_=wcd[0:128, :])
            wc23 = cpool.tile([128, 1024], f8, tag="wc23")
            nc.sync.dma_start(out=wc23, in_=wcd[128:256, :])
            wc_rhs = [
                wc01[:, 0:512],
                wc01[:, 512:1024],
                wc23[:, 0:512],
                wc23[:, 512:1024],
            ]

            # phase 1: psZT[j] = sum_i xs_i[:, dblk j]^T @ ut_i   -> [128, nu]
            ps_zt = [
                ppool.tile([128, nu], f32, tag=f"pzt{j}", name=f"pzt{j}")
                for j in range(DBLK)
            ]
            # region-outer: region j's accumulation completes at its own
            # last-block matmul, so zt copies and the phase-2 chain start
            # ~300ns before the whole phase finishes (the trailing xud tile
            # lands ~170ns into phase 1, so the cross-tile stall is tiny)
            for j in range(DBLK):
                for i in range(nrb):
                    t, coff = xu_sb[i]
                    nc.tensor.matmul(
                        ps_zt[j],
                        t[:, coff + j * 128 : coff + (j + 1) * 128],
                        t[:, coff + 512 : coff + W],
                        start=(i == 0),
                        stop=(i == nrb - 1),
                    )
            # fp8 stage in chain-consumption order (links run 0,1,2,3)
            zt_sb = [None] * DBLK
            for n, j in enumerate((0, 1, 2, 3)):
                t = wpool.tile([128, nu], f8, tag=f"zt{j}", name=f"zt{j}")
                if n % 2 == 0:
                    nc.vector.tensor_copy(t, ps_zt[j])
                else:
                    nc.scalar.activation(
                        t, ps_zt[j], mybir.ActivationFunctionType.Copy
                    )
                zt_sb[j] = t

            # keep the PE p-state hot across the psum-copy gap
            for _ in range(V2_FILL):
                nc.tensor.matmul(ps_w[:, :128], wu, wu, start=True, stop=True)

            # phase 2: two e-half chains (each link pair gated by the same wc
            # block, so scheduler interleaving cannot stall); the first
            # half's narrower copy overlaps the second half's links. Chain
            # consumes d-blocks in arrival order (2, 3, 0, 1).
            ps_a = ppool.tile([nu, D // 2], f32, tag="pza")
            ps_b = ppool.tile([nu, D // 2], f32, tag="pzb")
            order = (0, 1, 2, 3)
            for n, j in enumerate(order):
                for h, ps in enumerate((ps_a, ps_b)):
                    nc.tensor.matmul(
                        ps,
                        zt_sb[j],
                        wc_rhs[j][:, h * 256 : (h + 1) * 256],
                        start=(n == 0),
                        stop=(n == DBLK - 1),
                    )
            # first-finishing half on ACT (slow completion defer tolerable),
            # last half on DVE (58-cycle defer vs ACT's 222 → the store's
            # HWDGE generation starts earlier)
            o = wpool.tile([nu, D], bf16, tag="o")
            nc.scalar.activation(
                o[:, :256], ps_a, mybir.ActivationFunctionType.Copy
            )
            nc.vector.tensor_copy(o[:, 256:], ps_b)
            nc.sync.dma_start(out=zc[:, :], in_=o)

    nc.finalize()
    return nc


def _build_module(nk=KBLK, nu=0):
    """v1 fallback module. Two variants by nk:

    - fused (nk <= FUSED_NK_MAX): phase P projects the packed x blocks by Wc
      first (xc = x_sel @ Wc, cheap since only nk blocks), then a single
      accumulation phase A' computes outT = xc_sel^T-chain @ A^T. Phase P
      fills the startup hole while the A^T stream is still arriving, and
      there is no post-phase projection tail.
    - split (nk > FUSED_NK_MAX): big phase A (x^T-chain @ A^T) then a small
      projection phase B by Wc. Cheaper when nk is large because P would
      scale with nk while B is constant.
    """
    import concourse.mybir as mybir
    import concourse.tile as tile
    from concourse import bacc

    f32 = mybir.dt.float32
    bf16 = mybir.dt.bfloat16
    fused = nk <= FUSED_NK_MAX
    # nu > 0: additionally compress A^T to its nu (<=128) distinct columns
    # and expand the result back with a one-hot selection matmul.
    dedup = fused and nu > 0

    nc = bacc.Bacc("TRN2", target_bir_lowering=True)

    if fused:
        # packed x^T: [D, nk*128]; entry [d, i*128 + c] = x_block_i[c, d]
        xtp = nc.dram_tensor("xtp", [D, nk * 128], bf16, kind="ExternalInput")
    else:
        xb = nc.dram_tensor("xb", [nk * 128, D], bf16, kind="ExternalInput")
    if dedup:
        at = nc.dram_tensor("at", [nk * 128, nu], bf16, kind="ExternalInput")
        sel = nc.dram_tensor("sel", [nu, QROWS], bf16, kind="ExternalInput")
    else:
        at = nc.dram_tensor("at", [nk * 128, QROWS], bf16, kind="ExternalInput")
    wc = nc.dram_tensor("wc", [D, D], bf16, kind="ExternalInput")
    xrb = nc.dram_tensor("xrb", [D, QROWS], f32, kind="ExternalInput")
    outT = nc.dram_tensor("outT", [D, QROWS], f32, kind="ExternalOutput")

    with tile.TileContext(nc) as tc:
        with (
            tc.tile_pool(name="const", bufs=1) as cpool,
            tc.tile_pool(name="work", bufs=3) as wpool,
            tc.tile_pool(name="psum", bufs=8 if fused else 4, space="PSUM") as ppool,
        ):
            # PE warm-up: matmuls on a memset tile (no DMA dependency) fill
            # the DMA-latency startup hole and lift the HAM clock gate to
            # 8/8 before the real chains start.
            wu = cpool.tile([128, 128], bf16, tag="wu")
            nc.gpsimd.memset(wu, 0.0)
            ps_w = ppool.tile(
                [128, 512], f32, tag="ps" if fused else "ps2", name="ps_w"
            )
            for _ in range(23):
                nc.tensor.matmul(ps_w[:, :128], wu, wu, start=True, stop=True)
            wu2 = wpool.tile([128, 1], bf16, tag="wu2")
            nc.vector.tensor_copy(wu2, ps_w[:, :1])  # release the bank

            # --- streamed loads ---------------------------------------------
            if fused:
                wc_sb = []
                xtp_sb = []  # x^T tile per d1: [128, nk*128], block i at cols i*128
                for d1 in range(DBLK):
                    t = cpool.tile([128, D], bf16, tag=f"wc{d1}")
                    nc.gpsimd.dma_start(out=t, in_=wc[d1 * 128 : (d1 + 1) * 128, :])
                    wc_sb.append(t)
                    t = cpool.tile([128, nk * 128], bf16, tag=f"xtp{d1}")
                    nc.sync.dma_start(
                        out=t, in_=xtp[d1 * 128 : (d1 + 1) * 128, :]
                    )
                    xtp_sb.append(t)
            else:
                xb_sb = []  # packed x[b] row-block k: [128, D]
                for k in range(nk):
                    t = cpool.tile([128, D], bf16, tag=f"xb{k}")
                    nc.sync.dma_start(out=t, in_=xb[k * 128 : (k + 1) * 128, :])
                    xb_sb.append(t)

            sel_sb = None
            if dedup:
                sel_sb = cpool.tile([nu, QROWS], bf16, tag="sel")
                nc.scalar.dma_start(out=sel_sb, in_=sel[:, :])

            atw = nu if dedup else QROWS
            at_sb = []  # packed A^T row-block k: [128, atw]
            for k in range(nk):
                t = cpool.tile([128, atw], bf16, tag=f"at{k}")
                if fused:
                    # spread the stream over all three DMA queues so it has
                    # fully landed before phase A' consumes it back-to-back
                    eng = (nc.scalar, nc.scalar, nc.sync, nc.gpsimd)[k % 4]
                else:
                    eng = nc.scalar
                eng.dma_start(out=t, in_=at[k * 128 : (k + 1) * 128, :])
                at_sb.append(t)

            if not fused:
                wc_sb = []
                for d1 in range(DBLK):
                    t = cpool.tile([128, D], bf16, tag=f"wc{d1}")
                    nc.sync.dma_start(out=t, in_=wc[d1 * 128 : (d1 + 1) * 128, :])
                    wc_sb.append(t)

            xrb_sb = []  # (x^T + b_out) block d2: [128, QROWS] fp32
            for d2 in range(DBLK):
                t = cpool.tile([128, QROWS], f32, tag=f"xrb{d2}")
                eng = nc.gpsimd if fused else nc.sync
                eng.dma_start(out=t, in_=xrb[d2 * 128 : (d2 + 1) * 128, :])
                xrb_sb.append(t)

            if fused:
                # --- phase P: xc[i] = x_block[i] @ Wc ------------------------
                # d1 outer: paced by the (xtp[d1], wc[d1]) tile arrivals, all
                # nk accumulation groups advance together.
                ps_p = [
                    ppool.tile([128, D], f32, tag="ps", name=f"ps_p{i}")
                    for i in range(nk)
                ]
                for d1 in range(DBLK):
                    for i in range(nk):
                        nc.tensor.matmul(
                            ps_p[i],
                            xtp_sb[d1][:, i * 128 : (i + 1) * 128],
                            wc_sb[d1],
                            start=(d1 == 0),
                            stop=(d1 == DBLK - 1),
                        )
                xc_sb = []
                for i in range(nk):
                    t = wpool.tile([128, D], bf16, tag=f"xc{i % 4}", name=f"xc{i}")
                    if i % 2 == 0:
                        nc.vector.tensor_copy(t, ps_p[i])
                    else:
                        nc.scalar.activation(
                            t, ps_p[i], mybir.ActivationFunctionType.Copy
                        )
                    xc_sb.append(t)

                if dedup:
                    # --- phase A'': zUn[u, d2] = sum_i atU[i]^T @ xc[i] ------
                    ps_u = ppool.tile([nu, D], f32, tag="ps", name="ps_u")
                    for i in range(nk):
                        nc.tensor.matmul(
                            ps_u,
                            at_sb[i],
                            xc_sb[i],
                            start=(i == 0),
                            stop=(i == nk - 1),
                        )
                    zun = []  # per-d2-block [nu, 128] so deps are precise
                    # only d2=0 on DVE: keeps the DVE queue clear for the
                    # 8-add epilogue chain that follows immediately
                    for d2 in range(DBLK):
                        t = wpool.tile([nu, 128], bf16, tag=f"zun{d2}")
                        if d2 == 0:
                            nc.vector.tensor_copy(
                                t, ps_u[:, d2 * 128 : (d2 + 1) * 128]
                            )
                        else:
                            nc.scalar.activation(
                                t,
                                ps_u[:, d2 * 128 : (d2 + 1) * 128],
                                mybir.ActivationFunctionType.Copy,
                            )
                        zun.append(t)

                    # --- expand: outT[d2, s] = zUn-col-d2 ^T @ Sel + xrb -----
                    for d2 in range(DBLK):
                        for h in range(2):
                            hs = slice(h * 512, (h + 1) * 512)
                            ps_e = ppool.tile(
                                [128, 512], f32, tag="ps", name=f"ps_e{d2}_{h}"
                            )
                            nc.tensor.matmul(
                                ps_e,
                                zun[d2],
                                sel_sb[:, hs],
                                start=True,
                                stop=True,
                            )
                            o = wpool.tile(
                                [128, 512], f32, tag=f"osb{h}", name=f"o{d2}_{h}"
                            )
                            nc.vector.tensor_tensor(
                                o,
                                ps_e,
                                xrb_sb[d2][:, hs],
                                mybir.AluOpType.add,
                            )
                            ring = nc.sync if (d2 + h) % 2 == 0 else nc.scalar
                            ring.dma_start(
                                out=outT[d2 * 128 : (d2 + 1) * 128, hs],
                                in_=o,
                            )
                    _done = True
                else:
                    _done = False

                # --- phase A': outT-psum[d2,h] = xc-chain @ A^T --------------
                # group outer: each (d2, h) output group finishes its whole
                # block chain early so its residual-add + store pipeline
                # behind the PE while later groups stream.
                for d2 in range(DBLK) if not _done else []:
                    o = wpool.tile([128, QROWS], f32, tag="osb", name=f"osb{d2}")
                    for h in range(2):
                        hs = slice(h * 512, (h + 1) * 512)
                        ps_o = ppool.tile(
                            [128, 512], f32, tag="ps", name=f"ps_o{d2}_{h}"
                        )
                        for i in range(nk):
                            nc.tensor.matmul(
                                ps_o,
                                xc_sb[i][:, d2 * 128 : (d2 + 1) * 128],
                                at_sb[i][:, h * 512 : (h + 1) * 512],
                                start=(i == 0),
                                stop=(i == nk - 1),
                            )
                        nc.vector.tensor_tensor(
                            o[:, hs],
                            ps_o,
                            xrb_sb[d2][:, hs],
                            mybir.AluOpType.add,
                        )
                        ring = nc.sync if (d2 + h) % 2 == 0 else nc.scalar
                        ring.dma_start(
                            out=outT[d2 * 128 : (d2 + 1) * 128, hs], in_=o[:, hs]
                        )
            else:
                # --- phase A: axT[d] = x-block-col-d ^T @ A^T ----------------
                # k outer / d inner: each at-tile is consumed right after its
                # DMA lands, so the PE never waits on the A^T stream.
                ps_a = [
                    ppool.tile([128, QROWS], f32, tag="ps2", name=f"ps_a{d}")
                    for d in range(DBLK)
                ]
                for k in range(nk):
                    for d in range(DBLK):
                        for h in range(2):
                            nc.tensor.matmul(
                                ps_a[d][:, h * 512 : (h + 1) * 512],
                                xb_sb[k][:, d * 128 : (d + 1) * 128],
                                at_sb[k][:, h * 512 : (h + 1) * 512],
                                start=(k == 0),
                                stop=(k == nk - 1),
                            )
                axT = []
                for d in range(DBLK):
                    t = wpool.tile([128, QROWS], bf16, tag=f"axT{d}")
                    if d % 2 == 0:
                        nc.vector.tensor_copy(t, ps_a[d])
                    else:
                        nc.scalar.activation(
                            t, ps_a[d], mybir.ActivationFunctionType.Copy
                        )
                    axT.append(t)

                # --- phase B: outT[d2] = Wc-chain @ axT + (x^T + b_out) ------
                for d2 in range(DBLK):
                    ps_b = ppool.tile(
                        [128, QROWS], f32, tag="ps2", name=f"ps_b{d2}"
                    )
                    for d1 in range(DBLK):
                        for h in range(2):
                            nc.tensor.matmul(
                                ps_b[:, h * 512 : (h + 1) * 512],
                                wc_sb[d1][:, d2 * 128 : (d2 + 1) * 128],
                                axT[d1][:, h * 512 : (h + 1) * 512],
                                start=(d1 == 0),
                                stop=(d1 == DBLK - 1),
                            )
                    for h in range(2):
                        hs = slice(h * 512, (h + 1) * 512)
                        o = wpool.tile(
                            [128, 512], f32, tag=f"osb{h}", name=f"o{d2}_{h}"
                        )
                        nc.vector.tensor_tensor(
                            o,
                            ps_b[:, hs],
                            xrb_sb[d2][:, hs],
                            mybir.AluOpType.add,
                        )
                        ring = nc.sync if (d2 + h) % 2 == 0 else nc.scalar
                        ring.dma_start(
                            out=outT[d2 * 128 : (d2 + 1) * 128, hs], in_=o
                        )

    nc.finalize()
    return nc


def _get_runner(key):
    """Compile once per module key; return a callable(in_maps) -> out dicts.

    key: ("v1", nk, nu) or ("v2", rblocks_tuple, nu).
    """
    ckey = ("runner", key)
    if ckey in _cache:
        return _cache[ckey]

    import jax
    from jax.sharding import Mesh, PartitionSpec
    from jax.experimental.shard_map import shard_map
    from concourse import bass2jax
    import concourse.mybir as mybir

    bass2jax.install_neuronx_cc_hook()
    if key[0] == "v2":
        nc = _build_module_v2(key[1], key[2])
    else:
        nc = _build_module(key[1], key[2])

    part_name = nc.partition_id_tensor.name if nc.partition_id_tensor else None
    in_names = []
    out_names = []
    out_avals = []
    for alloc in nc.m.functions[0].allocations:
        if not isinstance(alloc, bass2jax.mybir.MemoryLocationSet):
            continue
        name = alloc.memorylocations[0].name
        if alloc.kind == "ExternalInput":
            if name != part_name:
                in_names.append(name)
        elif alloc.kind == "ExternalOutput":
            out_names.append(name)
            out_avals.append(
                jax.core.ShapedArray(
                    tuple(alloc.tensor_shape), mybir.dt.np(alloc.dtype)
                )
            )
    n_params = len(in_names)
    all_names = in_names + out_names
    if part_name is not None:
        all_names = all_names + [part_name]

    def _body(*args):
        operands = list(args)
        if part_name is not None:
            operands.append(bass2jax.partition_id_tensor())
        outs = bass2jax._bass_exec_p.bind(
            *operands,
            out_avals=tuple(out_avals),
            in_names=tuple(all_names),
            out_names=tuple(out_names),
            lowering_input_output_aliases=(),
            sim_require_finite=True,
            sim_require_nnan=True,
            nc=nc,
        )
        return tuple(outs)

    devices = jax.devices()[:NCORES]
    mesh = Mesh(np.asarray(devices), ("core",))
    nin = n_params + len(out_names)
    sharded = jax.jit(
        shard_map(
            _body,
            mesh=mesh,
            in_specs=(PartitionSpec("core"),) * nin,
            out_specs=(PartitionSpec("core"),) * len(out_names),
            check_rep=False,
        ),
        keep_unused=True,
    )

    zero_shapes = [(NCORES * a.shape[0], *a.shape[1:]) for a in out_avals]
    zero_dtypes = [a.dtype for a in out_avals]

    def run(in_maps):
        concat_in = [
            np.concatenate([np.asarray(m[name]) for m in in_maps], axis=0)
            for name in in_names
        ]
        zeros = [np.zeros(s, d) for s, d in zip(zero_shapes, zero_dtypes)]
        out_arrs = sharded(*concat_in, *zeros)
        jax.block_until_ready(out_arrs)
        res = [
            {
                name: np.asarray(out_arrs[i]).reshape(NCORES, *out_avals[i].shape)[c]
                for i, name in enumerate(out_names)
            }
            for c in range(NCORES)
        ]
        return res

    _cache[ckey] = run
    _cache[("sharded", key)] = sharded
    _cache[("meta", key)] = (in_names, out_names, out_avals)
    return run


def _host_prep_v2(x, W_in, W_out, b_out, fusion_weights, routes):
    """Fast-path host prep. Returns None if the routing tables don't have
    enough duplicate structure (falls back to v1), else
    (key, in_maps, epilogue_meta)."""
    x = np.asarray(x, dtype=np.float32)
    W_in = np.asarray(W_in, dtype=np.float32)
    W_out = np.asarray(W_out, dtype=np.float32)
    fw = np.asarray(fusion_weights, dtype=np.float32)
    rt = np.asarray(routes)

    # dedup output rows by exact (routes, weights) byte pattern
    pat = np.concatenate(
        [np.ascontiguousarray(rt).view(np.uint8),
         np.ascontiguousarray(fw).view(np.uint8)],
        axis=1,
    )
    _, uidx, inv = np.unique(pat, axis=0, return_index=True, return_inverse=True)
    inv = inv.ravel()
    n_uni = len(uidx)
    if n_uni > NGRP * 128:
        return None

    rt64 = rt.astype(np.int64)
    # per-unique source rows; group uniques by source-row locality
    srcs = [np.unique(rt64[i]) for i in uidx]
    minrow = np.array([s[0] for s in srcs])
    order = np.argsort(minrow, kind="stable")
    bounds = [round(n_uni * g / NGRP) for g in range(NGRP + 1)]
    groups = []  # (ids, rows, ut)
    numax = nrmax = 0
    for g in range(NGRP):
        ids = order[bounds[g] : bounds[g + 1]]
        if len(ids) == 0:
            ids = order[:1]
        rows = np.unique(np.concatenate([srcs[i] for i in ids]))
        nu_c, nr_c = len(ids), len(rows)
        if nu_c > 128:
            return None
        ut = np.zeros((nr_c, nu_c), np.float32)
        ri = np.searchsorted(rows, rt64[uidx[ids]].ravel())
        uu = np.repeat(np.arange(nu_c), K)
        np.add.at(ut, (ri, uu), fw[uidx[ids]].ravel())
        groups.append((ids, rows, ut))
        numax = max(numax, nu_c)
        nrmax = max(nrmax, nr_c)

    if nrmax > 16 * 128:
        return None

    rblocks = []
    left = nrmax
    while left > 0:
        rblocks.append(min(128, left))
        left -= 128
    rblocks = tuple(rblocks)

    Wc = (W_in @ W_out).astype(_f8)
    # 2-up packing: tile t row r = Wc rows (2t)*128+r (cols 0:512) and
    # (2t+1)*128+r (cols 512:1024)
    wcd = np.zeros((256, 1024), _f8)
    for t in range(2):
        wcd[t * 128 : (t + 1) * 128, :512] = Wc[2 * t * 128 : (2 * t + 1) * 128]
        wcd[t * 128 : (t + 1) * 128, 512:] = Wc[(2 * t + 1) * 128 : (2 * t + 2) * 128]

    # 2-up r-block packing mirroring _build_module_v2
    W = 512 + numax
    dtiles = []
    i = 0
    while i < len(rblocks):
        if i + 1 < len(rblocks):
            dtiles.append((max(rblocks[i], rblocks[i + 1]), [i, i + 1]))
            i += 2
        else:
            dtiles.append((rblocks[i], [i]))
            i += 1

    in_maps = []
    for c in range(NCORES):
        b, g = divmod(c, NGRP)
        ids, rows, ut = groups[g]
        xu = np.zeros((len(rblocks) * 128, W), _f8)
        xu[: len(rows), :512] = x[b][rows].astype(_f8)
        xu[: len(rows), 512 : 512 + ut.shape[1]] = ut.astype(_f8)
        xud = np.zeros((sum(p for p, _ in dtiles), 2 * W), _f8)
        r0 = 0
        for p, blks in dtiles:
            for s, bi in enumerate(blks):
                xud[r0 : r0 + rblocks[bi], s * W : s * W + W] = xu[
                    bi * 128 : bi * 128 + rblocks[bi]
                ]
            r0 += p
        in_maps.append({"xud": xud, "wcd": wcd})

    # epilogue: map each output position s to (group, local unique index)
    gid = np.empty(n_uni, np.int64)
    lix = np.empty(n_uni, np.int64)
    for g in range(NGRP):
        ids = groups[g][0]
        gid[ids] = g
        lix[ids] = np.arange(len(ids))
    # flat index into the per-batch stacked [NGRP*numax, D] result
    flat = gid[inv] * numax + lix[inv]  # [S]

    key = ("v2", rblocks, numax)
    return key, in_maps, (flat, numax)


def _host_prep(x, W_in, W_out, b_out, fusion_weights, routes):
    """v1 host prep. Returns (nk, nu, in_maps). Packs only the nonzero
    128-row source blocks of A^T (and the matching x blocks) per core,
    padded to the max count nk."""
    x = np.asarray(x, dtype=np.float32)
    W_in = np.asarray(W_in, dtype=np.float32)
    W_out = np.asarray(W_out, dtype=np.float32)
    b_out = np.asarray(b_out, dtype=np.float32)
    fw = np.asarray(fusion_weights, dtype=np.float32)
    rt = np.asarray(routes)

    Wc = (W_in @ W_out).astype(_f8)
    xb16 = [x[b].astype(_bf16) for b in range(B)]
    # residual + bias, pre-transposed: [D, QROWS] fp32 per (b, q)
    xrb = [
        [
            np.ascontiguousarray(x[b, q * QROWS : (q + 1) * QROWS].T)
            + b_out[:, None]
            for q in range(4)
        ]
        for b in range(B)
    ]

    # densify A^T per seq-quarter and find its nonzero source blocks
    cols = np.repeat(np.arange(QROWS, dtype=np.int64), K)
    at_q = []
    kset_q = []
    for q in range(4):
        r = rt[q * QROWS : (q + 1) * QROWS].astype(np.int64).ravel()
        a = np.zeros((S, QROWS), np.float32)
        np.add.at(a, (r, cols), fw[q * QROWS : (q + 1) * QROWS].ravel())
        blocks = a.reshape(KBLK, 128, QROWS)
        ks = [k for k in range(KBLK) if np.any(blocks[k])]
        if not ks:
            ks = [0]
        at_q.append(a.astype(_bf16))
        kset_q.append(ks)

    nk = max(len(ks) for ks in kset_q)

    fused = nk <= FUSED_NK_MAX
    # distinct-column compression: for Cantor routing many output positions
    # share identical A^T columns; contract over the unique columns and
    # expand with a one-hot matmul when they all fit in one 128-partition
    # tile.
    nu = 0
    uniq_q = None
    if fused:
        uniq_q = []
        for q in range(4):
            u16 = at_q[q].view(np.uint16)
            uc, inv = np.unique(u16.T, axis=0, return_inverse=True)
            uniq_q.append((uc, inv))
        if max(len(uc) for uc, _ in uniq_q) <= 128:
            nu = 128

    in_maps = []
    for c in range(NCORES):
        b, q = divmod(c, 4)
        ks = kset_q[q]
        if nu:
            uc, inv = uniq_q[q]
            atu_full = np.ascontiguousarray(uc.T).view(_bf16)  # [S, Uq]
            at_p = np.zeros((nk * 128, nu), _bf16)
            for i, k in enumerate(ks):
                at_p[i * 128 : (i + 1) * 128, : uc.shape[0]] = atu_full[
                    k * 128 : (k + 1) * 128
                ]
            sel_p = np.zeros((nu, QROWS), _bf16)
            sel_p[inv, np.arange(QROWS)] = _bf16(1.0)
            m = {"at": at_p, "sel": sel_p, "wc": Wc, "xrb": xrb[b][q]}
        else:
            at_p = np.zeros((nk * 128, QROWS), _bf16)
            for i, k in enumerate(ks):
                at_p[i * 128 : (i + 1) * 128] = at_q[q][k * 128 : (k + 1) * 128]
            m = {"at": at_p, "wc": Wc, "xrb": xrb[b][q]}
        if fused:
            xtp = np.zeros((D, nk * 128), _bf16)
            for i, k in enumerate(ks):
                xtp[:, i * 128 : (i + 1) * 128] = xb16[b][
                    k * 128 : (k + 1) * 128
                ].T
            m["xtp"] = xtp
        else:
            xb_p = np.zeros((nk * 128, D), _bf16)
            for i, k in enumerate(ks):
                xb_p[i * 128 : (i + 1) * 128] = xb16[b][k * 128 : (k + 1) * 128]
            m["xb"] = xb_p
        in_maps.append(m)
    return nk, nu, in_maps


def kernel(x, W_in, W_out, b_out, fusion_weights, routes):
    x = np.asarray(x, dtype=np.float32)
    b_out = np.asarray(b_out, dtype=np.float32)

    prep = _host_prep_v2(x, W_in, W_out, b_out, fusion_weights, routes)
    if prep is not None:
        key, in_maps, (flat, numax) = prep
        run = _get_runner(key)
        res = run(in_maps)
        out = np.empty((B, S, D), np.float32)
        for b in range(B):
            zall = np.concatenate(
                [res[b * NGRP + g]["zc"][:numax] for g in range(NGRP)], axis=0
            ).astype(np.float32)  # [NGRP*numax, D]
            out[b] = x[b] + zall[flat] + b_out
        return out

    nk, nu, in_maps = _host_prep(x, W_in, W_out, b_out, fusion_weights, routes)
    run = _get_runner(("v1", nk, nu))
    res = run(in_maps)
    out = np.empty((B, S, D), np.float32)
    for c in range(NCORES):
        b, q = divmod(c, 4)
        out[b, q * QROWS : (q + 1) * QROWS] = res[c]["outT"].T
    return out


# revision 75
# speedup vs baseline: 1.1559x; 1.0142x over previous
"""CantorMultiheadFusion kernel for 8 Trainium2 NeuronCores.

Math: out = x + A @ x @ (W_in @ W_out) + b_out, where A is the (S,S) sparse
fusion matrix with A[s, routes[s,k]] += fusion_weights[s,k].

Fast path (v2): the Cantor routing tables make A massively degenerate — the
whole sequence has only ~353 DISTINCT rows (output positions sharing an
identical (routes, weights) pattern), and those rows touch only ~1.4K distinct
source positions. Each core therefore computes only the unique fused rows:

    Zc = (U^T X)^T @ Wc        U: [nr, nu] unique-row table (nu <= 128)
                               X: [nr, D]  the used source rows of x
                               Wc = W_in @ W_out

sharded (batch b x unique-group g) over 8 cores, with the uniques split into
4 groups ordered by source-row locality so per-core (nu, nr) stay small. The
host expands Zc back to the full (B, S, D) output with a pure gather and adds
the residual x + b_out in fp32 (the same class of host epilogue work the v1
path did when assembling its xrb residual tensor and transposed output).

Per-core HBM traffic is ~0.57MB (vs ~6MB for v1): xs+U^T packed into wide
fp8 e4m3 tensors, Wc fp8, and a [nu, D] bf16 result. On-device matmuls run
fp8 with fp32 PSUM accumulation (validated 8.1e-3 relative error against the
2e-2 budget).

The v1 kernel (dense block-matmul on A^T) is kept as a fallback for routing
tables without enough structure (e.g. uniform-random routes).
"""

import numpy as np
import ml_dtypes

B, S, D, K = 2, 4096, 512, 32
NCORES = 8
QROWS = S // 4  # rows per core = 1024
DBLK = D // 128  # 4
KBLK = S // 128  # 32
NGRP = 4  # unique-row groups per batch (v2)

_bf16 = ml_dtypes.bfloat16
_f8 = ml_dtypes.float8_e4m3

_cache = {}


FUSED_NK_MAX = 8

# v2 tuning knobs
V2_WARMUP = 6  # PE p-state warm-up matmuls
V2_FILL = 0  # PE keep-busy matmuls between phase 1 and phase 2


def _build_module_v2(rblocks, nu):
    """v2 module. Inputs per core:

    - xud: r-blocks of (x rows | U^T) packed 2-up into DMA tiles.
    - wcd [256, 1024] fp8: Wc 2-up packed (tile t row r = Wc rows 2t*128+r
      and (2t+1)*128+r); d01 via the Act HWDGE slot, d23 via SP's second
      HWDGE slot (last).
    - zc [nu, 512] bf16 out: the unique fused+projected rows.

    Phase 1 (per r-block i): psZT[j][d, u] += xs_i[:, dblk j]^T @ ut_i
    Phase 2: two e-half psum chains over matmul(lhsT=zt_j, rhs=wc_j) in
    d-block arrival order (0, 1, 2, 3); each half copies out on its own
    engine and one HWDGE store writes the result.
    """
    import concourse.mybir as mybir
    import concourse.tile as tile
    from concourse import bacc

    f32 = mybir.dt.float32
    bf16 = mybir.dt.bfloat16
    f8 = mybir.dt.float8e4

    nrb = len(rblocks)
    W = 512 + nu

    # r-blocks packed 2-up into DMA tiles (tile t holds blocks 2t, 2t+1 side
    # by side) so the x+U stream needs only ceil(nrb/2) HWDGE generations
    # while m1 still pipelines per tile.
    dtiles = []  # (partitions, [r-block indices])
    i = 0
    while i < nrb:
        if i + 1 < nrb:
            dtiles.append((max(rblocks[i], rblocks[i + 1]), [i, i + 1]))
            i += 2
        else:
            dtiles.append((rblocks[i], [i]))
            i += 1

    nc = bacc.Bacc("TRN2", target_bir_lowering=True)

    # fp8 e4m3 for the whole input stream (x, U^T, Wc): the matmul error
    # lands at ~8e-3 relative on the final output, well inside the 2e-2
    # budget, and halves the HBM stream. PSUM accumulation stays fp32 and
    # the result leaves in bf16.
    xud = nc.dram_tensor(
        "xud", [sum(p for p, _ in dtiles), 2 * W], f8, kind="ExternalInput"
    )
    # wc 2-up packed: row r of tile t holds Wc rows (2t)*128+r | (2t+1)*128+r
    wcd = nc.dram_tensor("wcd", [256, 1024], f8, kind="ExternalInput")
    # fp8 output too: the residual is added in fp32 on the host, so only
    # the delta term rounds (measured 9.3e-3 total vs the 2e-2 budget)
    zc = nc.dram_tensor("zc", [nu, D], f8, kind="ExternalOutput")

    with tile.TileContext(nc) as tc:
        with (
            tc.tile_pool(name="const", bufs=1) as cpool,
            tc.tile_pool(name="work", bufs=1) as wpool,
            tc.tile_pool(name="psum", bufs=1, space="PSUM") as ppool,
        ):
            # PE p-state warm-up on a memset tile (no DMA dependency);
            # memset on DVE so the Pool queue's head slot goes to the
            # trailing xud tile's descriptor generation.
            wu = cpool.tile([128, 128], bf16, tag="wu")
            nc.vector.memset(wu, 0.0)
            ps_w = ppool.tile([128, 512], f32, tag="ps_w")
            for _ in range(V2_WARMUP):
                nc.tensor.matmul(ps_w[:, :128], wu, wu, start=True, stop=True)

            # streamed loads: xud tiles first, wc tiles last
            # xud0 on SP (first HWDGE slot); the trailing xud tile via the
            # Pool queue — its SWDGE generation finishes by ~2.9us so it
            # transfers right behind xud0 instead of waiting for the Act
            # HWDGE slot's 784ns DGE delay (which left a 339ns hole and
            # delayed everything chained on the last x block).
            xu_sb = {}  # r-block index -> (tile, col offset)
            r0 = 0
            for ti, (p, blks) in enumerate(dtiles):
                tw = len(blks) * W
                t = cpool.tile([p, tw], f8, tag=f"xud{ti}", name=f"xud{ti}")
                eng = (nc.sync, nc.gpsimd)[min(ti, 1)]
                eng.dma_start(out=t, in_=xud[r0 : r0 + p, :tw])
                for s, i in enumerate(blks):
                    xu_sb[i] = (t, s * W)
                r0 += p
            # d01 via the Act HWDGE slot (transfers 3rd), d23 via SP's
            # second HWDGE slot (transfers last); the chain consumes in
            # arrival order 0,1,2,3. (Arrival order only affects timing —
            # every consumer waits its own DMA semaphore.)
            wc01 = cpool.tile([128, 1024], f8, tag="wc01")
            nc.scalar.dma_start(out=wc01, in_=wcd[0:128, :])
            wc23 = cpool.tile([128, 1024], f8, tag="wc23")
            nc.sync.dma_start(out=wc23, in_=wcd[128:256, :])
            wc_rhs = [
                wc01[:, 0:512],
                wc01[:, 512:1024],
                wc23[:, 0:512],
                wc23[:, 512:1024],
            ]

            # phase 1: psZT[j] = sum_i xs_i[:, dblk j]^T @ ut_i   -> [128, nu]
            ps_zt = [
                ppool.tile([128, nu], f32, tag=f"pzt{j}", name=f"pzt{j}")
                for j in range(DBLK)
            ]
            # region-outer: region j's accumulation completes at its own
            # last-block matmul, so zt copies and the phase-2 chain start
            # ~300ns before the whole phase finishes (the trailing xud tile
            # lands ~170ns into phase 1, so the cross-tile stall is tiny)
            for j in range(DBLK):
                for i in range(nrb):
                    t, coff = xu_sb[i]
                    nc.tensor.matmul(
                        ps_zt[j],
                        t[:, coff + j * 128 : coff + (j + 1) * 128],
                        t[:, coff + 512 : coff + W],
                        start=(i == 0),
                        stop=(i == nrb - 1),
                    )
            # fp8 stage in chain-consumption order (links run 0,1,2,3)
            zt_sb = [None] * DBLK
            for n, j in enumerate((0, 1, 2, 3)):
                t = wpool.tile([128, nu], f8, tag=f"zt{j}", name=f"zt{j}")
                if n % 2 == 0:
                    nc.vector.tensor_copy(t, ps_zt[j])
                else:
                    nc.scalar.activation(
                        t, ps_zt[j], mybir.ActivationFunctionType.Copy
                    )
                zt_sb[j] = t

            # keep the PE p-state hot across the psum-copy gap
            for _ in range(V2_FILL):
                nc.tensor.matmul(ps_w[:, :128], wu, wu, start=True, stop=True)

            # phase 2: two e-half chains (each link pair gated by the same wc
            # block, so scheduler interleaving cannot stall); the first
            # half's narrower copy overlaps the second half's links. Chain
            # consumes d-blocks in arrival order (2, 3, 0, 1).
            ps_a = ppool.tile([nu, D // 2], f32, tag="pza")
            ps_b = ppool.tile([nu, D // 2], f32, tag="pzb")
            order = (0, 1, 2, 3)
            for n, j in enumerate(order):
                for h, ps in enumerate((ps_a, ps_b)):
                    nc.tensor.matmul(
                        ps,
                        zt_sb[j],
                        wc_rhs[j][:, h * 256 : (h + 1) * 256],
                        start=(n == 0),
                        stop=(n == DBLK - 1),
                    )
            # first-finishing half on ACT (slow completion defer tolerable),
            # last half on DVE (58-cycle defer vs ACT's 222 → the store's
            # HWDGE generation starts earlier)
            o = wpool.tile([nu, D], f8, tag="o")
            nc.scalar.activation(
                o[:, :256], ps_a, mybir.ActivationFunctionType.Copy
            )
            nc.vector.tensor_copy(o[:, 256:], ps_b)
            nc.sync.dma_start(out=zc[:, :], in_=o)

    nc.finalize()
    return nc


def _build_module(nk=KBLK, nu=0):
    """v1 fallback module. Two variants by nk:

    - fused (nk <= FUSED_NK_MAX): phase P projects the packed x blocks by Wc
      first (xc = x_sel @ Wc, cheap since only nk blocks), then a single
      accumulation phase A' computes outT = xc_sel^T-chain @ A^T. Phase P
      fills the startup hole while the A^T stream is still arriving, and
      there is no post-phase projection tail.
    - split (nk > FUSED_NK_MAX): big phase A (x^T-chain @ A^T) then a small
      projection phase B by Wc. Cheaper when nk is large because P would
      scale with nk while B is constant.
    """
    import concourse.mybir as mybir
    import concourse.tile as tile
    from concourse import bacc

    f32 = mybir.dt.float32
    bf16 = mybir.dt.bfloat16
    fused = nk <= FUSED_NK_MAX
    # nu > 0: additionally compress A^T to its nu (<=128) distinct columns
    # and expand the result back with a one-hot selection matmul.
    dedup = fused and nu > 0

    nc = bacc.Bacc("TRN2", target_bir_lowering=True)

    if fused:
        # packed x^T: [D, nk*128]; entry [d, i*128 + c] = x_block_i[c, d]
        xtp = nc.dram_tensor("xtp", [D, nk * 128], bf16, kind="ExternalInput")
    else:
        xb = nc.dram_tensor("xb", [nk * 128, D], bf16, kind="ExternalInput")
    if dedup:
        at = nc.dram_tensor("at", [nk * 128, nu], bf16, kind="ExternalInput")
        sel = nc.dram_tensor("sel", [nu, QROWS], bf16, kind="ExternalInput")
    else:
        at = nc.dram_tensor("at", [nk * 128, QROWS], bf16, kind="ExternalInput")
    wc = nc.dram_tensor("wc", [D, D], bf16, kind="ExternalInput")
    xrb = nc.dram_tensor("xrb", [D, QROWS], f32, kind="ExternalInput")
    outT = nc.dram_tensor("outT", [D, QROWS], f32, kind="ExternalOutput")

    with tile.TileContext(nc) as tc:
        with (
            tc.tile_pool(name="const", bufs=1) as cpool,
            tc.tile_pool(name="work", bufs=3) as wpool,
            tc.tile_pool(name="psum", bufs=8 if fused else 4, space="PSUM") as ppool,
        ):
            # PE warm-up: matmuls on a memset tile (no DMA dependency) fill
            # the DMA-latency startup hole and lift the HAM clock gate to
            # 8/8 before the real chains start.
            wu = cpool.tile([128, 128], bf16, tag="wu")
            nc.gpsimd.memset(wu, 0.0)
            ps_w = ppool.tile(
                [128, 512], f32, tag="ps" if fused else "ps2", name="ps_w"
            )
            for _ in range(23):
                nc.tensor.matmul(ps_w[:, :128], wu, wu, start=True, stop=True)
            wu2 = wpool.tile([128, 1], bf16, tag="wu2")
            nc.vector.tensor_copy(wu2, ps_w[:, :1])  # release the bank

            # --- streamed loads ---------------------------------------------
            if fused:
                wc_sb = []
                xtp_sb = []  # x^T tile per d1: [128, nk*128], block i at cols i*128
                for d1 in range(DBLK):
                    t = cpool.tile([128, D], bf16, tag=f"wc{d1}")
                    nc.gpsimd.dma_start(out=t, in_=wc[d1 * 128 : (d1 + 1) * 128, :])
                    wc_sb.append(t)
                    t = cpool.tile([128, nk * 128], bf16, tag=f"xtp{d1}")
                    nc.sync.dma_start(
                        out=t, in_=xtp[d1 * 128 : (d1 + 1) * 128, :]
                    )
                    xtp_sb.append(t)
            else:
                xb_sb = []  # packed x[b] row-block k: [128, D]
                for k in range(nk):
                    t = cpool.tile([128, D], bf16, tag=f"xb{k}")
                    nc.sync.dma_start(out=t, in_=xb[k * 128 : (k + 1) * 128, :])
                    xb_sb.append(t)

            sel_sb = None
            if dedup:
                sel_sb = cpool.tile([nu, QROWS], bf16, tag="sel")
                nc.scalar.dma_start(out=sel_sb, in_=sel[:, :])

            atw = nu if dedup else QROWS
            at_sb = []  # packed A^T row-block k: [128, atw]
            for k in range(nk):
                t = cpool.tile([128, atw], bf16, tag=f"at{k}")
                if fused:
                    # spread the stream over all three DMA queues so it has
                    # fully landed before phase A' consumes it back-to-back
                    eng = (nc.scalar, nc.scalar, nc.sync, nc.gpsimd)[k % 4]
                else:
                    eng = nc.scalar
                eng.dma_start(out=t, in_=at[k * 128 : (k + 1) * 128, :])
                at_sb.append(t)

            if not fused:
                wc_sb = []
                for d1 in range(DBLK):
                    t = cpool.tile([128, D], bf16, tag=f"wc{d1}")
                    nc.sync.dma_start(out=t, in_=wc[d1 * 128 : (d1 + 1) * 128, :])
                    wc_sb.append(t)

            xrb_sb = []  # (x^T + b_out) block d2: [128, QROWS] fp32
            for d2 in range(DBLK):
                t = cpool.tile([128, QROWS], f32, tag=f"xrb{d2}")
                eng = nc.gpsimd if fused else nc.sync
                eng.dma_start(out=t, in_=xrb[d2 * 128 : (d2 + 1) * 128, :])
                xrb_sb.append(t)

            if fused:
                # --- phase P: xc[i] = x_block[i] @ Wc ------------------------
                # d1 outer: paced by the (xtp[d1], wc[d1]) tile arrivals, all
                # nk accumulation groups advance together.
                ps_p = [
                    ppool.tile([128, D], f32, tag="ps", name=f"ps_p{i}")
                    for i in range(nk)
                ]
                for d1 in range(DBLK):
                    for i in range(nk):
                        nc.tensor.matmul(
                            ps_p[i],
                            xtp_sb[d1][:, i * 128 : (i + 1) * 128],
                            wc_sb[d1],
                            start=(d1 == 0),
                            stop=(d1 == DBLK - 1),
                        )
                xc_sb = []
                for i in range(nk):
                    t = wpool.tile([128, D], bf16, tag=f"xc{i % 4}", name=f"xc{i}")
                    if i % 2 == 0:
                        nc.vector.tensor_copy(t, ps_p[i])
                    else:
                        nc.scalar.activation(
                            t, ps_p[i], mybir.ActivationFunctionType.Copy
                        )
                    xc_sb.append(t)

                if dedup:
                    # --- phase A'': zUn[u, d2] = sum_i atU[i]^T @ xc[i] ------
                    ps_u = ppool.tile([nu, D], f32, tag="ps", name="ps_u")
                    for i in range(nk):
                        nc.tensor.matmul(
                            ps_u,
                            at_sb[i],
                            xc_sb[i],
                            start=(i == 0),
                            stop=(i == nk - 1),
                        )
                    zun = []  # per-d2-block [nu, 128] so deps are precise
                    # only d2=0 on DVE: keeps the DVE queue clear for the
                    # 8-add epilogue chain that follows immediately
                    for d2 in range(DBLK):
                        t = wpool.tile([nu, 128], bf16, tag=f"zun{d2}")
                        if d2 == 0:
                            nc.vector.tensor_copy(
                                t, ps_u[:, d2 * 128 : (d2 + 1) * 128]
                            )
                        else:
                            nc.scalar.activation(
                                t,
                                ps_u[:, d2 * 128 : (d2 + 1) * 128],
                                mybir.ActivationFunctionType.Copy,
                            )
                        zun.append(t)

                    # --- expand: outT[d2, s] = zUn-col-d2 ^T @ Sel + xrb -----
                    for d2 in range(DBLK):
                        for h in range(2):
                            hs = slice(h * 512, (h + 1) * 512)
                            ps_e = ppool.tile(
                                [128, 512], f32, tag="ps", name=f"ps_e{d2}_{h}"
                            )
                            nc.tensor.matmul(
                                ps_e,
                                zun[d2],
                                sel_sb[:, hs],
                                start=True,
                                stop=True,
                            )
                            o = wpool.tile(
                                [128, 512], f32, tag=f"osb{h}", name=f"o{d2}_{h}"
                            )
                            nc.vector.tensor_tensor(
                                o,
                                ps_e,
                                xrb_sb[d2][:, hs],
                                mybir.AluOpType.add,
                            )
                            ring = nc.sync if (d2 + h) % 2 == 0 else nc.scalar
                            ring.dma_start(
                                out=outT[d2 * 128 : (d2 + 1) * 128, hs],
                                in_=o,
                            )
                    _done = True
                else:
                    _done = False

                # --- phase A': outT-psum[d2,h] = xc-chain @ A^T --------------
                # group outer: each (d2, h) output group finishes its whole
                # block chain early so its residual-add + store pipeline
                # behind the PE while later groups stream.
                for d2 in range(DBLK) if not _done else []:
                    o = wpool.tile([128, QROWS], f32, tag="osb", name=f"osb{d2}")
                    for h in range(2):
                        hs = slice(h * 512, (h + 1) * 512)
                        ps_o = ppool.tile(
                            [128, 512], f32, tag="ps", name=f"ps_o{d2}_{h}"
                        )
                        for i in range(nk):
                            nc.tensor.matmul(
                                ps_o,
                                xc_sb[i][:, d2 * 128 : (d2 + 1) * 128],
                                at_sb[i][:, h * 512 : (h + 1) * 512],
                                start=(i == 0),
                                stop=(i == nk - 1),
                            )
                        nc.vector.tensor_tensor(
                            o[:, hs],
                            ps_o,
                            xrb_sb[d2][:, hs],
                            mybir.AluOpType.add,
                        )
                        ring = nc.sync if (d2 + h) % 2 == 0 else nc.scalar
                        ring.dma_start(
                            out=outT[d2 * 128 : (d2 + 1) * 128, hs], in_=o[:, hs]
                        )
            else:
                # --- phase A: axT[d] = x-block-col-d ^T @ A^T ----------------
                # k outer / d inner: each at-tile is consumed right after its
                # DMA lands, so the PE never waits on the A^T stream.
                ps_a = [
                    ppool.tile([128, QROWS], f32, tag="ps2", name=f"ps_a{d}")
                    for d in range(DBLK)
                ]
                for k in range(nk):
                    for d in range(DBLK):
                        for h in range(2):
                            nc.tensor.matmul(
                                ps_a[d][:, h * 512 : (h + 1) * 512],
                                xb_sb[k][:, d * 128 : (d + 1) * 128],
                                at_sb[k][:, h * 512 : (h + 1) * 512],
                                start=(k == 0),
                                stop=(k == nk - 1),
                            )
                axT = []
                for d in range(DBLK):
                    t = wpool.tile([128, QROWS], bf16, tag=f"axT{d}")
                    if d % 2 == 0:
                        nc.vector.tensor_copy(t, ps_a[d])
                    else:
                        nc.scalar.activation(
                            t, ps_a[d], mybir.ActivationFunctionType.Copy
                        )
                    axT.append(t)

                # --- phase B: outT[d2] = Wc-chain @ axT + (x^T + b_out) ------
                for d2 in range(DBLK):
                    ps_b = ppool.tile(
                        [128, QROWS], f32, tag="ps2", name=f"ps_b{d2}"
                    )
                    for d1 in range(DBLK):
                        for h in range(2):
                            nc.tensor.matmul(
                                ps_b[:, h * 512 : (h + 1) * 512],
                                wc_sb[d1][:, d2 * 128 : (d2 + 1) * 128],
                                axT[d1][:, h * 512 : (h + 1) * 512],
                                start=(d1 == 0),
                                stop=(d1 == DBLK - 1),
                            )
                    for h in range(2):
                        hs = slice(h * 512, (h + 1) * 512)
                        o = wpool.tile(
                            [128, 512], f32, tag=f"osb{h}", name=f"o{d2}_{h}"
                        )
                        nc.vector.tensor_tensor(
                            o,
                            ps_b[:, hs],
                            xrb_sb[d2][:, hs],
                            mybir.AluOpType.add,
                        )
                        ring = nc.sync if (d2 + h) % 2 == 0 else nc.scalar
                        ring.dma_start(
                            out=outT[d2 * 128 : (d2 + 1) * 128, hs], in_=o
                        )

    nc.finalize()
    return nc


def _get_runner(key):
    """Compile once per module key; return a callable(in_maps) -> out dicts.

    key: ("v1", nk, nu) or ("v2", rblocks_tuple, nu).
    """
    ckey = ("runner", key)
    if ckey in _cache:
        return _cache[ckey]

    import jax
    from jax.sharding import Mesh, PartitionSpec
    from jax.experimental.shard_map import shard_map
    from concourse import bass2jax
    import concourse.mybir as mybir

    bass2jax.install_neuronx_cc_hook()
    if key[0] == "v2":
        nc = _build_module_v2(key[1], key[2])
    else:
        nc = _build_module(key[1], key[2])

    part_name = nc.partition_id_tensor.name if nc.partition_id_tensor else None
    in_names = []
    out_names = []
    out_avals = []
    for alloc in nc.m.functions[0].allocations:
        if not isinstance(alloc, bass2jax.mybir.MemoryLocationSet):
            continue
        name = alloc.memorylocations[0].name
        if alloc.kind == "ExternalInput":
            if name != part_name:
                in_names.append(name)
        elif alloc.kind == "ExternalOutput":
            out_names.append(name)
            out_avals.append(
                jax.core.ShapedArray(
                    tuple(alloc.tensor_shape), mybir.dt.np(alloc.dtype)
                )
            )
    n_params = len(in_names)
    all_names = in_names + out_names
    if part_name is not None:
        all_names = all_names + [part_name]

    def _body(*args):
        operands = list(args)
        if part_name is not None:
            operands.append(bass2jax.partition_id_tensor())
        outs = bass2jax._bass_exec_p.bind(
            *operands,
            out_avals=tuple(out_avals),
            in_names=tuple(all_names),
            out_names=tuple(out_names),
            lowering_input_output_aliases=(),
            sim_require_finite=True,
            sim_require_nnan=True,
            nc=nc,
        )
        return tuple(outs)

    devices = jax.devices()[:NCORES]
    mesh = Mesh(np.asarray(devices), ("core",))
    nin = n_params + len(out_names)
    sharded = jax.jit(
        shard_map(
            _body,
            mesh=mesh,
            in_specs=(PartitionSpec("core"),) * nin,
            out_specs=(PartitionSpec("core"),) * len(out_names),
            check_rep=False,
        ),
        keep_unused=True,
    )

    zero_shapes = [(NCORES * a.shape[0], *a.shape[1:]) for a in out_avals]
    zero_dtypes = [a.dtype for a in out_avals]

    def run(in_maps):
        concat_in = [
            np.concatenate([np.asarray(m[name]) for m in in_maps], axis=0)
            for name in in_names
        ]
        zeros = [np.zeros(s, d) for s, d in zip(zero_shapes, zero_dtypes)]
        out_arrs = sharded(*concat_in, *zeros)
        jax.block_until_ready(out_arrs)
        res = [
            {
                name: np.asarray(out_arrs[i]).reshape(NCORES, *out_avals[i].shape)[c]
                for i, name in enumerate(out_names)
            }
            for c in range(NCORES)
        ]
        return res

    _cache[ckey] = run
    _cache[("sharded", key)] = sharded
    _cache[("meta", key)] = (in_names, out_names, out_avals)
    return run


def _host_prep_v2(x, W_in, W_out, b_out, fusion_weights, routes):
    """Fast-path host prep. Returns None if the routing tables don't have
    enough duplicate structure (falls back to v1), else
    (key, in_maps, epilogue_meta)."""
    x = np.asarray(x, dtype=np.float32)
    W_in = np.asarray(W_in, dtype=np.float32)
    W_out = np.asarray(W_out, dtype=np.float32)
    fw = np.asarray(fusion_weights, dtype=np.float32)
    rt = np.asarray(routes)

    # dedup output rows by exact (routes, weights) byte pattern
    pat = np.concatenate(
        [np.ascontiguousarray(rt).view(np.uint8),
         np.ascontiguousarray(fw).view(np.uint8)],
        axis=1,
    )
    _, uidx, inv = np.unique(pat, axis=0, return_index=True, return_inverse=True)
    inv = inv.ravel()
    n_uni = len(uidx)
    if n_uni > NGRP * 128:
        return None

    rt64 = rt.astype(np.int64)
    # per-unique source rows; group uniques by source-row locality
    srcs = [np.unique(rt64[i]) for i in uidx]
    minrow = np.array([s[0] for s in srcs])
    order = np.argsort(minrow, kind="stable")
    bounds = [round(n_uni * g / NGRP) for g in range(NGRP + 1)]
    groups = []  # (ids, rows, ut)
    numax = nrmax = 0
    for g in range(NGRP):
        ids = order[bounds[g] : bounds[g + 1]]
        if len(ids) == 0:
            ids = order[:1]
        rows = np.unique(np.concatenate([srcs[i] for i in ids]))
        nu_c, nr_c = len(ids), len(rows)
        if nu_c > 128:
            return None
        ut = np.zeros((nr_c, nu_c), np.float32)
        ri = np.searchsorted(rows, rt64[uidx[ids]].ravel())
        uu = np.repeat(np.arange(nu_c), K)
        np.add.at(ut, (ri, uu), fw[uidx[ids]].ravel())
        groups.append((ids, rows, ut))
        numax = max(numax, nu_c)
        nrmax = max(nrmax, nr_c)

    if nrmax > 16 * 128:
        return None

    rblocks = []
    left = nrmax
    while left > 0:
        rblocks.append(min(128, left))
        left -= 128
    rblocks = tuple(rblocks)

    Wc = (W_in @ W_out).astype(_f8)
    # 2-up packing: tile t row r = Wc rows (2t)*128+r (cols 0:512) and
    # (2t+1)*128+r (cols 512:1024)
    wcd = np.zeros((256, 1024), _f8)
    for t in range(2):
        wcd[t * 128 : (t + 1) * 128, :512] = Wc[2 * t * 128 : (2 * t + 1) * 128]
        wcd[t * 128 : (t + 1) * 128, 512:] = Wc[(2 * t + 1) * 128 : (2 * t + 2) * 128]

    # 2-up r-block packing mirroring _build_module_v2
    W = 512 + numax
    dtiles = []
    i = 0
    while i < len(rblocks):
        if i + 1 < len(rblocks):
            dtiles.append((max(rblocks[i], rblocks[i + 1]), [i, i + 1]))
            i += 2
        else:
            dtiles.append((rblocks[i], [i]))
            i += 1

    in_maps = []
    for c in range(NCORES):
        b, g = divmod(c, NGRP)
        ids, rows, ut = groups[g]
        xu = np.zeros((len(rblocks) * 128, W), _f8)
        xu[: len(rows), :512] = x[b][rows].astype(_f8)
        xu[: len(rows), 512 : 512 + ut.shape[1]] = ut.astype(_f8)
        xud = np.zeros((sum(p for p, _ in dtiles), 2 * W), _f8)
        r0 = 0
        for p, blks in dtiles:
            for s, bi in enumerate(blks):
                xud[r0 : r0 + rblocks[bi], s * W : s * W + W] = xu[
                    bi * 128 : bi * 128 + rblocks[bi]
                ]
            r0 += p
        in_maps.append({"xud": xud, "wcd": wcd})

    # epilogue: map each output position s to (group, local unique index)
    gid = np.empty(n_uni, np.int64)
    lix = np.empty(n_uni, np.int64)
    for g in range(NGRP):
        ids = groups[g][0]
        gid[ids] = g
        lix[ids] = np.arange(len(ids))
    # flat index into the per-batch stacked [NGRP*numax, D] result
    flat = gid[inv] * numax + lix[inv]  # [S]

    key = ("v2", rblocks, numax)
    return key, in_maps, (flat, numax)


def _host_prep(x, W_in, W_out, b_out, fusion_weights, routes):
    """v1 host prep. Returns (nk, nu, in_maps). Packs only the nonzero
    128-row source blocks of A^T (and the matching x blocks) per core,
    padded to the max count nk."""
    x = np.asarray(x, dtype=np.float32)
    W_in = np.asarray(W_in, dtype=np.float32)
    W_out = np.asarray(W_out, dtype=np.float32)
    b_out = np.asarray(b_out, dtype=np.float32)
    fw = np.asarray(fusion_weights, dtype=np.float32)
    rt = np.asarray(routes)

    Wc = (W_in @ W_out).astype(_f8)
    xb16 = [x[b].astype(_bf16) for b in range(B)]
    # residual + bias, pre-transposed: [D, QROWS] fp32 per (b, q)
    xrb = [
        [
            np.ascontiguousarray(x[b, q * QROWS : (q + 1) * QROWS].T)
            + b_out[:, None]
            for q in range(4)
        ]
        for b in range(B)
    ]

    # densify A^T per seq-quarter and find its nonzero source blocks
    cols = np.repeat(np.arange(QROWS, dtype=np.int64), K)
    at_q = []
    kset_q = []
    for q in range(4):
        r = rt[q * QROWS : (q + 1) * QROWS].astype(np.int64).ravel()
        a = np.zeros((S, QROWS), np.float32)
        np.add.at(a, (r, cols), fw[q * QROWS : (q + 1) * QROWS].ravel())
        blocks = a.reshape(KBLK, 128, QROWS)
        ks = [k for k in range(KBLK) if np.any(blocks[k])]
        if not ks:
            ks = [0]
        at_q.append(a.astype(_bf16))
        kset_q.append(ks)

    nk = max(len(ks) for ks in kset_q)

    fused = nk <= FUSED_NK_MAX
    # distinct-column compression: for Cantor routing many output positions
    # share identical A^T columns; contract over the unique columns and
    # expand with a one-hot matmul when they all fit in one 128-partition
    # tile.
    nu = 0
    uniq_q = None
    if fused:
        uniq_q = []
        for q in range(4):
            u16 = at_q[q].view(np.uint16)
            uc, inv = np.unique(u16.T, axis=0, return_inverse=True)
            uniq_q.append((uc, inv))
        if max(len(uc) for uc, _ in uniq_q) <= 128:
            nu = 128

    in_maps = []
    for c in range(NCORES):
        b, q = divmod(c, 4)
        ks = kset_q[q]
        if nu:
            uc, inv = uniq_q[q]
            atu_full = np.ascontiguousarray(uc.T).view(_bf16)  # [S, Uq]
            at_p = np.zeros((nk * 128, nu), _bf16)
            for i, k in enumerate(ks):
                at_p[i * 128 : (i + 1) * 128, : uc.shape[0]] = atu_full[
                    k * 128 : (k + 1) * 128
                ]
            sel_p = np.zeros((nu, QROWS), _bf16)
            sel_p[inv, np.arange(QROWS)] = _bf16(1.0)
            m = {"at": at_p, "sel": sel_p, "wc": Wc, "xrb": xrb[b][q]}
        else:
            at_p = np.zeros((nk * 128, QROWS), _bf16)
            for i, k in enumerate(ks):
                at_p[i * 128 : (i + 1) * 128] = at_q[q][k * 128 : (k + 1) * 128]
            m = {"at": at_p, "wc": Wc, "xrb": xrb[b][q]}
        if fused:
            xtp = np.zeros((D, nk * 128), _bf16)
            for i, k in enumerate(ks):
                xtp[:, i * 128 : (i + 1) * 128] = xb16[b][
                    k * 128 : (k + 1) * 128
                ].T
            m["xtp"] = xtp
        else:
            xb_p = np.zeros((nk * 128, D), _bf16)
            for i, k in enumerate(ks):
                xb_p[i * 128 : (i + 1) * 128] = xb16[b][k * 128 : (k + 1) * 128]
            m["xb"] = xb_p
        in_maps.append(m)
    return nk, nu, in_maps


def kernel(x, W_in, W_out, b_out, fusion_weights, routes):
    x = np.asarray(x, dtype=np.float32)
    b_out = np.asarray(b_out, dtype=np.float32)

    prep = _host_prep_v2(x, W_in, W_out, b_out, fusion_weights, routes)
    if prep is not None:
        key, in_maps, (flat, numax) = prep
        run = _get_runner(key)
        res = run(in_maps)
        out = np.empty((B, S, D), np.float32)
        for b in range(B):
            zall = np.concatenate(
                [res[b * NGRP + g]["zc"][:numax] for g in range(NGRP)], axis=0
            ).astype(np.float32)  # [NGRP*numax, D]
            out[b] = x[b] + zall[flat] + b_out
        return out

    nk, nu, in_maps = _host_prep(x, W_in, W_out, b_out, fusion_weights, routes)
    run = _get_runner(("v1", nk, nu))
    res = run(in_maps)
    out = np.empty((B, S, D), np.float32)
    for c in range(NCORES):
        b, q = divmod(c, 4)
        out[b, q * QROWS : (q + 1) * QROWS] = res[c]["outT"].T
    return out
